# revision 21
# baseline (speedup 1.0000x reference)
"""Trainium2 Bass kernel for the binary CNN (CNV/BNN) forward pass.

Strategy
--------
Pure data parallel: batch 128 -> 16 images per NeuronCore x 8 cores.

Math transformations (all exact, validated against the jax reference):
  * sign(htanh(bn(y))) == sign(y + beta) with beta = b/s - m, since the BN
    scale s = g*rsqrt(v+eps) > 0 always.  htanh and the BN multiply never
    need to be materialized on the trunk.
  * sign is monotone, so sign(maxpool(u)) == maxpool(sign(u)); signs are
    computed directly from PSUM (fused bias via the ScalarE Sign activation)
    and pooling runs on +-1 bf16 values.
  * maj3_conv (majority-of-3 popcount conv) uses the identity, exact for all
    zero-padding cases that occur:
        sign(w0x0 + w1x1 + w2x2) = (w0x0 + w1x1 + w2x2 - (w0w1w2)(x0x1x2))/2
    so  maj3_conv(x, w) = 0.5*conv3x3(xb, wb)
                          - 0.5*conv3x1_vertical(Q, W3)
    where Q = horizontal triple products of the padded activations and
    W3[f,c,kh] = prod_kw wb[f,c,kh,kw].  Same identity for fcmaj3.
  * The whole binary trunk runs in bf16: +-1 and +-0.5 are exact, popcount
    partial sums accumulate in fp32 PSUM, and bf16 rounding of (u + beta)
    preserves sign.

Layouts: activations [C(part), N, H, W]; for C=64 layers the batch is split
across partition halves (p = c + 64*nh) and the two halves run as
concurrent tile_position row/col tiles of the 128x128 PE array.
"""

import os
import numpy as np
import ml_dtypes

import concourse.bass as bass
import concourse.tile as tile
from concourse import bacc, mybir
from concourse.bass_utils import run_bass_kernel_spmd
from concourse.masks import make_identity

F32 = mybir.dt.float32
BF16 = mybir.dt.bfloat16
AX = mybir.AxisListType
ALU = mybir.AluOpType
ACT = mybir.ActivationFunctionType

EPS = 1e-5
NCORES = 8
NPC = 16  # images per core

# conv-weight table free-dim offsets inside the packed "wc" tensor
OFF_W1 = 0            # 12 taps x 64
OFF_W2 = 768          # 12 taps x 128
OFF_W3 = 768 + 1536   # 9 taps x 128
OFF_W4 = OFF_W3 + 1152  # (9 taps x 2 mb) x 128
OFF_W5 = OFF_W4 + 2304  # (2 kb x 9 taps x 2 mb) x 128
WC_COLS = OFF_W5 + 4608


def build_nc():
    nc = bacc.Bacc()

    xim_d = nc.dram_tensor("xim", [27, 2, 8, 1024], F32, kind="ExternalInput")
    wt0_d = nc.dram_tensor("wt0", [27, 64], F32, kind="ExternalInput")
    wc_d = nc.dram_tensor("wc", [128, WC_COLS], BF16, kind="ExternalInput")
    wf1_d = nc.dram_tensor("wf1", [32, 128, 512], BF16, kind="ExternalInput")
    wpx_d = nc.dram_tensor("wpx", [11, 128, 512], BF16, kind="ExternalInput")
    wf2_d = nc.dram_tensor("wf2", [4, 128, 512], BF16, kind="ExternalInput")
    wf3_d = nc.dram_tensor("wf3", [4, 128, 1024], BF16, kind="ExternalInput")
    bias_d = nc.dram_tensor("bias", [128, 8], F32, kind="ExternalInput")
    biasf_d = nc.dram_tensor("biasf", [4, 1024], F32, kind="ExternalInput")
    out_d = nc.dram_tensor("out", [16, 1000], F32, kind="ExternalOutput")

    with tile.TileContext(nc) as tc:
        emit(nc, tc, xim_d, wt0_d, wc_d, wf1_d, wpx_d, wf2_d, wf3_d,
             bias_d, biasf_d, out_d)
    nc.compile()
    return nc


def emit(nc, tc, xim_d, wt0_d, wc_d, wf1_d, wpx_d, wf2_d, wf3_d,
         bias_d, biasf_d, out_d):
    from contextlib import ExitStack
    ctx = ExitStack()
    with ctx:
        wpool = ctx.enter_context(tc.tile_pool(name="wpool", bufs=1))
        acts = ctx.enter_context(tc.tile_pool(name="acts", bufs=1))
        ximp = ctx.enter_context(tc.tile_pool(name="ximp", bufs=2))
        qtmp = ctx.enter_context(tc.tile_pool(name="qtmp", bufs=2))
        wstream = ctx.enter_context(tc.tile_pool(name="wstream", bufs=1))
        cps = ctx.enter_context(tc.tile_pool(name="cps", bufs=4, space="PSUM"))
        fps = ctx.enter_context(tc.tile_pool(name="fps", bufs=2, space="PSUM"))
        tps = ctx.enter_context(tc.tile_pool(name="tps", bufs=2, space="PSUM"))

        # ---------------- persistent weights ----------------
        wt0 = wpool.tile([27, 64], F32, name="wt0s")
        nc.sync.dma_start(out=wt0, in_=wt0_d[:])
        wc = wpool.tile([128, WC_COLS], BF16, name="wcs")
        nc.sync.dma_start(out=wc, in_=wc_d[:])
        bias = wpool.tile([128, 8], F32, name="biass")
        nc.sync.dma_start(out=bias, in_=bias_d[:])
        # bias rows broadcast to 16 partitions (DVE can't take step-0 APs)
        # layout [16, 3072]: bf1(512) | bf2(512) | rs(1024) | mrs(1024)
        biasf = wpool.tile([16, 3072], F32, name="biasfs")
        bf_ap = biasf_d[:]

        def _bcast_row(dst, row, n):
            nc.sync.dma_start(out=dst, in_=bass.AP(
                tensor=bf_ap.tensor, offset=bf_ap.offset + row * 1024,
                ap=[[0, 16], [1, n]]))
        _bcast_row(biasf[:, 0:512], 0, 512)
        _bcast_row(biasf[:, 512:1024], 1, 512)
        _bcast_row(biasf[:, 1024:2048], 2, 1024)
        _bcast_row(biasf[:, 2048:3072], 3, 1024)
        ident = wpool.tile([16, 16], F32, name="idents")
        make_identity(nc, ident)

        # ---------------- persistent activation buffers ----------------
        xpad1 = acts.tile([128, 8, 34, 34], BF16, name="xpad1")
        q1 = acts.tile([128, 8, 34, 32], BF16, name="q1")
        s1 = acts.tile([128, 8, 32, 32], BF16, name="s1")
        p1h = acts.tile([128, 8, 16, 32], BF16, name="p1h")
        xpad2 = acts.tile([128, 8, 18, 18], BF16, name="xpad2")
        q2 = acts.tile([128, 8, 18, 16], BF16, name="q2")
        xpad3 = acts.tile([128, 16, 18, 18], BF16, name="xpad3")
        s3 = acts.tile([128, 16, 16, 16], BF16, name="s3")
        p3h = acts.tile([128, 16, 8, 16], BF16, name="p3h")
        xpad4 = acts.tile([128, 16, 10, 10], BF16, name="xpad4")
        a5 = [acts.tile([128, 16, 10, 10], BF16, name=f"a5_{i}") for i in (0, 1)]
        s5 = [acts.tile([128, 16, 8, 8], BF16, name=f"s5_{i}") for i in (0, 1)]
        p5h = [acts.tile([128, 16, 4, 8], BF16, name=f"p5h_{i}") for i in (0, 1)]
        a6 = [acts.tile([128, 16, 16], BF16, name=f"a6_{i}") for i in (0, 1)]

        xbuf = acts.tile([16, 4098], BF16, name="xbuf")
        pxt1 = acts.tile([16, 1366], BF16, name="pxt1")
        px = acts.tile([16, 1408], F32, name="px")
        pxt = acts.tile([128, 176], BF16, name="pxt")
        af2f = acts.tile([16, 512], F32, name="af2f")
        af3f = acts.tile([16, 512], F32, name="af3f")
        af2t = acts.tile([128, 64], BF16, name="af2t")
        af3t = acts.tile([128, 64], BF16, name="af3t")
        yb = acts.tile([16, 1024], F32, name="yb")
        eb = acts.tile([16, 1000], F32, name="eb")
        mx = acts.tile([16, 1], F32, name="mx")
        negmx = acts.tile([16, 1], F32, name="negmx")
        sm = acts.tile([16, 1], F32, name="sm")
        lse = acts.tile([16, 1], F32, name="lse")
        outsb = acts.tile([16, 1000], F32, name="outsb")

        # zero the zero-padding halos (interiors are overwritten by drains)
        nc.gpsimd.memset(xpad1, 0.0)
        nc.gpsimd.memset(xpad2, 0.0)
        nc.gpsimd.memset(xpad3, 0.0)
        nc.gpsimd.memset(xpad4, 0.0)
        nc.gpsimd.memset(a5[0], 0.0)
        nc.gpsimd.memset(a5[1], 0.0)
        nc.vector.memset(xbuf[:, 0:1], -1.0)
        nc.vector.memset(xbuf[:, 4097:4098], -1.0)
        nc.vector.memset(px[:, 1366:1408], 0.0)

        # ================ L0: conv 3->64 on real-valued x ================
        # K=27 im2col rows; two batch halves run as concurrent col tiles.
        for n8 in range(8):
            for hh in range(2):
                ximt = ximp.tile([27, 2, 512], F32, name="ximt")
                nc.sync.dma_start(out=ximt, in_=xim_d[:][:, :, n8, hh * 512:(hh + 1) * 512])
                ps = cps.tile([128, 512], F32, name="ps0", tag="cpsum")
                nc.tensor.matmul(out=ps[0:64], lhsT=wt0, rhs=ximt[:, 0],
                                 start=True, stop=True, tile_position=(0, 0))
                nc.tensor.matmul(out=ps[64:128], lhsT=wt0, rhs=ximt[:, 1],
                                 start=True, stop=True, tile_position=(0, 64),
                                 skip_group_check=True)
                psv = ps.rearrange("p (h w) -> p h w", h=16)
                nc.scalar.activation(
                    out=xpad1[:, n8, 1 + hh * 16:17 + hh * 16, 1:33],
                    in_=psv, func=ACT.Sign, bias=bias[:, 0:1])

        # Q1 = horizontal triple products of xpad1 (zeros propagate)
        for n8 in range(8):
            qt = qtmp.tile([128, 34, 32], BF16, name="qt1", tag="qt1")
            nc.vector.tensor_mul(qt, xpad1[:, n8, :, 0:32], xpad1[:, n8, :, 1:33])
            nc.vector.tensor_mul(q1[:, n8], qt, xpad1[:, n8, :, 2:34])

        # ================ L1: maj3 64->64 @32x32 + pool ================
        # 12 taps (9 conv + 3 Q); batch halves as diagonal quadrants.
        for n8 in range(8):
            for hh in range(2):
                ps = cps.tile([128, 512], F32, name="ps1", tag="cpsum")
                for t in range(12):
                    for nh in (0, 1):
                        p0 = 64 * nh
                        if t < 9:
                            kh, kw = t // 3, t % 3
                            rhs = xpad1[p0:p0 + 64, n8,
                                        kh + hh * 16:kh + hh * 16 + 16, kw:kw + 32]
                        else:
                            kh = t - 9
                            rhs = q1[p0:p0 + 64, n8,
                                     kh + hh * 16:kh + hh * 16 + 16, :]
                        nc.tensor.matmul(
                            out=ps[p0:p0 + 64], lhsT=wc[p0:p0 + 64, OFF_W1 + t * 64:OFF_W1 + (t + 1) * 64],
                            rhs=rhs, start=(t == 0), stop=(t == 11),
                            tile_position=(p0, p0),
                            skip_group_check=(nh == 1))
                psv = ps.rearrange("p (h w) -> p h w", h=16)
                nc.scalar.activation(out=s1[:, n8, hh * 16:hh * 16 + 16, :],
                                     in_=psv, func=ACT.Sign, bias=bias[:, 1:2])

        # maxpool 2x2 on signs -> write interior of xpad2
        s1r = s1.rearrange("p n (h2 pr) w -> p n h2 pr w", pr=2)
        nc.vector.tensor_max(p1h, s1r[:, :, :, 0, :], s1r[:, :, :, 1, :])
        p1r = p1h.rearrange("p n h (w2 pr) -> p n h w2 pr", pr=2)
        nc.vector.tensor_max(xpad2[:, :, 1:17, 1:17],
                             p1r[:, :, :, :, 0], p1r[:, :, :, :, 1])

        # Q2
        for n8 in range(8):
            qt = qtmp.tile([128, 18, 16], BF16, name="qt2", tag="qt2")
            nc.vector.tensor_mul(qt, xpad2[:, n8, :, 0:16], xpad2[:, n8, :, 1:17])
            nc.vector.tensor_mul(q2[:, n8], qt, xpad2[:, n8, :, 2:18])

        # ================ L2: maj3 64->128 @16x16 ================
        # batch halves as row tiles -> two separate PSUM banks.
        for j in range(4):
            psA = cps.tile([128, 512], F32, name="ps2a", tag="cpsum")
            psB = cps.tile([128, 512], F32, name="ps2b", tag="cpsum")
            for t in range(12):
                for nh, pst in ((0, psA), (1, psB)):
                    p0 = 64 * nh
                    if t < 9:
                        kh, kw = t // 3, t % 3
                        rhs = xpad2[p0:p0 + 64, 2 * j:2 * j + 2,
                                    kh:kh + 16, kw:kw + 16]
                    else:
                        kh = t - 9
                        rhs = q2[p0:p0 + 64, 2 * j:2 * j + 2, kh:kh + 16, :]
                    nc.tensor.matmul(
                        out=pst, lhsT=wc[p0:p0 + 64, OFF_W2 + t * 128:OFF_W2 + (t + 1) * 128],
                        rhs=rhs, start=(t == 0), stop=(t == 11),
                        tile_position=(p0, 0))
            for nh, pst in ((0, psA), (1, psB)):
                psv = pst.rearrange("p (n h w) -> p n h w", n=2, h=16)
                nc.scalar.activation(
                    out=xpad3[:, 8 * nh + 2 * j:8 * nh + 2 * j + 2, 1:17, 1:17],
                    in_=psv, func=ACT.Sign, bias=bias[:, 2:3])

        # ================ L3: bin_conv 128->128 @16x16 + pool ================
        for j in range(8):
            ps = cps.tile([128, 512], F32, name="ps3", tag="cpsum")
            for t in range(9):
                kh, kw = t // 3, t % 3
                nc.tensor.matmul(
                    out=ps, lhsT=wc[:, OFF_W3 + t * 128:OFF_W3 + (t + 1) * 128],
                    rhs=xpad3[:, 2 * j:2 * j + 2, kh:kh + 16, kw:kw + 16],
                    start=(t == 0), stop=(t == 8))
            psv = ps.rearrange("p (n h w) -> p n h w", n=2, h=16)
            nc.scalar.activation(out=s3[:, 2 * j:2 * j + 2], in_=psv,
                                 func=ACT.Sign, bias=bias[:, 3:4])

        s3r = s3.rearrange("p n (h2 pr) w -> p n h2 pr w", pr=2)
        nc.vector.tensor_max(p3h, s3r[:, :, :, 0, :], s3r[:, :, :, 1, :])
        p3r = p3h.rearrange("p n h (w2 pr) -> p n h w2 pr", pr=2)
        nc.vector.tensor_max(xpad4[:, :, 1:9, 1:9],
                             p3r[:, :, :, :, 0], p3r[:, :, :, :, 1])

        # ================ L4: bin_conv 128->256 @8x8 ================
        for mb in range(2):
            for c8 in range(2):
                ps = cps.tile([128, 512], F32, name="ps4", tag="cpsum")
                for t in range(9):
                    kh, kw = t // 3, t % 3
                    nc.tensor.matmul(
                        out=ps,
                        lhsT=wc[:, OFF_W4 + (t * 2 + mb) * 128:OFF_W4 + (t * 2 + mb + 1) * 128],
                        rhs=xpad4[:, c8 * 8:c8 * 8 + 8, kh:kh + 8, kw:kw + 8],
                        start=(t == 0), stop=(t == 8))
                psv = ps.rearrange("p (n h w) -> p n h w", n=8, h=8)
                nc.scalar.activation(
                    out=a5[mb][:, c8 * 8:c8 * 8 + 8, 1:9, 1:9], in_=psv,
                    func=ACT.Sign, bias=bias[:, 4 + mb:5 + mb])

        # ================ L5: bin_conv 256->256 @8x8 + pool ================
        for mb in range(2):
            for c8 in range(2):
                ps = cps.tile([128, 512], F32, name="ps5", tag="cpsum")
                for kb in range(2):
                    for t in range(9):
                        kh, kw = t // 3, t % 3
                        nc.tensor.matmul(
                            out=ps,
                            lhsT=wc[:, OFF_W5 + ((kb * 9 + t) * 2 + mb) * 128:
                                    OFF_W5 + ((kb * 9 + t) * 2 + mb + 1) * 128],
                            rhs=a5[kb][:, c8 * 8:c8 * 8 + 8, kh:kh + 8, kw:kw + 8],
                            start=(kb == 0 and t == 0), stop=(kb == 1 and t == 8))
                psv = ps.rearrange("p (n h w) -> p n h w", n=8, h=8)
                nc.scalar.activation(out=s5[mb][:, c8 * 8:c8 * 8 + 8], in_=psv,
                                     func=ACT.Sign, bias=bias[:, 6 + mb:7 + mb])
        for mb in range(2):
            s5r = s5[mb].rearrange("p n (h2 pr) w -> p n h2 pr w", pr=2)
            nc.vector.tensor_max(p5h[mb], s5r[:, :, :, 0, :], s5r[:, :, :, 1, :])
            p5r = p5h[mb].rearrange("p n h (w2 pr) -> p n h w2 pr", pr=2)
            a6v = a6[mb].rearrange("p n (h w) -> p n h w", h=4)
            nc.vector.tensor_max(a6v, p5r[:, :, :, :, 0], p5r[:, :, :, :, 1])

        # ================ FC head ================
        # gather a6 -> X[n, d] rows (d = c*16 + hw), with -1 pads at both ends
        for cb in range(2):
            for n in range(16):
                nc.sync.dma_start(
                    out=xbuf[n:n + 1, 1 + cb * 2048:1 + (cb + 1) * 2048],
                    in_=a6[cb][:, n, :])

        # Px = triple products of consecutive padded features
        xr3 = xbuf.rearrange("p (g k) -> p g k", k=3)
        nc.vector.tensor_mul(pxt1, xr3[:, :, 0], xr3[:, :, 1])
        nc.vector.tensor_mul(px[:, 0:1366], pxt1, xr3[:, :, 2])

        # transpose Px -> [g, n] blocks for use as matmul lhsT
        for gb in range(11):
            txp = tps.tile([128, 16], F32, name="txp", tag="tx")
            nc.tensor.transpose(txp, px[:, gb * 128:(gb + 1) * 128], ident)
            nc.scalar.copy(pxt[:, gb * 16:(gb + 1) * 16], txp)

        # FC1: out[n, f] accumulated over 32 feature K-groups + 11 Px groups
        psf = fps.tile([16, 512], F32, name="psf1", tag="fc")
        nmm = 43
        i = 0
        for kg in range(32):
            cb, hw = kg // 16, kg % 16
            wt = wstream.tile([128, 512], BF16, name="wf1t", tag="w512", bufs=8)
            nc.sync.dma_start(out=wt, in_=wf1_d[:][kg])
            nc.tensor.matmul(out=psf, lhsT=a6[cb][:, :, hw], rhs=wt,
                             start=(i == 0), stop=(i == nmm - 1))
            i += 1
        for gb in range(11):
            wt = wstream.tile([128, 512], BF16, name="wpxt", tag="w512", bufs=8)
            nc.sync.dma_start(out=wt, in_=wpx_d[:][gb])
            nc.tensor.matmul(out=psf, lhsT=pxt[:, gb * 16:(gb + 1) * 16], rhs=wt,
                             start=(i == 0), stop=(i == nmm - 1))
            i += 1
        # bias + sign (beta_f1 along the free dim -> DVE add then ACT sign)
        nc.vector.tensor_add(af2f, psf, biasf[:, 0:512])
        nc.scalar.activation(out=af2f, in_=af2f, func=ACT.Sign, bias=0.0)
        for fb in range(4):
            txp = tps.tile([128, 16], F32, name="txa", tag="tx")
            nc.tensor.transpose(txp, af2f[:, fb * 128:(fb + 1) * 128], ident)
            nc.scalar.copy(af2t[:, fb * 16:(fb + 1) * 16], txp)

        # FC2
        psf2 = fps.tile([16, 512], F32, name="psf2", tag="fc")
        for kb in range(4):
            wt = wstream.tile([128, 512], BF16, name="wf2t", tag="w512", bufs=8)
            nc.sync.dma_start(out=wt, in_=wf2_d[:][kb])
            nc.tensor.matmul(out=psf2, lhsT=af2t[:, kb * 16:(kb + 1) * 16], rhs=wt,
                             start=(kb == 0), stop=(kb == 3))
        nc.vector.tensor_add(af3f, psf2, biasf[:, 512:1024])
        nc.scalar.activation(out=af3f, in_=af3f, func=ACT.Sign, bias=0.0)
        for fb in range(4):
            txp = tps.tile([128, 16], F32, name="txb", tag="tx")
            nc.tensor.transpose(txp, af3f[:, fb * 128:(fb + 1) * 128], ident)
            nc.scalar.copy(af3t[:, fb * 16:(fb + 1) * 16], txp)

        # FC3 (+ affine-only BN)
        psh = [fps.tile([16, 512], F32, name=f"psh{i}", tag="fc") for i in (0, 1)]
        for kb in range(4):
            wt = wstream.tile([128, 1024], BF16, name="wf3t", tag="w1024", bufs=2)
            nc.sync.dma_start(out=wt, in_=wf3_d[:][kb])
            for half in range(2):
                nc.tensor.matmul(out=psh[half], lhsT=af3t[:, kb * 16:(kb + 1) * 16],
                                 rhs=wt[:, half * 512:(half + 1) * 512],
                                 start=(kb == 0), stop=(kb == 3))
        for half in range(2):
            nc.vector.tensor_mul(yb[:, half * 512:(half + 1) * 512], psh[half],
                                 biasf[:, 1024 + half * 512:1024 + (half + 1) * 512])
        nc.vector.tensor_add(yb[:, 0:1000], yb[:, 0:1000], biasf[:, 2048:3048])

        # log-softmax over classes
        nc.vector.tensor_reduce(out=mx, in_=yb[:, 0:1000], axis=AX.X, op=ALU.max)
        nc.vector.tensor_scalar_mul(negmx, mx, -1.0)
        nc.scalar.activation(out=eb, in_=yb[:, 0:1000], func=ACT.Exp, bias=negmx)
        nc.vector.tensor_reduce(out=sm, in_=eb, axis=AX.X, op=ALU.add)
        nc.scalar.activation(out=lse, in_=sm, func=ACT.Ln, bias=0.0)
        nc.vector.tensor_scalar(out=outsb, in0=yb[:, 0:1000], scalar1=mx,
                                scalar2=lse, op0=ALU.subtract, op1=ALU.subtract)
        nc.sync.dma_start(out=out_d[:], in_=outsb)


# ======================= host-side preparation =======================

def _sgn(a):
    return np.sign(np.asarray(a, np.float32)).astype(np.float32)


def _beta(bn):
    g, b, m, v = [np.asarray(t, np.float32) for t in bn]
    s = g / np.sqrt(v + EPS)
    return (b / s - m).astype(np.float32)


def prepare_weights(params):
    p = params
    w1b = _sgn(p['w1']); w2b = _sgn(p['w2']); w3b = _sgn(p['w3'])
    w4b = _sgn(p['w4']); w5b = _sgn(p['w5'])
    f1b = _sgn(p['fc1']); f2b = _sgn(p['fc2']); f3b = _sgn(p['fc3'])

    wt0 = np.zeros([27, 64], np.float32)
    w0b = _sgn(p['w0'])
    for c in range(3):
        for kh in range(3):
            for kw in range(3):
                wt0[c * 9 + kh * 3 + kw] = w0b[:, c, kh, kw]

    wc = np.zeros([128, WC_COLS], np.float32)

    def fill_maj(off, wb, F):
        W3 = wb[:, :, :, 0] * wb[:, :, :, 1] * wb[:, :, :, 2]
        for t in range(12):
            if t < 9:
                kh, kw = t // 3, t % 3
                blk = 0.5 * wb[:, :, kh, kw].T      # [c, F]
            else:
                blk = -0.5 * W3[:, :, t - 9].T
            wc[0:64, off + t * F: off + (t + 1) * F] = blk
            wc[64:128, off + t * F: off + (t + 1) * F] = blk

    fill_maj(OFF_W1, w1b, 64)
    fill_maj(OFF_W2, w2b, 128)
    for t in range(9):
        kh, kw = t // 3, t % 3
        wc[:, OFF_W3 + t * 128: OFF_W3 + (t + 1) * 128] = w3b[:, :, kh, kw].T
        for mb in range(2):
            wc[:, OFF_W4 + (t * 2 + mb) * 128: OFF_W4 + (t * 2 + mb + 1) * 128] = \
                w4b[mb * 128:(mb + 1) * 128, :, kh, kw].T
            for kb in range(2):
                wc[:, OFF_W5 + ((kb * 9 + t) * 2 + mb) * 128:
                   OFF_W5 + ((kb * 9 + t) * 2 + mb + 1) * 128] = \
                    w5b[mb * 128:(mb + 1) * 128, kb * 128:(kb + 1) * 128, kh, kw].T

    # FC1 main: wf1[kg=(cb*16+hw)][c_local, f] = 0.5*f1b[f, (cb*128+c)*16+hw+1]
    core = f1b[:, 1:4097].reshape(512, 256, 16)           # [f, c, hw]
    wf1 = np.zeros([32, 128, 512], np.float32)
    for cb in range(2):
        for hw in range(16):
            wf1[cb * 16 + hw] = 0.5 * core[:, cb * 128:(cb + 1) * 128, hw].T
    # FC1 Px term
    W3f = f1b[:, 0::3] * f1b[:, 1::3] * f1b[:, 2::3]      # [512, 1366]
    wpx = np.zeros([11, 128, 512], np.float32)
    W3p = np.zeros([512, 1408], np.float32)
    W3p[:, 0:1366] = -0.5 * W3f
    for gb in range(11):
        wpx[gb] = W3p[:, gb * 128:(gb + 1) * 128].T
    wf2 = np.zeros([4, 128, 512], np.float32)
    for kb in range(4):
        wf2[kb] = f2b[:, kb * 128:(kb + 1) * 128].T
    f3p = np.zeros([1024, 512], np.float32)
    f3p[0:1000] = f3b
    wf3 = np.zeros([4, 128, 1024], np.float32)
    for kb in range(4):
        wf3[kb] = f3p[:, kb * 128:(kb + 1) * 128].T

    bias = np.zeros([128, 8], np.float32)
    b0 = _beta(p['bn0']); b1 = _beta(p['bn1'])
    bias[:, 0] = np.concatenate([b0, b0]); bias[:, 1] = np.concatenate([b1, b1])
    bias[:, 2] = _beta(p['bn2']); bias[:, 3] = _beta(p['bn3'])
    b4 = _beta(p['bn4']); b5 = _beta(p['bn5'])
    bias[:, 4] = b4[0:128]; bias[:, 5] = b4[128:256]
    bias[:, 6] = b5[0:128]; bias[:, 7] = b5[128:256]

    biasf = np.zeros([4, 1024], np.float32)
    bf1 = _beta(p['bnf1']) - 0.5 * (f1b[:, 0] + f1b[:, 4097])
    biasf[0, 0:512] = bf1
    biasf[1, 0:512] = _beta(p['bnf2'])
    m3, v3 = [np.asarray(t, np.float32) for t in p['bnf3']]
    rs = 1.0 / np.sqrt(v3 + EPS)
    biasf[2, 0:1000] = rs
    biasf[3, 0:1000] = -m3 * rs

    bf = ml_dtypes.bfloat16
    return {
        "wt0": wt0, "wc": wc.astype(bf), "wf1": wf1.astype(bf),
        "wpx": wpx.astype(bf), "wf2": wf2.astype(bf), "wf3": wf3.astype(bf),
        "bias": bias, "biasf": biasf,
    }


def prepare_xim(xc):
    """Per-core im2col: xc [16, 3, 32, 32] -> [27, 2, 8, 1024] fp32."""
    xp = np.pad(np.asarray(xc, np.float32), ((0, 0), (0, 0), (1, 1), (1, 1)))
    xim = np.empty([27, 16, 1024], np.float32)
    for c in range(3):
        for kh in range(3):
            for kw in range(3):
                xim[c * 9 + kh * 3 + kw] = \
                    xp[:, c, kh:kh + 32, kw:kw + 32].reshape(16, 1024)
    return xim.reshape(27, 2, 8, 1024)


def make_in_maps(x, params):
    shared = prepare_weights(params)
    x = np.asarray(x, np.float32)
    in_maps = []
    for ci in range(NCORES):
        m = dict(shared)
        m["xim"] = prepare_xim(x[ci * NPC:(ci + 1) * NPC])
        in_maps.append(m)
    return in_maps


def kernel(x, params):
    in_maps = make_in_maps(x, params)
    nc = build_nc()
    res = run_bass_kernel_spmd(nc, in_maps, core_ids=list(range(NCORES)))
    out = np.concatenate([np.asarray(res.results[i]["out"], np.float32)
                          for i in range(NCORES)], axis=0)
    return out


# revision 30
# speedup vs baseline: 1.0857x; 1.0857x over previous
"""Trainium2 Bass kernel for the binary CNN (CNV/BNN) forward pass.

Strategy
--------
Pure data parallel: batch 128 -> 16 images per NeuronCore x 8 cores.

Math transformations (all exact, validated against the jax reference):
  * sign(htanh(bn(y))) == sign(y + beta) with beta = b/s - m, since the BN
    scale s = g*rsqrt(v+eps) > 0 always.  htanh and the BN multiply never
    need to be materialized on the trunk.
  * sign is monotone, so sign(maxpool(u)) == maxpool(sign(u)); signs are
    computed directly from PSUM (fused bias via the ScalarE Sign activation)
    and pooling runs on +-1 bf16 values.
  * maj3_conv (majority-of-3 popcount conv) uses the identity, exact for all
    zero-padding cases that occur:
        sign(w0x0 + w1x1 + w2x2) = (w0x0 + w1x1 + w2x2 - (w0w1w2)(x0x1x2))/2
    so  maj3_conv(x, w) = 0.5*conv3x3(xb, wb)
                          - 0.5*conv3x1_vertical(Q, W3)
    where Q = horizontal triple products of the padded activations and
    W3[f,c,kh] = prod_kw wb[f,c,kh,kw].  Same identity for fcmaj3.
  * The whole binary trunk runs in bf16: +-1 and +-0.5 are exact, popcount
    partial sums accumulate in fp32 PSUM, and bf16 rounding of (u + beta)
    preserves sign.

Layouts: activations [C(part), N, H, W]; for C=64 layers the batch is split
across partition halves (p = c + 64*nh) and the two halves run as
concurrent tile_position row/col tiles of the 128x128 PE array.
"""

import os
import numpy as np
import ml_dtypes

import concourse.bass as bass
import concourse.tile as tile
from concourse import bacc, mybir
from concourse.bass_utils import run_bass_kernel_spmd
from concourse.masks import make_identity

F32 = mybir.dt.float32
BF16 = mybir.dt.bfloat16
FP8 = mybir.dt.float8e4
AX = mybir.AxisListType
ALU = mybir.AluOpType
ACT = mybir.ActivationFunctionType

EPS = 1e-5
NCORES = 8
NPC = 16  # images per core

# conv-weight table free-dim offsets inside the packed "wc" tensor
OFF_W1 = 0            # 12 taps x 64
OFF_W2 = 768          # 12 taps x 128
OFF_W3 = 768 + 1536   # 9 taps x 128
OFF_W4 = OFF_W3 + 1152  # (9 taps x 2 mb) x 128
OFF_W5 = OFF_W4 + 2304  # (2 kb x 9 taps x 2 mb) x 128
WC_COLS = OFF_W5 + 4608


def build_nc():
    nc = bacc.Bacc()

    xim_d = nc.dram_tensor("xim", [27, 2, 8, 1024], F32, kind="ExternalInput")
    wt0_d = nc.dram_tensor("wt0", [27, 64], F32, kind="ExternalInput")
    wc_d = nc.dram_tensor("wc", [128, WC_COLS], BF16, kind="ExternalInput")
    wf1_d = nc.dram_tensor("wf1", [32, 128, 512], FP8, kind="ExternalInput")
    wpx_d = nc.dram_tensor("wpx", [11, 128, 512], FP8, kind="ExternalInput")
    wf2_d = nc.dram_tensor("wf2", [4, 128, 512], FP8, kind="ExternalInput")
    wf3_d = nc.dram_tensor("wf3", [4, 128, 1024], FP8, kind="ExternalInput")
    bias_d = nc.dram_tensor("bias", [128, 8], F32, kind="ExternalInput")
    biasf_d = nc.dram_tensor("biasf", [4, 1024], F32, kind="ExternalInput")
    out_d = nc.dram_tensor("out", [16, 1000], F32, kind="ExternalOutput")

    with tile.TileContext(nc) as tc:
        emit(nc, tc, xim_d, wt0_d, wc_d, wf1_d, wpx_d, wf2_d, wf3_d,
             bias_d, biasf_d, out_d)
    nc.compile()
    return nc


def emit(nc, tc, xim_d, wt0_d, wc_d, wf1_d, wpx_d, wf2_d, wf3_d,
         bias_d, biasf_d, out_d):
    from contextlib import ExitStack
    ctx = ExitStack()
    with ctx:
        wpool = ctx.enter_context(tc.tile_pool(name="wpool", bufs=1))
        acts = ctx.enter_context(tc.tile_pool(name="acts", bufs=1))
        ximp = ctx.enter_context(tc.tile_pool(name="ximp", bufs=3))
        qtmp = ctx.enter_context(tc.tile_pool(name="qtmp", bufs=2))
        wstream = ctx.enter_context(tc.tile_pool(name="wstream", bufs=1))
        cps = ctx.enter_context(tc.tile_pool(name="cps", bufs=4, space="PSUM"))
        fps = ctx.enter_context(tc.tile_pool(name="fps", bufs=2, space="PSUM"))
        tps = ctx.enter_context(tc.tile_pool(name="tps", bufs=2, space="PSUM"))

        # ---------------- persistent weights ----------------
        wt0 = wpool.tile([27, 64], F32, name="wt0s")
        nc.sync.dma_start(out=wt0, in_=wt0_d[:])
        wc = wpool.tile([128, WC_COLS], BF16, name="wcs")
        nc.sync.dma_start(out=wc, in_=wc_d[:])
        bias = wpool.tile([128, 8], F32, name="biass")
        nc.sync.dma_start(out=bias, in_=bias_d[:])
        # bias rows broadcast to 16 partitions (DVE can't take step-0 APs)
        # layout [16, 3072]: bf1(512) | bf2(512) | rs(1024) | mrs(1024)
        biasf = wpool.tile([16, 3072], F32, name="biasfs")
        bf_ap = biasf_d[:]

        def _bcast_row(dst, row, n):
            nc.sync.dma_start(out=dst, in_=bass.AP(
                tensor=bf_ap.tensor, offset=bf_ap.offset + row * 1024,
                ap=[[0, 16], [1, n]]))
        _bcast_row(biasf[:, 0:512], 0, 512)
        _bcast_row(biasf[:, 512:1024], 1, 512)
        _bcast_row(biasf[:, 1024:2048], 2, 1024)
        _bcast_row(biasf[:, 2048:3072], 3, 1024)
        ident = wpool.tile([16, 16], F32, name="idents")
        make_identity(nc, ident)

        # ---------------- persistent activation buffers ----------------
        xpad1 = acts.tile([128, 8, 34, 34], BF16, name="xpad1")
        q1 = acts.tile([128, 8, 34, 32], BF16, name="q1")
        s1 = acts.tile([128, 8, 32, 32], BF16, name="s1")
        p1h = acts.tile([128, 8, 16, 32], BF16, name="p1h")
        xpad2 = acts.tile([128, 8, 18, 18], BF16, name="xpad2")
        q2 = acts.tile([128, 8, 18, 16], BF16, name="q2")
        xpad3 = acts.tile([128, 16, 18, 18], BF16, name="xpad3")
        s3 = acts.tile([128, 16, 16, 16], BF16, name="s3")
        p3h = acts.tile([128, 16, 8, 16], BF16, name="p3h")
        xpad4 = acts.tile([128, 16, 10, 10], BF16, name="xpad4")
        a5 = [acts.tile([128, 16, 10, 10], BF16, name=f"a5_{i}") for i in (0, 1)]
        s5 = [acts.tile([128, 16, 8, 8], BF16, name=f"s5_{i}") for i in (0, 1)]
        p5h = [acts.tile([128, 16, 4, 8], BF16, name=f"p5h_{i}") for i in (0, 1)]
        a6 = [acts.tile([128, 16, 16], BF16, name=f"a6_{i}") for i in (0, 1)]

        xbuf = acts.tile([16, 4098], BF16, name="xbuf")
        pxt1 = acts.tile([16, 1366], BF16, name="pxt1")
        px = acts.tile([16, 1408], F32, name="px")
        pxt = acts.tile([128, 176], BF16, name="pxt")
        af2f = acts.tile([16, 512], F32, name="af2f")
        af3f = acts.tile([16, 512], F32, name="af3f")
        af2t = acts.tile([128, 64], BF16, name="af2t")
        af3t = acts.tile([128, 64], BF16, name="af3t")
        yb = acts.tile([16, 1024], F32, name="yb")
        eb = acts.tile([16, 1000], F32, name="eb")
        mx = acts.tile([16, 1], F32, name="mx")
        negmx = acts.tile([16, 1], F32, name="negmx")
        sm = acts.tile([16, 1], F32, name="sm")
        lse = acts.tile([16, 1], F32, name="lse")
        outsb = acts.tile([16, 1000], F32, name="outsb")

        # zero only the padding halos (interiors are overwritten by drains)
        def halo_memset(eng, buf, H, W):
            eng.memset(buf[:, :, 0, :], 0.0)          # top row
            eng.memset(buf[:, :, H - 1, :], 0.0)      # bottom row
            eng.memset(buf[:, :, 1:H - 1, 0], 0.0)    # left col
            eng.memset(buf[:, :, 1:H - 1, W - 1], 0.0)  # right col
        halo_memset(nc.gpsimd, xpad1, 34, 34)
        halo_memset(nc.vector, xpad2, 18, 18)
        halo_memset(nc.gpsimd, xpad3, 18, 18)
        halo_memset(nc.vector, xpad4, 10, 10)
        halo_memset(nc.gpsimd, a5[0], 10, 10)
        halo_memset(nc.gpsimd, a5[1], 10, 10)
        nc.vector.memset(xbuf[:, 0:1], -1.0)
        nc.vector.memset(xbuf[:, 4097:4098], -1.0)
        nc.vector.memset(px[:, 1366:1408], 0.0)

        # ================ L0: conv 3->64 on real-valued x ================
        # K=27 im2col rows; two batch halves run as concurrent col tiles.
        for n8 in range(8):
            for hh in range(2):
                ximt = ximp.tile([27, 2, 512], F32, name="ximt")
                nc.sync.dma_start(out=ximt, in_=xim_d[:][:, :, n8, hh * 512:(hh + 1) * 512])
                ps = cps.tile([128, 512], F32, name="ps0", tag="cpsum")
                nc.tensor.matmul(out=ps[0:64], lhsT=wt0, rhs=ximt[:, 0],
                                 start=True, stop=True, tile_position=(0, 0))
                nc.tensor.matmul(out=ps[64:128], lhsT=wt0, rhs=ximt[:, 1],
                                 start=True, stop=True, tile_position=(0, 64),
                                 skip_group_check=True)
                psv = ps.rearrange("p (h w) -> p h w", h=16)
                nc.scalar.activation(
                    out=xpad1[:, n8, 1 + hh * 16:17 + hh * 16, 1:33],
                    in_=psv, func=ACT.Sign, bias=bias[:, 0:1])

        # Q1 = horizontal triple products of xpad1 (zeros propagate)
        for n8 in range(8):
            qt = qtmp.tile([128, 34, 32], BF16, name="qt1", tag="qt1")
            nc.vector.tensor_mul(qt, xpad1[:, n8, :, 0:32], xpad1[:, n8, :, 1:33])
            nc.vector.tensor_mul(q1[:, n8], qt, xpad1[:, n8, :, 2:34])

        # ================ L1: maj3 64->64 @32x32 + pool ================
        # 12 taps (9 conv + 3 Q); batch halves as diagonal quadrants.
        for n8 in range(8):
            for hh in range(2):
                ps = cps.tile([128, 512], F32, name="ps1", tag="cpsum")
                for t in range(12):
                    for nh in (0, 1):
                        p0 = 64 * nh
                        if t < 9:
                            kh, kw = t // 3, t % 3
                            rhs = xpad1[p0:p0 + 64, n8,
                                        kh + hh * 16:kh + hh * 16 + 16, kw:kw + 32]
                        else:
                            kh = t - 9
                            rhs = q1[p0:p0 + 64, n8,
                                     kh + hh * 16:kh + hh * 16 + 16, :]
                        nc.tensor.matmul(
                            out=ps[p0:p0 + 64], lhsT=wc[p0:p0 + 64, OFF_W1 + t * 64:OFF_W1 + (t + 1) * 64],
                            rhs=rhs, start=(t == 0), stop=(t == 11),
                            tile_position=(p0, p0),
                            skip_group_check=(nh == 1))
                psv = ps.rearrange("p (h w) -> p h w", h=16)
                nc.scalar.activation(out=s1[:, n8, hh * 16:hh * 16 + 16, :],
                                     in_=psv, func=ACT.Sign, bias=bias[:, 1:2])

        # maxpool 2x2 on signs -> write interior of xpad2 (split for pipelining)
        s1r = s1.rearrange("p n (h2 pr) w -> p n h2 pr w", pr=2)
        p1r = p1h.rearrange("p n h (w2 pr) -> p n h w2 pr", pr=2)
        for j in range(4):
            sl = slice(2 * j, 2 * j + 2)
            nc.vector.tensor_max(p1h[:, sl], s1r[:, sl, :, 0, :], s1r[:, sl, :, 1, :])
            nc.vector.tensor_max(xpad2[:, sl, 1:17, 1:17],
                                 p1r[:, sl, :, :, 0], p1r[:, sl, :, :, 1])
            for n8 in (2 * j, 2 * j + 1):
                qt = qtmp.tile([128, 18, 16], BF16, name="qt2", tag="qt2")
                nc.vector.tensor_mul(qt, xpad2[:, n8, :, 0:16], xpad2[:, n8, :, 1:17])
                nc.vector.tensor_mul(q2[:, n8], qt, xpad2[:, n8, :, 2:18])

        # ================ L2: maj3 64->128 @16x16 ================
        # batch halves as row tiles -> two separate PSUM banks.
        for j in range(4):
            psA = cps.tile([128, 512], F32, name="ps2a", tag="cpsum")
            psB = cps.tile([128, 512], F32, name="ps2b", tag="cpsum")
            for t in range(12):
                for nh, pst in ((0, psA), (1, psB)):
                    p0 = 64 * nh
                    if t < 9:
                        kh, kw = t // 3, t % 3
                        rhs = xpad2[p0:p0 + 64, 2 * j:2 * j + 2,
                                    kh:kh + 16, kw:kw + 16]
                    else:
                        kh = t - 9
                        rhs = q2[p0:p0 + 64, 2 * j:2 * j + 2, kh:kh + 16, :]
                    nc.tensor.matmul(
                        out=pst, lhsT=wc[p0:p0 + 64, OFF_W2 + t * 128:OFF_W2 + (t + 1) * 128],
                        rhs=rhs, start=(t == 0), stop=(t == 11),
                        tile_position=(p0, 0))
            for nh, pst in ((0, psA), (1, psB)):
                psv = pst.rearrange("p (n h w) -> p n h w", n=2, h=16)
                nc.scalar.activation(
                    out=xpad3[:, 8 * nh + 2 * j:8 * nh + 2 * j + 2, 1:17, 1:17],
                    in_=psv, func=ACT.Sign, bias=bias[:, 2:3])

        # ================ L3: bin_conv 128->128 @16x16 + pool ================
        for j in range(8):
            ps = cps.tile([128, 512], F32, name="ps3", tag="cpsum")
            for t in range(9):
                kh, kw = t // 3, t % 3
                nc.tensor.matmul(
                    out=ps, lhsT=wc[:, OFF_W3 + t * 128:OFF_W3 + (t + 1) * 128],
                    rhs=xpad3[:, 2 * j:2 * j + 2, kh:kh + 16, kw:kw + 16],
                    start=(t == 0), stop=(t == 8))
            psv = ps.rearrange("p (n h w) -> p n h w", n=2, h=16)
            nc.scalar.activation(out=s3[:, 2 * j:2 * j + 2], in_=psv,
                                 func=ACT.Sign, bias=bias[:, 3:4])

        s3r = s3.rearrange("p n (h2 pr) w -> p n h2 pr w", pr=2)
        p3r = p3h.rearrange("p n h (w2 pr) -> p n h w2 pr", pr=2)
        for j in range(2):
            sl = slice(8 * j, 8 * j + 8)
            nc.vector.tensor_max(p3h[:, sl], s3r[:, sl, :, 0, :], s3r[:, sl, :, 1, :])
            nc.vector.tensor_max(xpad4[:, sl, 1:9, 1:9],
                                 p3r[:, sl, :, :, 0], p3r[:, sl, :, :, 1])

        # ================ L4: bin_conv 128->256 @8x8 ================
        for mb in range(2):
            for c8 in range(2):
                ps = cps.tile([128, 512], F32, name="ps4", tag="cpsum")
                for t in range(9):
                    kh, kw = t // 3, t % 3
                    nc.tensor.matmul(
                        out=ps,
                        lhsT=wc[:, OFF_W4 + (t * 2 + mb) * 128:OFF_W4 + (t * 2 + mb + 1) * 128],
                        rhs=xpad4[:, c8 * 8:c8 * 8 + 8, kh:kh + 8, kw:kw + 8],
                        start=(t == 0), stop=(t == 8))
                psv = ps.rearrange("p (n h w) -> p n h w", n=8, h=8)
                nc.scalar.activation(
                    out=a5[mb][:, c8 * 8:c8 * 8 + 8, 1:9, 1:9], in_=psv,
                    func=ACT.Sign, bias=bias[:, 4 + mb:5 + mb])

        # ================ L5: bin_conv 256->256 @8x8 + pool ================
        for mb in range(2):
            for c8 in range(2):
                ps = cps.tile([128, 512], F32, name="ps5", tag="cpsum")
                for kb in range(2):
                    for t in range(9):
                        kh, kw = t // 3, t % 3
                        nc.tensor.matmul(
                            out=ps,
                            lhsT=wc[:, OFF_W5 + ((kb * 9 + t) * 2 + mb) * 128:
                                    OFF_W5 + ((kb * 9 + t) * 2 + mb + 1) * 128],
                            rhs=a5[kb][:, c8 * 8:c8 * 8 + 8, kh:kh + 8, kw:kw + 8],
                            start=(kb == 0 and t == 0), stop=(kb == 1 and t == 8))
                psv = ps.rearrange("p (n h w) -> p n h w", n=8, h=8)
                nc.scalar.activation(out=s5[mb][:, c8 * 8:c8 * 8 + 8], in_=psv,
                                     func=ACT.Sign, bias=bias[:, 6 + mb:7 + mb])
        # pool + gather a6 -> X[n, d] rows (d = c*16 + hw), -1 pads at ends.
        # Gathers go on the Scalar DMA queue so they don't head-of-line
        # block the FC weight stream on the Sync queue.
        for mb in range(2):
            s5r = s5[mb].rearrange("p n (h2 pr) w -> p n h2 pr w", pr=2)
            nc.vector.tensor_max(p5h[mb], s5r[:, :, :, 0, :], s5r[:, :, :, 1, :])
            p5r = p5h[mb].rearrange("p n h (w2 pr) -> p n h w2 pr", pr=2)
            a6v = a6[mb].rearrange("p n (h w) -> p n h w", h=4)
            nc.vector.tensor_max(a6v, p5r[:, :, :, :, 0], p5r[:, :, :, :, 1])
            for n in range(16):
                nc.scalar.dma_start(
                    out=xbuf[n:n + 1, 1 + mb * 2048:1 + (mb + 1) * 2048],
                    in_=a6[mb][:, n, :])

        # ================ FC head ================

        # Px = triple products of consecutive padded features
        xr3 = xbuf.rearrange("p (g k) -> p g k", k=3)
        nc.vector.tensor_mul(pxt1, xr3[:, :, 0], xr3[:, :, 1])
        nc.vector.tensor_mul(px[:, 0:1366], pxt1, xr3[:, :, 2])

        # transpose Px -> [g, n] blocks for use as matmul lhsT
        for gb in range(11):
            txp = tps.tile([128, 16], F32, name="txp", tag="tx")
            nc.tensor.transpose(txp, px[:, gb * 128:(gb + 1) * 128], ident)
            nc.scalar.copy(pxt[:, gb * 16:(gb + 1) * 16], txp)

        # FC1: out[n, f] accumulated over 32 feature K-groups + 11 Px groups
        psf = fps.tile([16, 512], F32, name="psf1", tag="fc")
        nmm = 43
        i = 0
        for kg in range(32):
            cb, hw = kg // 16, kg % 16
            wt = wstream.tile([128, 512], FP8, name="wf1t", tag="w512", bufs=16)
            nc.sync.dma_start(out=wt, in_=wf1_d[:][kg])
            nc.tensor.matmul(out=psf, lhsT=a6[cb][:, :, hw], rhs=wt,
                             start=(i == 0), stop=(i == nmm - 1))
            i += 1
        for gb in range(11):
            wt = wstream.tile([128, 512], FP8, name="wpxt", tag="w512", bufs=16)
            nc.sync.dma_start(out=wt, in_=wpx_d[:][gb])
            nc.tensor.matmul(out=psf, lhsT=pxt[:, gb * 16:(gb + 1) * 16], rhs=wt,
                             start=(i == 0), stop=(i == nmm - 1))
            i += 1
        # bias + sign (beta_f1 along the free dim -> DVE add then ACT sign)
        nc.vector.tensor_add(af2f, psf, biasf[:, 0:512])
        nc.scalar.activation(out=af2f, in_=af2f, func=ACT.Sign, bias=0.0)
        for fb in range(4):
            txp = tps.tile([128, 16], F32, name="txa", tag="tx")
            nc.tensor.transpose(txp, af2f[:, fb * 128:(fb + 1) * 128], ident)
            nc.scalar.copy(af2t[:, fb * 16:(fb + 1) * 16], txp)

        # FC2
        psf2 = fps.tile([16, 512], F32, name="psf2", tag="fc")
        for kb in range(4):
            wt = wstream.tile([128, 512], FP8, name="wf2t", tag="w512", bufs=16)
            nc.sync.dma_start(out=wt, in_=wf2_d[:][kb])
            nc.tensor.matmul(out=psf2, lhsT=af2t[:, kb * 16:(kb + 1) * 16], rhs=wt,
                             start=(kb == 0), stop=(kb == 3))
        nc.vector.tensor_add(af3f, psf2, biasf[:, 512:1024])
        nc.scalar.activation(out=af3f, in_=af3f, func=ACT.Sign, bias=0.0)
        for fb in range(4):
            txp = tps.tile([128, 16], F32, name="txb", tag="tx")
            nc.tensor.transpose(txp, af3f[:, fb * 128:(fb + 1) * 128], ident)
            nc.scalar.copy(af3t[:, fb * 16:(fb + 1) * 16], txp)

        # FC3 (+ affine-only BN)
        psh = [fps.tile([16, 512], F32, name=f"psh{i}", tag="fc") for i in (0, 1)]
        for kb in range(4):
            wt = wstream.tile([128, 1024], FP8, name="wf3t", tag="w1024", bufs=2)
            nc.sync.dma_start(out=wt, in_=wf3_d[:][kb])
            for half in range(2):
                nc.tensor.matmul(out=psh[half], lhsT=af3t[:, kb * 16:(kb + 1) * 16],
                                 rhs=wt[:, half * 512:(half + 1) * 512],
                                 start=(kb == 0), stop=(kb == 3))
        for half in range(2):
            nc.vector.tensor_mul(yb[:, half * 512:(half + 1) * 512], psh[half],
                                 biasf[:, 1024 + half * 512:1024 + (half + 1) * 512])
        nc.vector.tensor_add(yb[:, 0:1000], yb[:, 0:1000], biasf[:, 2048:3048])

        # log-softmax over classes
        nc.vector.tensor_reduce(out=mx, in_=yb[:, 0:1000], axis=AX.X, op=ALU.max)
        nc.vector.tensor_scalar_mul(negmx, mx, -1.0)
        nc.scalar.activation(out=eb, in_=yb[:, 0:1000], func=ACT.Exp, bias=negmx)
        nc.vector.tensor_reduce(out=sm, in_=eb, axis=AX.X, op=ALU.add)
        nc.scalar.activation(out=lse, in_=sm, func=ACT.Ln, bias=0.0)
        nc.vector.tensor_scalar(out=outsb, in0=yb[:, 0:1000], scalar1=mx,
                                scalar2=lse, op0=ALU.subtract, op1=ALU.subtract)
        nc.gpsimd.dma_start(out=out_d[:], in_=outsb)


# ======================= host-side preparation =======================

def _sgn(a):
    return np.sign(np.asarray(a, np.float32)).astype(np.float32)


def _beta(bn):
    g, b, m, v = [np.asarray(t, np.float32) for t in bn]
    s = g / np.sqrt(v + EPS)
    return (b / s - m).astype(np.float32)


def prepare_weights(params):
    p = params
    w1b = _sgn(p['w1']); w2b = _sgn(p['w2']); w3b = _sgn(p['w3'])
    w4b = _sgn(p['w4']); w5b = _sgn(p['w5'])
    f1b = _sgn(p['fc1']); f2b = _sgn(p['fc2']); f3b = _sgn(p['fc3'])

    wt0 = np.zeros([27, 64], np.float32)
    w0b = _sgn(p['w0'])
    for c in range(3):
        for kh in range(3):
            for kw in range(3):
                wt0[c * 9 + kh * 3 + kw] = w0b[:, c, kh, kw]

    wc = np.zeros([128, WC_COLS], np.float32)

    def fill_maj(off, wb, F):
        W3 = wb[:, :, :, 0] * wb[:, :, :, 1] * wb[:, :, :, 2]
        for t in range(12):
            if t < 9:
                kh, kw = t // 3, t % 3
                blk = 0.5 * wb[:, :, kh, kw].T      # [c, F]
            else:
                blk = -0.5 * W3[:, :, t - 9].T
            wc[0:64, off + t * F: off + (t + 1) * F] = blk
            wc[64:128, off + t * F: off + (t + 1) * F] = blk

    fill_maj(OFF_W1, w1b, 64)
    fill_maj(OFF_W2, w2b, 128)
    for t in range(9):
        kh, kw = t // 3, t % 3
        wc[:, OFF_W3 + t * 128: OFF_W3 + (t + 1) * 128] = w3b[:, :, kh, kw].T
        for mb in range(2):
            wc[:, OFF_W4 + (t * 2 + mb) * 128: OFF_W4 + (t * 2 + mb + 1) * 128] = \
                w4b[mb * 128:(mb + 1) * 128, :, kh, kw].T
            for kb in range(2):
                wc[:, OFF_W5 + ((kb * 9 + t) * 2 + mb) * 128:
                   OFF_W5 + ((kb * 9 + t) * 2 + mb + 1) * 128] = \
                    w5b[mb * 128:(mb + 1) * 128, kb * 128:(kb + 1) * 128, kh, kw].T

    # FC1 main: wf1[kg=(cb*16+hw)][c_local, f] = 0.5*f1b[f, (cb*128+c)*16+hw+1]
    core = f1b[:, 1:4097].reshape(512, 256, 16)           # [f, c, hw]
    wf1 = np.zeros([32, 128, 512], np.float32)
    for cb in range(2):
        for hw in range(16):
            wf1[cb * 16 + hw] = 0.5 * core[:, cb * 128:(cb + 1) * 128, hw].T
    # FC1 Px term
    W3f = f1b[:, 0::3] * f1b[:, 1::3] * f1b[:, 2::3]      # [512, 1366]
    wpx = np.zeros([11, 128, 512], np.float32)
    W3p = np.zeros([512, 1408], np.float32)
    W3p[:, 0:1366] = -0.5 * W3f
    for gb in range(11):
        wpx[gb] = W3p[:, gb * 128:(gb + 1) * 128].T
    wf2 = np.zeros([4, 128, 512], np.float32)
    for kb in range(4):
        wf2[kb] = f2b[:, kb * 128:(kb + 1) * 128].T
    f3p = np.zeros([1024, 512], np.float32)
    f3p[0:1000] = f3b
    wf3 = np.zeros([4, 128, 1024], np.float32)
    for kb in range(4):
        wf3[kb] = f3p[:, kb * 128:(kb + 1) * 128].T

    bias = np.zeros([128, 8], np.float32)
    b0 = _beta(p['bn0']); b1 = _beta(p['bn1'])
    bias[:, 0] = np.concatenate([b0, b0]); bias[:, 1] = np.concatenate([b1, b1])
    bias[:, 2] = _beta(p['bn2']); bias[:, 3] = _beta(p['bn3'])
    b4 = _beta(p['bn4']); b5 = _beta(p['bn5'])
    bias[:, 4] = b4[0:128]; bias[:, 5] = b4[128:256]
    bias[:, 6] = b5[0:128]; bias[:, 7] = b5[128:256]

    biasf = np.zeros([4, 1024], np.float32)
    bf1 = _beta(p['bnf1']) - 0.5 * (f1b[:, 0] + f1b[:, 4097])
    biasf[0, 0:512] = bf1
    biasf[1, 0:512] = _beta(p['bnf2'])
    m3, v3 = [np.asarray(t, np.float32) for t in p['bnf3']]
    rs = 1.0 / np.sqrt(v3 + EPS)
    biasf[2, 0:1000] = rs
    biasf[3, 0:1000] = -m3 * rs

    bf = ml_dtypes.bfloat16
    f8 = ml_dtypes.float8_e4m3
    return {
        "wt0": wt0, "wc": wc.astype(bf), "wf1": wf1.astype(f8),
        "wpx": wpx.astype(f8), "wf2": wf2.astype(f8), "wf3": wf3.astype(f8),
        "bias": bias, "biasf": biasf,
    }


def prepare_xim(xc):
    """Per-core im2col: xc [16, 3, 32, 32] -> [27, 2, 8, 1024] fp32."""
    xp = np.pad(np.asarray(xc, np.float32), ((0, 0), (0, 0), (1, 1), (1, 1)))
    xim = np.empty([27, 16, 1024], np.float32)
    for c in range(3):
        for kh in range(3):
            for kw in range(3):
                xim[c * 9 + kh * 3 + kw] = \
                    xp[:, c, kh:kh + 32, kw:kw + 32].reshape(16, 1024)
    return xim.reshape(27, 2, 8, 1024)


def make_in_maps(x, params):
    shared = prepare_weights(params)
    x = np.asarray(x, np.float32)
    in_maps = []
    for ci in range(NCORES):
        m = dict(shared)
        m["xim"] = prepare_xim(x[ci * NPC:(ci + 1) * NPC])
        in_maps.append(m)
    return in_maps


def kernel(x, params):
    in_maps = make_in_maps(x, params)
    nc = build_nc()
    res = run_bass_kernel_spmd(nc, in_maps, core_ids=list(range(NCORES)))
    out = np.concatenate([np.asarray(res.results[i]["out"], np.float32)
                          for i in range(NCORES)], axis=0)
    return out


# revision 36
# speedup vs baseline: 1.3046x; 1.2017x over previous
"""Trainium2 Bass kernel for the binary CNN (CNV/BNN) forward pass.

Strategy
--------
Pure data parallel: batch 128 -> 16 images per NeuronCore x 8 cores.

Math transformations (all exact, validated against the jax reference):
  * sign(htanh(bn(y))) == sign(y + beta) with beta = b/s - m, since the BN
    scale s = g*rsqrt(v+eps) > 0 always.  htanh and the BN multiply never
    need to be materialized on the trunk.
  * sign is monotone, so sign(maxpool(u)) == maxpool(sign(u)); signs are
    computed directly from PSUM (fused bias via the ScalarE Sign activation)
    and pooling runs on +-1 bf16 values.
  * maj3_conv (majority-of-3 popcount conv) uses the identity, exact for all
    zero-padding cases that occur:
        sign(w0x0 + w1x1 + w2x2) = (w0x0 + w1x1 + w2x2 - (w0w1w2)(x0x1x2))/2
    so  maj3_conv(x, w) = 0.5*conv3x3(xb, wb)
                          - 0.5*conv3x1_vertical(Q, W3)
    where Q = horizontal triple products of the padded activations and
    W3[f,c,kh] = prod_kw wb[f,c,kh,kw].  Same identity for fcmaj3.
  * The whole binary trunk runs in bf16: +-1 and +-0.5 are exact, popcount
    partial sums accumulate in fp32 PSUM, and bf16 rounding of (u + beta)
    preserves sign.

Layouts: activations [C(part), N, H, W]; for C=64 layers the batch is split
across partition halves (p = c + 64*nh) and the two halves run as
concurrent tile_position row/col tiles of the 128x128 PE array.
"""

import os
import numpy as np
import ml_dtypes

import concourse.bass as bass
import concourse.tile as tile
from concourse import bacc, mybir
from concourse.bass_utils import run_bass_kernel_spmd
from concourse.masks import make_identity

F32 = mybir.dt.float32
BF16 = mybir.dt.bfloat16
FP8 = mybir.dt.float8e4
AX = mybir.AxisListType
ALU = mybir.AluOpType
ACT = mybir.ActivationFunctionType

EPS = 1e-5
NCORES = 8
NPC = 16  # images per core

# conv-weight table free-dim offsets inside the packed "wc" tensor
OFF_W1 = 0            # 12 taps x 64
OFF_W2 = 768          # 12 taps x 128
OFF_W3 = 768 + 1536   # 9 taps x 128
OFF_W4 = OFF_W3 + 1152  # (9 taps x 2 mb) x 128
OFF_W5 = OFF_W4 + 2304  # (2 kb x 9 taps x 2 mb) x 128
WC_COLS = OFF_W5 + 4608


def build_nc():
    nc = bacc.Bacc()

    xim_d = nc.dram_tensor("xim", [27, 2, 8, 1024], F32, kind="ExternalInput")
    wt0_d = nc.dram_tensor("wt0", [27, 64], F32, kind="ExternalInput")
    wc_d = nc.dram_tensor("wc", [128, WC_COLS], BF16, kind="ExternalInput")
    wf1_d = nc.dram_tensor("wf1", [32, 128, 512], FP8, kind="ExternalInput")
    wpx_d = nc.dram_tensor("wpx", [11, 128, 512], FP8, kind="ExternalInput")
    wf2_d = nc.dram_tensor("wf2", [4, 128, 512], FP8, kind="ExternalInput")
    wf3_d = nc.dram_tensor("wf3", [4, 128, 1024], FP8, kind="ExternalInput")
    bias_d = nc.dram_tensor("bias", [128, 8], F32, kind="ExternalInput")
    biasf_d = nc.dram_tensor("biasf", [4, 1024], F32, kind="ExternalInput")
    out_d = nc.dram_tensor("out", [16, 1000], F32, kind="ExternalOutput")
    a6d = nc.dram_tensor("a6d", [2, 128, 256], BF16)

    with tile.TileContext(nc) as tc:
        emit(nc, tc, xim_d, wt0_d, wc_d, wf1_d, wpx_d, wf2_d, wf3_d,
             bias_d, biasf_d, out_d, a6d)
    nc.compile()
    return nc


def emit(nc, tc, xim_d, wt0_d, wc_d, wf1_d, wpx_d, wf2_d, wf3_d,
         bias_d, biasf_d, out_d, a6d):
    from contextlib import ExitStack
    ctx = ExitStack()
    with ctx:
        wpool = ctx.enter_context(tc.tile_pool(name="wpool", bufs=1))
        acts = ctx.enter_context(tc.tile_pool(name="acts", bufs=1))
        ximp = ctx.enter_context(tc.tile_pool(name="ximp", bufs=2))
        qtmp = ctx.enter_context(tc.tile_pool(name="qtmp", bufs=2))
        cps = ctx.enter_context(tc.tile_pool(name="cps", bufs=4, space="PSUM"))
        fps = ctx.enter_context(tc.tile_pool(name="fps", bufs=2, space="PSUM"))
        tps = ctx.enter_context(tc.tile_pool(name="tps", bufs=2, space="PSUM"))

        # ---------------- weights needed by L0 (issue first) ----------------
        wt0 = wpool.tile([27, 64], F32, name="wt0s")
        nc.sync.dma_start(out=wt0, in_=wt0_d[:])
        bias = wpool.tile([128, 8], F32, name="biass")
        nc.sync.dma_start(out=bias, in_=bias_d[:])
        # PE warm-up: ~4.5us of dense matmuls so the HAM clock-gate opens
        # before L0 (and stays open).  Results are discarded.
        wu = wpool.tile([128, 512], BF16, name="wu")
        nc.vector.memset(wu, 1.0)
        wups = fps.tile([16, 512], F32, name="wups", tag="fc")
        for _ in range(22):
            nc.tensor.matmul(out=wups, lhsT=wu[:, 0:16], rhs=wu,
                             start=True, stop=True, skip_group_check=True)

        # ---------------- persistent activation buffers ----------------
        xpad1 = acts.tile([128, 8, 34, 34], BF16, name="xpad1")
        q1 = acts.tile([128, 8, 34, 32], BF16, name="q1")
        s1 = acts.tile([128, 8, 32, 32], BF16, name="s1")
        p1h = acts.tile([128, 8, 16, 32], BF16, name="p1h")
        xpad2 = acts.tile([128, 8, 18, 18], BF16, name="xpad2")
        q2 = acts.tile([128, 8, 18, 16], BF16, name="q2")
        xpad3 = acts.tile([128, 16, 18, 18], BF16, name="xpad3")
        s3 = acts.tile([128, 16, 16, 16], BF16, name="s3")
        p3h = acts.tile([128, 16, 8, 16], BF16, name="p3h")
        xpad4 = acts.tile([128, 16, 10, 10], BF16, name="xpad4")
        a5 = [acts.tile([128, 16, 10, 10], BF16, name=f"a5_{i}") for i in (0, 1)]
        s5 = [acts.tile([128, 16, 8, 8], BF16, name=f"s5_{i}") for i in (0, 1)]
        p5h = [acts.tile([128, 16, 4, 8], BF16, name=f"p5h_{i}") for i in (0, 1)]
        a6 = [acts.tile([128, 16, 16], BF16, name=f"a6_{i}") for i in (0, 1)]

        xbuf = acts.tile([16, 4098], BF16, name="xbuf")
        pxt1 = acts.tile([16, 1366], BF16, name="pxt1")
        px = acts.tile([16, 1408], BF16, name="px")
        pxt = acts.tile([128, 176], BF16, name="pxt")
        af2f = acts.tile([16, 512], BF16, name="af2f")
        af3f = acts.tile([16, 512], BF16, name="af3f")
        af2t = acts.tile([128, 64], BF16, name="af2t")
        af3t = acts.tile([128, 64], BF16, name="af3t")
        yb = acts.tile([16, 1024], F32, name="yb")
        mx = acts.tile([16, 1], F32, name="mx")
        negmx = acts.tile([16, 1], F32, name="negmx")
        sm = acts.tile([16, 1], F32, name="sm")
        lse = acts.tile([16, 1], F32, name="lse")
        outsb = acts.tile([16, 1000], F32, name="outsb")

        # zero only the padding halos (interiors are overwritten by drains)
        def halo_memset(eng, buf, H, W):
            eng.memset(buf[:, :, 0, :], 0.0)          # top row
            eng.memset(buf[:, :, H - 1, :], 0.0)      # bottom row
            eng.memset(buf[:, :, 1:H - 1, 0], 0.0)    # left col
            eng.memset(buf[:, :, 1:H - 1, W - 1], 0.0)  # right col
        halo_memset(nc.gpsimd, xpad1, 34, 34)
        halo_memset(nc.vector, xpad2, 18, 18)
        halo_memset(nc.gpsimd, xpad3, 18, 18)
        halo_memset(nc.vector, xpad4, 10, 10)
        halo_memset(nc.gpsimd, a5[0], 10, 10)
        halo_memset(nc.gpsimd, a5[1], 10, 10)
        nc.vector.memset(xbuf[:, 0:1], -1.0)
        nc.vector.memset(xbuf[:, 4097:4098], -1.0)
        nc.vector.memset(px[:, 1366:1408], 0.0)

        # ================ L0: conv 3->64 on real-valued x ================
        # K=27 im2col rows; two batch halves run as concurrent col tiles.
        for n8 in range(8):
            for hh in range(2):
                ximt = ximp.tile([27, 2, 512], F32, name="ximt")
                nc.sync.dma_start(out=ximt, in_=xim_d[:][:, :, n8, hh * 512:(hh + 1) * 512])
                ps = cps.tile([128, 512], F32, name="ps0", tag="cpsum")
                nc.tensor.matmul(out=ps[0:64], lhsT=wt0, rhs=ximt[:, 0],
                                 start=True, stop=True, tile_position=(0, 0))
                nc.tensor.matmul(out=ps[64:128], lhsT=wt0, rhs=ximt[:, 1],
                                 start=True, stop=True, tile_position=(0, 64),
                                 skip_group_check=True)
                psv = ps.rearrange("p (h w) -> p h w", h=16)
                nc.scalar.activation(
                    out=xpad1[:, n8, 1 + hh * 16:17 + hh * 16, 1:33],
                    in_=psv, func=ACT.Sign, bias=bias[:, 0:1])

        # ------- remaining persistent weights (issued after L0's DMAs) -------
        wc = wpool.tile([128, WC_COLS], BF16, name="wcs")
        nc.sync.dma_start(out=wc, in_=wc_d[:])
        wf1s = wpool.tile([128, 32, 512], FP8, name="wf1s")
        nc.sync.dma_start(out=wf1s, in_=wf1_d[:].rearrange("k c f -> c k f"))
        wpxs = wpool.tile([128, 11, 512], FP8, name="wpxs")
        nc.sync.dma_start(out=wpxs, in_=wpx_d[:].rearrange("k c f -> c k f"))
        wf2s = wpool.tile([128, 4, 512], FP8, name="wf2s")
        nc.sync.dma_start(out=wf2s, in_=wf2_d[:].rearrange("k c f -> c k f"))
        wf3s = wpool.tile([128, 4, 1024], FP8, name="wf3s")
        nc.sync.dma_start(out=wf3s, in_=wf3_d[:].rearrange("k c f -> c k f"))
        # bias rows broadcast to 16 partitions (DVE can't take step-0 APs)
        # layout [16, 3072]: bf1(512) | bf2(512) | rs(1024) | mrs(1024)
        biasf = wpool.tile([16, 3072], F32, name="biasfs")
        bf_ap = biasf_d[:]

        def _bcast_row(dst, row, n):
            nc.sync.dma_start(out=dst, in_=bass.AP(
                tensor=bf_ap.tensor, offset=bf_ap.offset + row * 1024,
                ap=[[0, 16], [1, n]]))
        _bcast_row(biasf[:, 0:512], 0, 512)
        _bcast_row(biasf[:, 512:1024], 1, 512)
        _bcast_row(biasf[:, 1024:2048], 2, 1024)
        _bcast_row(biasf[:, 2048:3072], 3, 1024)
        ident = wpool.tile([16, 16], BF16, name="idents")
        make_identity(nc, ident)

        # Q1 = horizontal triple products of xpad1 (zeros propagate)
        for n8 in range(8):
            qt = qtmp.tile([128, 34, 32], BF16, name="qt1", tag="qt1")
            nc.vector.tensor_mul(qt, xpad1[:, n8, :, 0:32], xpad1[:, n8, :, 1:33])
            nc.vector.tensor_mul(q1[:, n8], qt, xpad1[:, n8, :, 2:34])

        # ================ L1: maj3 64->64 @32x32 + pool ================
        # 12 taps (9 conv + 3 Q); batch halves as diagonal quadrants.
        for n8 in range(8):
            for hh in range(2):
                ps = cps.tile([128, 512], F32, name="ps1", tag="cpsum")
                for t in range(12):
                    for nh in (0, 1):
                        p0 = 64 * nh
                        if t < 9:
                            kh, kw = t // 3, t % 3
                            rhs = xpad1[p0:p0 + 64, n8,
                                        kh + hh * 16:kh + hh * 16 + 16, kw:kw + 32]
                        else:
                            kh = t - 9
                            rhs = q1[p0:p0 + 64, n8,
                                     kh + hh * 16:kh + hh * 16 + 16, :]
                        nc.tensor.matmul(
                            out=ps[p0:p0 + 64], lhsT=wc[p0:p0 + 64, OFF_W1 + t * 64:OFF_W1 + (t + 1) * 64],
                            rhs=rhs, start=(t == 0), stop=(t == 11),
                            tile_position=(p0, p0),
                            skip_group_check=(nh == 1))
                psv = ps.rearrange("p (h w) -> p h w", h=16)
                nc.scalar.activation(out=s1[:, n8, hh * 16:hh * 16 + 16, :],
                                     in_=psv, func=ACT.Sign, bias=bias[:, 1:2])

        # maxpool 2x2 on signs -> write interior of xpad2 (split for pipelining)
        s1r = s1.rearrange("p n (h2 pr) w -> p n h2 pr w", pr=2)
        p1r = p1h.rearrange("p n h (w2 pr) -> p n h w2 pr", pr=2)
        for j in range(4):
            sl = slice(2 * j, 2 * j + 2)
            nc.vector.tensor_max(p1h[:, sl], s1r[:, sl, :, 0, :], s1r[:, sl, :, 1, :])
            nc.vector.tensor_max(xpad2[:, sl, 1:17, 1:17],
                                 p1r[:, sl, :, :, 0], p1r[:, sl, :, :, 1])
            for n8 in (2 * j, 2 * j + 1):
                qt = qtmp.tile([128, 18, 16], BF16, name="qt2", tag="qt2")
                nc.vector.tensor_mul(qt, xpad2[:, n8, :, 0:16], xpad2[:, n8, :, 1:17])
                nc.vector.tensor_mul(q2[:, n8], qt, xpad2[:, n8, :, 2:18])

        # ================ L2: maj3 64->128 @16x16 ================
        # batch halves as row tiles -> two separate PSUM banks.
        for j in range(4):
            psA = cps.tile([128, 512], F32, name="ps2a", tag="cpsum")
            psB = cps.tile([128, 512], F32, name="ps2b", tag="cpsum")
            for t in range(12):
                for nh, pst in ((0, psA), (1, psB)):
                    p0 = 64 * nh
                    if t < 9:
                        kh, kw = t // 3, t % 3
                        rhs = xpad2[p0:p0 + 64, 2 * j:2 * j + 2,
                                    kh:kh + 16, kw:kw + 16]
                    else:
                        kh = t - 9
                        rhs = q2[p0:p0 + 64, 2 * j:2 * j + 2, kh:kh + 16, :]
                    nc.tensor.matmul(
                        out=pst, lhsT=wc[p0:p0 + 64, OFF_W2 + t * 128:OFF_W2 + (t + 1) * 128],
                        rhs=rhs, start=(t == 0), stop=(t == 11),
                        tile_position=(p0, 0))
            for nh, pst in ((0, psA), (1, psB)):
                psv = pst.rearrange("p (n h w) -> p n h w", n=2, h=16)
                nc.scalar.activation(
                    out=xpad3[:, 8 * nh + 2 * j:8 * nh + 2 * j + 2, 1:17, 1:17],
                    in_=psv, func=ACT.Sign, bias=bias[:, 2:3])

        # ================ L3: bin_conv 128->128 @16x16 + pool ================
        for j in range(8):
            ps = cps.tile([128, 512], F32, name="ps3", tag="cpsum")
            for t in range(9):
                kh, kw = t // 3, t % 3
                nc.tensor.matmul(
                    out=ps, lhsT=wc[:, OFF_W3 + t * 128:OFF_W3 + (t + 1) * 128],
                    rhs=xpad3[:, 2 * j:2 * j + 2, kh:kh + 16, kw:kw + 16],
                    start=(t == 0), stop=(t == 8))
            psv = ps.rearrange("p (n h w) -> p n h w", n=2, h=16)
            nc.scalar.activation(out=s3[:, 2 * j:2 * j + 2], in_=psv,
                                 func=ACT.Sign, bias=bias[:, 3:4])

        s3r = s3.rearrange("p n (h2 pr) w -> p n h2 pr w", pr=2)
        p3r = p3h.rearrange("p n h (w2 pr) -> p n h w2 pr", pr=2)
        for j in range(2):
            sl = slice(8 * j, 8 * j + 8)
            nc.vector.tensor_max(p3h[:, sl], s3r[:, sl, :, 0, :], s3r[:, sl, :, 1, :])
            nc.vector.tensor_max(xpad4[:, sl, 1:9, 1:9],
                                 p3r[:, sl, :, :, 0], p3r[:, sl, :, :, 1])

        # ================ L4: bin_conv 128->256 @8x8 ================
        for mb in range(2):
            for c8 in range(2):
                ps = cps.tile([128, 512], F32, name="ps4", tag="cpsum")
                for t in range(9):
                    kh, kw = t // 3, t % 3
                    nc.tensor.matmul(
                        out=ps,
                        lhsT=wc[:, OFF_W4 + (t * 2 + mb) * 128:OFF_W4 + (t * 2 + mb + 1) * 128],
                        rhs=xpad4[:, c8 * 8:c8 * 8 + 8, kh:kh + 8, kw:kw + 8],
                        start=(t == 0), stop=(t == 8))
                psv = ps.rearrange("p (n h w) -> p n h w", n=8, h=8)
                nc.scalar.activation(
                    out=a5[mb][:, c8 * 8:c8 * 8 + 8, 1:9, 1:9], in_=psv,
                    func=ACT.Sign, bias=bias[:, 4 + mb:5 + mb])

        # ================ L5: bin_conv 256->256 @8x8 + pool ================
        for mb in range(2):
            for c8 in range(2):
                ps = cps.tile([128, 512], F32, name="ps5", tag="cpsum")
                for kb in range(2):
                    for t in range(9):
                        kh, kw = t // 3, t % 3
                        nc.tensor.matmul(
                            out=ps,
                            lhsT=wc[:, OFF_W5 + ((kb * 9 + t) * 2 + mb) * 128:
                                    OFF_W5 + ((kb * 9 + t) * 2 + mb + 1) * 128],
                            rhs=a5[kb][:, c8 * 8:c8 * 8 + 8, kh:kh + 8, kw:kw + 8],
                            start=(kb == 0 and t == 0), stop=(kb == 1 and t == 8))
                psv = ps.rearrange("p (n h w) -> p n h w", n=8, h=8)
                nc.scalar.activation(out=s5[mb][:, c8 * 8:c8 * 8 + 8], in_=psv,
                                     func=ACT.Sign, bias=bias[:, 6 + mb:7 + mb])
        # pool + gather a6 -> X[n, d] rows (d = c*16 + hw), -1 pads at ends.
        # Gathers go on the Scalar DMA queue so they don't head-of-line
        # block the FC weight stream on the Sync queue.
        for mb in range(2):
            s5r = s5[mb].rearrange("p n (h2 pr) w -> p n h2 pr w", pr=2)
            nc.vector.tensor_max(p5h[mb], s5r[:, :, :, 0, :], s5r[:, :, :, 1, :])
            p5r = p5h[mb].rearrange("p n h (w2 pr) -> p n h w2 pr", pr=2)
            a6v = a6[mb].rearrange("p n (h w) -> p n h w", h=4)
            nc.vector.tensor_max(a6v, p5r[:, :, :, :, 0], p5r[:, :, :, :, 1])
            nc.scalar.dma_start(out=a6d[:][mb], in_=a6[mb])
            a6d_ap = a6d[:]
            nc.scalar.dma_start(
                out=xbuf[:, 1 + mb * 2048:1 + (mb + 1) * 2048],
                in_=bass.AP(tensor=a6d_ap.tensor,
                            offset=a6d_ap.offset + mb * 32768,
                            ap=[[16, 16], [256, 128], [1, 16]]))

        # ================ FC head ================

        # Px = triple products of consecutive padded features
        xr3 = xbuf.rearrange("p (g k) -> p g k", k=3)
        nc.vector.tensor_mul(pxt1, xr3[:, :, 0], xr3[:, :, 1])
        nc.vector.tensor_mul(px[:, 0:1366], pxt1, xr3[:, :, 2])

        # transpose Px -> [g, n] blocks for use as matmul lhsT
        for gb in range(11):
            txp = tps.tile([128, 16], BF16, name="txp", tag="tx")
            nc.tensor.transpose(txp, px[:, gb * 128:(gb + 1) * 128], ident)
            nc.scalar.copy(pxt[:, gb * 16:(gb + 1) * 16], txp)

        # FC1: out[n, f] accumulated over 32 feature K-groups + 11 Px groups
        psf = fps.tile([16, 512], F32, name="psf1", tag="fc")
        nmm = 43
        i = 0
        for kg in range(32):
            cb, hw = kg // 16, kg % 16
            nc.tensor.matmul(out=psf, lhsT=a6[cb][:, :, hw], rhs=wf1s[:, kg],
                             start=(i == 0), stop=(i == nmm - 1))
            i += 1
        for gb in range(11):
            nc.tensor.matmul(out=psf, lhsT=pxt[:, gb * 16:(gb + 1) * 16],
                             rhs=wpxs[:, gb],
                             start=(i == 0), stop=(i == nmm - 1))
            i += 1
        # bias + sign (beta_f1 along the free dim -> DVE add then ACT sign)
        nc.vector.tensor_add(af2f, psf, biasf[:, 0:512])
        nc.scalar.activation(out=af2f, in_=af2f, func=ACT.Sign, bias=0.0)
        for fb in range(4):
            txp = tps.tile([128, 16], BF16, name="txa", tag="tx")
            nc.tensor.transpose(txp, af2f[:, fb * 128:(fb + 1) * 128], ident)
            nc.scalar.copy(af2t[:, fb * 16:(fb + 1) * 16], txp)

        # FC2
        psf2 = fps.tile([16, 512], F32, name="psf2", tag="fc")
        for kb in range(4):
            nc.tensor.matmul(out=psf2, lhsT=af2t[:, kb * 16:(kb + 1) * 16],
                             rhs=wf2s[:, kb], start=(kb == 0), stop=(kb == 3))
        nc.vector.tensor_add(af3f, psf2, biasf[:, 512:1024])
        nc.scalar.activation(out=af3f, in_=af3f, func=ACT.Sign, bias=0.0)
        for fb in range(4):
            txp = tps.tile([128, 16], BF16, name="txb", tag="tx")
            nc.tensor.transpose(txp, af3f[:, fb * 128:(fb + 1) * 128], ident)
            nc.scalar.copy(af3t[:, fb * 16:(fb + 1) * 16], txp)

        # FC3 (+ affine-only BN)
        psh = [fps.tile([16, 512], F32, name=f"psh{i}", tag="fc") for i in (0, 1)]
        for kb in range(4):
            for half in range(2):
                nc.tensor.matmul(out=psh[half], lhsT=af3t[:, kb * 16:(kb + 1) * 16],
                                 rhs=wf3s[:, kb, half * 512:(half + 1) * 512],
                                 start=(kb == 0), stop=(kb == 3))
        for half in range(2):
            nc.vector.tensor_mul(yb[:, half * 512:(half + 1) * 512], psh[half],
                                 biasf[:, 1024 + half * 512:1024 + (half + 1) * 512])
        nc.vector.tensor_add(yb[:, 0:1000], yb[:, 0:1000], biasf[:, 2048:3048])

        # log-softmax over classes
        nc.vector.tensor_reduce(out=mx, in_=yb[:, 0:1000], axis=AX.X, op=ALU.max)
        nc.vector.tensor_scalar_mul(negmx, mx, -1.0)
        nc.scalar.activation(out=outsb, in_=yb[:, 0:1000], func=ACT.Exp, bias=negmx)
        nc.vector.tensor_reduce(out=sm, in_=outsb, axis=AX.X, op=ALU.add)
        nc.scalar.activation(out=lse, in_=sm, func=ACT.Ln, bias=0.0)
        nc.vector.tensor_scalar(out=outsb, in0=yb[:, 0:1000], scalar1=mx,
                                scalar2=lse, op0=ALU.subtract, op1=ALU.subtract)
        nc.gpsimd.dma_start(out=out_d[:], in_=outsb)


# ======================= host-side preparation =======================

def _sgn(a):
    return np.sign(np.asarray(a, np.float32)).astype(np.float32)


def _beta(bn):
    g, b, m, v = [np.asarray(t, np.float32) for t in bn]
    s = g / np.sqrt(v + EPS)
    return (b / s - m).astype(np.float32)


def prepare_weights(params):
    p = params
    w1b = _sgn(p['w1']); w2b = _sgn(p['w2']); w3b = _sgn(p['w3'])
    w4b = _sgn(p['w4']); w5b = _sgn(p['w5'])
    f1b = _sgn(p['fc1']); f2b = _sgn(p['fc2']); f3b = _sgn(p['fc3'])

    wt0 = np.zeros([27, 64], np.float32)
    w0b = _sgn(p['w0'])
    for c in range(3):
        for kh in range(3):
            for kw in range(3):
                wt0[c * 9 + kh * 3 + kw] = w0b[:, c, kh, kw]

    wc = np.zeros([128, WC_COLS], np.float32)

    def fill_maj(off, wb, F):
        W3 = wb[:, :, :, 0] * wb[:, :, :, 1] * wb[:, :, :, 2]
        for t in range(12):
            if t < 9:
                kh, kw = t // 3, t % 3
                blk = 0.5 * wb[:, :, kh, kw].T      # [c, F]
            else:
                blk = -0.5 * W3[:, :, t - 9].T
            wc[0:64, off + t * F: off + (t + 1) * F] = blk
            wc[64:128, off + t * F: off + (t + 1) * F] = blk

    fill_maj(OFF_W1, w1b, 64)
    fill_maj(OFF_W2, w2b, 128)
    for t in range(9):
        kh, kw = t // 3, t % 3
        wc[:, OFF_W3 + t * 128: OFF_W3 + (t + 1) * 128] = w3b[:, :, kh, kw].T
        for mb in range(2):
            wc[:, OFF_W4 + (t * 2 + mb) * 128: OFF_W4 + (t * 2 + mb + 1) * 128] = \
                w4b[mb * 128:(mb + 1) * 128, :, kh, kw].T
            for kb in range(2):
                wc[:, OFF_W5 + ((kb * 9 + t) * 2 + mb) * 128:
                   OFF_W5 + ((kb * 9 + t) * 2 + mb + 1) * 128] = \
                    w5b[mb * 128:(mb + 1) * 128, kb * 128:(kb + 1) * 128, kh, kw].T

    # FC1 main: wf1[kg=(cb*16+hw)][c_local, f] = 0.5*f1b[f, (cb*128+c)*16+hw+1]
    core = f1b[:, 1:4097].reshape(512, 256, 16)           # [f, c, hw]
    wf1 = np.zeros([32, 128, 512], np.float32)
    for cb in range(2):
        for hw in range(16):
            wf1[cb * 16 + hw] = 0.5 * core[:, cb * 128:(cb + 1) * 128, hw].T
    # FC1 Px term
    W3f = f1b[:, 0::3] * f1b[:, 1::3] * f1b[:, 2::3]      # [512, 1366]
    wpx = np.zeros([11, 128, 512], np.float32)
    W3p = np.zeros([512, 1408], np.float32)
    W3p[:, 0:1366] = -0.5 * W3f
    for gb in range(11):
        wpx[gb] = W3p[:, gb * 128:(gb + 1) * 128].T
    wf2 = np.zeros([4, 128, 512], np.float32)
    for kb in range(4):
        wf2[kb] = f2b[:, kb * 128:(kb + 1) * 128].T
    f3p = np.zeros([1024, 512], np.float32)
    f3p[0:1000] = f3b
    wf3 = np.zeros([4, 128, 1024], np.float32)
    for kb in range(4):
        wf3[kb] = f3p[:, kb * 128:(kb + 1) * 128].T

    bias = np.zeros([128, 8], np.float32)
    b0 = _beta(p['bn0']); b1 = _beta(p['bn1'])
    bias[:, 0] = np.concatenate([b0, b0]); bias[:, 1] = np.concatenate([b1, b1])
    bias[:, 2] = _beta(p['bn2']); bias[:, 3] = _beta(p['bn3'])
    b4 = _beta(p['bn4']); b5 = _beta(p['bn5'])
    bias[:, 4] = b4[0:128]; bias[:, 5] = b4[128:256]
    bias[:, 6] = b5[0:128]; bias[:, 7] = b5[128:256]

    biasf = np.zeros([4, 1024], np.float32)
    bf1 = _beta(p['bnf1']) - 0.5 * (f1b[:, 0] + f1b[:, 4097])
    biasf[0, 0:512] = bf1
    biasf[1, 0:512] = _beta(p['bnf2'])
    m3, v3 = [np.asarray(t, np.float32) for t in p['bnf3']]
    rs = 1.0 / np.sqrt(v3 + EPS)
    biasf[2, 0:1000] = rs
    biasf[3, 0:1000] = -m3 * rs

    bf = ml_dtypes.bfloat16
    f8 = ml_dtypes.float8_e4m3
    return {
        "wt0": wt0, "wc": wc.astype(bf), "wf1": wf1.astype(f8),
        "wpx": wpx.astype(f8), "wf2": wf2.astype(f8), "wf3": wf3.astype(f8),
        "bias": bias, "biasf": biasf,
    }


def prepare_xim(xc):
    """Per-core im2col with bf16 hi/lo split: -> [54, 2, 8, 1024] bf16."""
    xp = np.pad(np.asarray(xc, np.float32), ((0, 0), (0, 0), (1, 1), (1, 1)))
    xim = np.empty([27, 16, 1024], np.float32)
    for c in range(3):
        for kh in range(3):
            for kw in range(3):
                xim[c * 9 + kh * 3 + kw] = \
                    xp[:, c, kh:kh + 32, kw:kw + 32].reshape(16, 1024)
    return xim.reshape(27, 2, 8, 1024)


def make_in_maps(x, params):
    shared = prepare_weights(params)
    x = np.asarray(x, np.float32)
    in_maps = []
    for ci in range(NCORES):
        m = dict(shared)
        m["xim"] = prepare_xim(x[ci * NPC:(ci + 1) * NPC])
        in_maps.append(m)
    return in_maps


def kernel(x, params):
    in_maps = make_in_maps(x, params)
    nc = build_nc()
    res = run_bass_kernel_spmd(nc, in_maps, core_ids=list(range(NCORES)))
    out = np.concatenate([np.asarray(res.results[i]["out"], np.float32)
                          for i in range(NCORES)], axis=0)
    return out


# revision 37
# speedup vs baseline: 1.3124x; 1.0060x over previous
"""Trainium2 Bass kernel for the binary CNN (CNV/BNN) forward pass.

Strategy
--------
Pure data parallel: batch 128 -> 16 images per NeuronCore x 8 cores.

Math transformations (all exact, validated against the jax reference):
  * sign(htanh(bn(y))) == sign(y + beta) with beta = b/s - m, since the BN
    scale s = g*rsqrt(v+eps) > 0 always.  htanh and the BN multiply never
    need to be materialized on the trunk.
  * sign is monotone, so sign(maxpool(u)) == maxpool(sign(u)); signs are
    computed directly from PSUM (fused bias via the ScalarE Sign activation)
    and pooling runs on +-1 bf16 values.
  * maj3_conv (majority-of-3 popcount conv) uses the identity, exact for all
    zero-padding cases that occur:
        sign(w0x0 + w1x1 + w2x2) = (w0x0 + w1x1 + w2x2 - (w0w1w2)(x0x1x2))/2
    so  maj3_conv(x, w) = 0.5*conv3x3(xb, wb)
                          - 0.5*conv3x1_vertical(Q, W3)
    where Q = horizontal triple products of the padded activations and
    W3[f,c,kh] = prod_kw wb[f,c,kh,kw].  Same identity for fcmaj3.
  * The whole binary trunk runs in bf16: +-1 and +-0.5 are exact, popcount
    partial sums accumulate in fp32 PSUM, and bf16 rounding of (u + beta)
    preserves sign.

Layouts: activations [C(part), N, H, W]; for C=64 layers the batch is split
across partition halves (p = c + 64*nh) and the two halves run as
concurrent tile_position row/col tiles of the 128x128 PE array.
"""

import os
import numpy as np
import ml_dtypes

import concourse.bass as bass
import concourse.tile as tile
from concourse import bacc, mybir
from concourse.bass_utils import run_bass_kernel_spmd
from concourse.masks import make_identity

F32 = mybir.dt.float32
BF16 = mybir.dt.bfloat16
FP8 = mybir.dt.float8e4
AX = mybir.AxisListType
ALU = mybir.AluOpType
ACT = mybir.ActivationFunctionType

EPS = 1e-5
NCORES = 8
NPC = 16  # images per core

# conv-weight table free-dim offsets inside the packed "wc" tensor
OFF_W1 = 0            # 12 taps x 64
OFF_W2 = 768          # 12 taps x 128
OFF_W3 = 768 + 1536   # 9 taps x 128
OFF_W4 = OFF_W3 + 1152  # (9 taps x 2 mb) x 128
OFF_W5 = OFF_W4 + 2304  # (2 kb x 9 taps x 2 mb) x 128
WC_COLS = OFF_W5 + 4608


def build_nc():
    nc = bacc.Bacc()

    xim_d = nc.dram_tensor("xim", [27, 2, 8, 1024], F32, kind="ExternalInput")
    wt0_d = nc.dram_tensor("wt0", [27, 64], F32, kind="ExternalInput")
    wc_d = nc.dram_tensor("wc", [128, WC_COLS], BF16, kind="ExternalInput")
    wf1_d = nc.dram_tensor("wf1", [32, 128, 512], FP8, kind="ExternalInput")
    wpx_d = nc.dram_tensor("wpx", [11, 128, 512], FP8, kind="ExternalInput")
    wf2_d = nc.dram_tensor("wf2", [4, 128, 512], FP8, kind="ExternalInput")
    wf3_d = nc.dram_tensor("wf3", [4, 128, 1024], FP8, kind="ExternalInput")
    bias_d = nc.dram_tensor("bias", [128, 8], F32, kind="ExternalInput")
    biasf_d = nc.dram_tensor("biasf", [4, 1024], F32, kind="ExternalInput")
    out_d = nc.dram_tensor("out", [16, 1000], F32, kind="ExternalOutput")
    a6d = nc.dram_tensor("a6d", [2, 128, 256], BF16)

    with tile.TileContext(nc) as tc:
        emit(nc, tc, xim_d, wt0_d, wc_d, wf1_d, wpx_d, wf2_d, wf3_d,
             bias_d, biasf_d, out_d, a6d)
    nc.compile()
    return nc


def emit(nc, tc, xim_d, wt0_d, wc_d, wf1_d, wpx_d, wf2_d, wf3_d,
         bias_d, biasf_d, out_d, a6d):
    from contextlib import ExitStack
    ctx = ExitStack()
    with ctx:
        wpool = ctx.enter_context(tc.tile_pool(name="wpool", bufs=1))
        acts = ctx.enter_context(tc.tile_pool(name="acts", bufs=1))
        ximp = ctx.enter_context(tc.tile_pool(name="ximp", bufs=2))
        qtmp = ctx.enter_context(tc.tile_pool(name="qtmp", bufs=2))
        cps = ctx.enter_context(tc.tile_pool(name="cps", bufs=4, space="PSUM"))
        fps = ctx.enter_context(tc.tile_pool(name="fps", bufs=2, space="PSUM"))
        tps = ctx.enter_context(tc.tile_pool(name="tps", bufs=2, space="PSUM"))

        # ---------------- weights needed by L0 (issue first) ----------------
        wt0 = wpool.tile([27, 64], F32, name="wt0s")
        nc.sync.dma_start(out=wt0, in_=wt0_d[:])
        bias = wpool.tile([128, 8], F32, name="biass")
        nc.sync.dma_start(out=bias, in_=bias_d[:])
        # PE warm-up: ~4.5us of dense matmuls so the HAM clock-gate opens
        # before L0 (and stays open).  Results are discarded.
        wu = wpool.tile([128, 512], BF16, name="wu")
        nc.vector.memset(wu, 1.0)
        wups = fps.tile([16, 512], F32, name="wups", tag="fc")
        for _ in range(22):
            nc.tensor.matmul(out=wups, lhsT=wu[:, 0:16], rhs=wu,
                             start=True, stop=True, skip_group_check=True)

        # ---------------- persistent activation buffers ----------------
        xpad1 = acts.tile([128, 8, 34, 34], BF16, name="xpad1")
        q1 = acts.tile([128, 8, 34, 32], BF16, name="q1")
        s1 = acts.tile([128, 8, 32, 32], BF16, name="s1")
        p1h = acts.tile([128, 8, 16, 32], BF16, name="p1h")
        xpad2 = acts.tile([128, 8, 18, 18], BF16, name="xpad2")
        q2 = acts.tile([128, 8, 18, 16], BF16, name="q2")
        xpad3 = acts.tile([128, 16, 18, 18], BF16, name="xpad3")
        s3 = acts.tile([128, 16, 16, 16], BF16, name="s3")
        p3h = acts.tile([128, 16, 8, 16], BF16, name="p3h")
        xpad4 = acts.tile([128, 16, 10, 10], BF16, name="xpad4")
        a5 = [acts.tile([128, 16, 10, 10], BF16, name=f"a5_{i}") for i in (0, 1)]
        s5 = [acts.tile([128, 16, 8, 8], BF16, name=f"s5_{i}") for i in (0, 1)]
        p5h = [acts.tile([128, 16, 4, 8], BF16, name=f"p5h_{i}") for i in (0, 1)]
        a6 = [acts.tile([128, 16, 16], BF16, name=f"a6_{i}") for i in (0, 1)]

        xbuf = acts.tile([16, 4098], BF16, name="xbuf")
        pxt1 = acts.tile([16, 1366], BF16, name="pxt1")
        px = acts.tile([16, 1408], BF16, name="px")
        pxt = acts.tile([128, 176], BF16, name="pxt")
        af2f = acts.tile([16, 512], BF16, name="af2f")
        af3f = acts.tile([16, 512], BF16, name="af3f")
        af2t = acts.tile([128, 64], BF16, name="af2t")
        af3t = acts.tile([128, 64], BF16, name="af3t")
        yb = acts.tile([16, 1024], F32, name="yb")
        mx = acts.tile([16, 1], F32, name="mx")
        negmx = acts.tile([16, 1], F32, name="negmx")
        sm = acts.tile([16, 1], F32, name="sm")
        lse = acts.tile([16, 1], F32, name="lse")
        outsb = acts.tile([16, 1000], F32, name="outsb")

        # zero only the padding halos (interiors are overwritten by drains)
        def halo_memset(eng, buf, H, W):
            eng.memset(buf[:, :, 0, :], 0.0)          # top row
            eng.memset(buf[:, :, H - 1, :], 0.0)      # bottom row
            eng.memset(buf[:, :, 1:H - 1, 0], 0.0)    # left col
            eng.memset(buf[:, :, 1:H - 1, W - 1], 0.0)  # right col
        halo_memset(nc.gpsimd, xpad1, 34, 34)
        halo_memset(nc.vector, xpad2, 18, 18)
        halo_memset(nc.gpsimd, xpad3, 18, 18)
        halo_memset(nc.vector, xpad4, 10, 10)
        halo_memset(nc.gpsimd, a5[0], 10, 10)
        halo_memset(nc.gpsimd, a5[1], 10, 10)
        nc.vector.memset(xbuf[:, 0:1], -1.0)
        nc.vector.memset(xbuf[:, 4097:4098], -1.0)
        nc.vector.memset(px[:, 1366:1408], 0.0)

        # ================ L0: conv 3->64 on real-valued x ================
        # K=27 im2col rows; two batch halves run as concurrent col tiles.
        for n8 in range(8):
            for hh in range(2):
                ximt = ximp.tile([27, 2, 512], F32, name="ximt")
                nc.sync.dma_start(out=ximt, in_=xim_d[:][:, :, n8, hh * 512:(hh + 1) * 512])
                ps = cps.tile([128, 512], F32, name="ps0", tag="cpsum")
                nc.tensor.matmul(out=ps[0:64], lhsT=wt0, rhs=ximt[:, 0],
                                 start=True, stop=True, tile_position=(0, 0))
                nc.tensor.matmul(out=ps[64:128], lhsT=wt0, rhs=ximt[:, 1],
                                 start=True, stop=True, tile_position=(0, 64),
                                 skip_group_check=True)
                psv = ps.rearrange("p (h w) -> p h w", h=16)
                nc.scalar.activation(
                    out=xpad1[:, n8, 1 + hh * 16:17 + hh * 16, 1:33],
                    in_=psv, func=ACT.Sign, bias=bias[:, 0:1])

        # ------- remaining persistent weights (issued after L0's DMAs) -------
        wc = wpool.tile([128, WC_COLS], BF16, name="wcs")
        nc.sync.dma_start(out=wc, in_=wc_d[:])
        # bias rows broadcast to 16 partitions (DVE can't take step-0 APs)
        # layout [16, 3072]: bf1(512) | bf2(512) | rs(1024) | mrs(1024)
        biasf = wpool.tile([16, 3072], F32, name="biasfs")
        bf_ap = biasf_d[:]

        def _bcast_row(dst, row, n):
            nc.sync.dma_start(out=dst, in_=bass.AP(
                tensor=bf_ap.tensor, offset=bf_ap.offset + row * 1024,
                ap=[[0, 16], [1, n]]))
        _bcast_row(biasf[:, 0:512], 0, 512)
        _bcast_row(biasf[:, 512:1024], 1, 512)
        _bcast_row(biasf[:, 1024:2048], 2, 1024)
        _bcast_row(biasf[:, 2048:3072], 3, 1024)
        ident = wpool.tile([16, 16], BF16, name="idents")
        make_identity(nc, ident)

        # Q1 = horizontal triple products of xpad1 (zeros propagate)
        for n8 in range(8):
            qt = qtmp.tile([128, 34, 32], BF16, name="qt1", tag="qt1")
            nc.vector.tensor_mul(qt, xpad1[:, n8, :, 0:32], xpad1[:, n8, :, 1:33])
            nc.vector.tensor_mul(q1[:, n8], qt, xpad1[:, n8, :, 2:34])

        # ================ L1: maj3 64->64 @32x32 + pool ================
        # 12 taps (9 conv + 3 Q); batch halves as diagonal quadrants.
        for n8 in range(8):
            for hh in range(2):
                ps = cps.tile([128, 512], F32, name="ps1", tag="cpsum")
                for t in range(12):
                    for nh in (0, 1):
                        p0 = 64 * nh
                        if t < 9:
                            kh, kw = t // 3, t % 3
                            rhs = xpad1[p0:p0 + 64, n8,
                                        kh + hh * 16:kh + hh * 16 + 16, kw:kw + 32]
                        else:
                            kh = t - 9
                            rhs = q1[p0:p0 + 64, n8,
                                     kh + hh * 16:kh + hh * 16 + 16, :]
                        nc.tensor.matmul(
                            out=ps[p0:p0 + 64], lhsT=wc[p0:p0 + 64, OFF_W1 + t * 64:OFF_W1 + (t + 1) * 64],
                            rhs=rhs, start=(t == 0), stop=(t == 11),
                            tile_position=(p0, p0),
                            skip_group_check=(nh == 1))
                psv = ps.rearrange("p (h w) -> p h w", h=16)
                nc.scalar.activation(out=s1[:, n8, hh * 16:hh * 16 + 16, :],
                                     in_=psv, func=ACT.Sign, bias=bias[:, 1:2])

        # maxpool 2x2 on signs -> write interior of xpad2 (split for pipelining)
        s1r = s1.rearrange("p n (h2 pr) w -> p n h2 pr w", pr=2)
        p1r = p1h.rearrange("p n h (w2 pr) -> p n h w2 pr", pr=2)
        for j in range(4):
            sl = slice(2 * j, 2 * j + 2)
            nc.vector.tensor_max(p1h[:, sl], s1r[:, sl, :, 0, :], s1r[:, sl, :, 1, :])
            nc.vector.tensor_max(xpad2[:, sl, 1:17, 1:17],
                                 p1r[:, sl, :, :, 0], p1r[:, sl, :, :, 1])
            for n8 in (2 * j, 2 * j + 1):
                qt = qtmp.tile([128, 18, 16], BF16, name="qt2", tag="qt2")
                nc.vector.tensor_mul(qt, xpad2[:, n8, :, 0:16], xpad2[:, n8, :, 1:17])
                nc.vector.tensor_mul(q2[:, n8], qt, xpad2[:, n8, :, 2:18])

        # ================ L2: maj3 64->128 @16x16 ================
        # batch halves as row tiles -> two separate PSUM banks.
        for j in range(4):
            psA = cps.tile([128, 512], F32, name="ps2a", tag="cpsum")
            psB = cps.tile([128, 512], F32, name="ps2b", tag="cpsum")
            for t in range(12):
                for nh, pst in ((0, psA), (1, psB)):
                    p0 = 64 * nh
                    if t < 9:
                        kh, kw = t // 3, t % 3
                        rhs = xpad2[p0:p0 + 64, 2 * j:2 * j + 2,
                                    kh:kh + 16, kw:kw + 16]
                    else:
                        kh = t - 9
                        rhs = q2[p0:p0 + 64, 2 * j:2 * j + 2, kh:kh + 16, :]
                    nc.tensor.matmul(
                        out=pst, lhsT=wc[p0:p0 + 64, OFF_W2 + t * 128:OFF_W2 + (t + 1) * 128],
                        rhs=rhs, start=(t == 0), stop=(t == 11),
                        tile_position=(p0, 0))
            for nh, pst in ((0, psA), (1, psB)):
                psv = pst.rearrange("p (n h w) -> p n h w", n=2, h=16)
                nc.scalar.activation(
                    out=xpad3[:, 8 * nh + 2 * j:8 * nh + 2 * j + 2, 1:17, 1:17],
                    in_=psv, func=ACT.Sign, bias=bias[:, 2:3])

        wf1s = wpool.tile([128, 32, 512], FP8, name="wf1s")
        nc.sync.dma_start(out=wf1s, in_=wf1_d[:].rearrange("k c f -> c k f"))
        wpxs = wpool.tile([128, 11, 512], FP8, name="wpxs")
        nc.sync.dma_start(out=wpxs, in_=wpx_d[:].rearrange("k c f -> c k f"))
        wf2s = wpool.tile([128, 4, 512], FP8, name="wf2s")
        nc.sync.dma_start(out=wf2s, in_=wf2_d[:].rearrange("k c f -> c k f"))
        wf3s = wpool.tile([128, 4, 1024], FP8, name="wf3s")
        nc.sync.dma_start(out=wf3s, in_=wf3_d[:].rearrange("k c f -> c k f"))
        # ================ L3: bin_conv 128->128 @16x16 + pool ================
        for j in range(8):
            ps = cps.tile([128, 512], F32, name="ps3", tag="cpsum")
            for t in range(9):
                kh, kw = t // 3, t % 3
                nc.tensor.matmul(
                    out=ps, lhsT=wc[:, OFF_W3 + t * 128:OFF_W3 + (t + 1) * 128],
                    rhs=xpad3[:, 2 * j:2 * j + 2, kh:kh + 16, kw:kw + 16],
                    start=(t == 0), stop=(t == 8))
            psv = ps.rearrange("p (n h w) -> p n h w", n=2, h=16)
            nc.scalar.activation(out=s3[:, 2 * j:2 * j + 2], in_=psv,
                                 func=ACT.Sign, bias=bias[:, 3:4])

        s3r = s3.rearrange("p n (h2 pr) w -> p n h2 pr w", pr=2)
        p3r = p3h.rearrange("p n h (w2 pr) -> p n h w2 pr", pr=2)
        for j in range(2):
            sl = slice(8 * j, 8 * j + 8)
            nc.vector.tensor_max(p3h[:, sl], s3r[:, sl, :, 0, :], s3r[:, sl, :, 1, :])
            nc.vector.tensor_max(xpad4[:, sl, 1:9, 1:9],
                                 p3r[:, sl, :, :, 0], p3r[:, sl, :, :, 1])

        # ================ L4: bin_conv 128->256 @8x8 ================
        for mb in range(2):
            for c8 in range(2):
                ps = cps.tile([128, 512], F32, name="ps4", tag="cpsum")
                for t in range(9):
                    kh, kw = t // 3, t % 3
                    nc.tensor.matmul(
                        out=ps,
                        lhsT=wc[:, OFF_W4 + (t * 2 + mb) * 128:OFF_W4 + (t * 2 + mb + 1) * 128],
                        rhs=xpad4[:, c8 * 8:c8 * 8 + 8, kh:kh + 8, kw:kw + 8],
                        start=(t == 0), stop=(t == 8))
                psv = ps.rearrange("p (n h w) -> p n h w", n=8, h=8)
                nc.scalar.activation(
                    out=a5[mb][:, c8 * 8:c8 * 8 + 8, 1:9, 1:9], in_=psv,
                    func=ACT.Sign, bias=bias[:, 4 + mb:5 + mb])

        # ================ L5: bin_conv 256->256 @8x8 + pool ================
        for mb in range(2):
            for c8 in range(2):
                ps = cps.tile([128, 512], F32, name="ps5", tag="cpsum")
                for kb in range(2):
                    for t in range(9):
                        kh, kw = t // 3, t % 3
                        nc.tensor.matmul(
                            out=ps,
                            lhsT=wc[:, OFF_W5 + ((kb * 9 + t) * 2 + mb) * 128:
                                    OFF_W5 + ((kb * 9 + t) * 2 + mb + 1) * 128],
                            rhs=a5[kb][:, c8 * 8:c8 * 8 + 8, kh:kh + 8, kw:kw + 8],
                            start=(kb == 0 and t == 0), stop=(kb == 1 and t == 8))
                psv = ps.rearrange("p (n h w) -> p n h w", n=8, h=8)
                nc.scalar.activation(out=s5[mb][:, c8 * 8:c8 * 8 + 8], in_=psv,
                                     func=ACT.Sign, bias=bias[:, 6 + mb:7 + mb])
        # pool + gather a6 -> X[n, d] rows (d = c*16 + hw), -1 pads at ends.
        # Gathers go on the Scalar DMA queue so they don't head-of-line
        # block the FC weight stream on the Sync queue.
        for mb in range(2):
            s5r = s5[mb].rearrange("p n (h2 pr) w -> p n h2 pr w", pr=2)
            nc.vector.tensor_max(p5h[mb], s5r[:, :, :, 0, :], s5r[:, :, :, 1, :])
            p5r = p5h[mb].rearrange("p n h (w2 pr) -> p n h w2 pr", pr=2)
            a6v = a6[mb].rearrange("p n (h w) -> p n h w", h=4)
            nc.vector.tensor_max(a6v, p5r[:, :, :, :, 0], p5r[:, :, :, :, 1])
            nc.scalar.dma_start(out=a6d[:][mb], in_=a6[mb])
            a6d_ap = a6d[:]
            nc.scalar.dma_start(
                out=xbuf[:, 1 + mb * 2048:1 + (mb + 1) * 2048],
                in_=bass.AP(tensor=a6d_ap.tensor,
                            offset=a6d_ap.offset + mb * 32768,
                            ap=[[16, 16], [256, 128], [1, 16]]))

        # ================ FC head ================

        # Px = triple products of consecutive padded features
        xr3 = xbuf.rearrange("p (g k) -> p g k", k=3)
        nc.vector.tensor_mul(pxt1, xr3[:, :, 0], xr3[:, :, 1])
        nc.vector.tensor_mul(px[:, 0:1366], pxt1, xr3[:, :, 2])

        # transpose Px -> [g, n] blocks for use as matmul lhsT
        for gb in range(11):
            txp = tps.tile([128, 16], BF16, name="txp", tag="tx")
            nc.tensor.transpose(txp, px[:, gb * 128:(gb + 1) * 128], ident)
            nc.scalar.copy(pxt[:, gb * 16:(gb + 1) * 16], txp)

        # FC1: out[n, f] accumulated over 32 feature K-groups + 11 Px groups
        psf = fps.tile([16, 512], F32, name="psf1", tag="fc")
        nmm = 43
        i = 0
        for kg in range(32):
            cb, hw = kg // 16, kg % 16
            nc.tensor.matmul(out=psf, lhsT=a6[cb][:, :, hw], rhs=wf1s[:, kg],
                             start=(i == 0), stop=(i == nmm - 1))
            i += 1
        for gb in range(11):
            nc.tensor.matmul(out=psf, lhsT=pxt[:, gb * 16:(gb + 1) * 16],
                             rhs=wpxs[:, gb],
                             start=(i == 0), stop=(i == nmm - 1))
            i += 1
        # bias + sign (beta_f1 along the free dim -> DVE add then ACT sign)
        nc.vector.tensor_add(af2f, psf, biasf[:, 0:512])
        nc.scalar.activation(out=af2f, in_=af2f, func=ACT.Sign, bias=0.0)
        for fb in range(4):
            txp = tps.tile([128, 16], BF16, name="txa", tag="tx")
            nc.tensor.transpose(txp, af2f[:, fb * 128:(fb + 1) * 128], ident)
            nc.scalar.copy(af2t[:, fb * 16:(fb + 1) * 16], txp)

        # FC2
        psf2 = fps.tile([16, 512], F32, name="psf2", tag="fc")
        for kb in range(4):
            nc.tensor.matmul(out=psf2, lhsT=af2t[:, kb * 16:(kb + 1) * 16],
                             rhs=wf2s[:, kb], start=(kb == 0), stop=(kb == 3))
        nc.vector.tensor_add(af3f, psf2, biasf[:, 512:1024])
        nc.scalar.activation(out=af3f, in_=af3f, func=ACT.Sign, bias=0.0)
        for fb in range(4):
            txp = tps.tile([128, 16], BF16, name="txb", tag="tx")
            nc.tensor.transpose(txp, af3f[:, fb * 128:(fb + 1) * 128], ident)
            nc.scalar.copy(af3t[:, fb * 16:(fb + 1) * 16], txp)

        # FC3 (+ affine-only BN)
        psh = [fps.tile([16, 512], F32, name=f"psh{i}", tag="fc") for i in (0, 1)]
        for kb in range(4):
            for half in range(2):
                nc.tensor.matmul(out=psh[half], lhsT=af3t[:, kb * 16:(kb + 1) * 16],
                                 rhs=wf3s[:, kb, half * 512:(half + 1) * 512],
                                 start=(kb == 0), stop=(kb == 3))
        for half in range(2):
            nc.vector.tensor_mul(yb[:, half * 512:(half + 1) * 512], psh[half],
                                 biasf[:, 1024 + half * 512:1024 + (half + 1) * 512])
        nc.vector.tensor_add(yb[:, 0:1000], yb[:, 0:1000], biasf[:, 2048:3048])

        # log-softmax over classes
        nc.vector.tensor_reduce(out=mx, in_=yb[:, 0:1000], axis=AX.X, op=ALU.max)
        nc.vector.tensor_scalar_mul(negmx, mx, -1.0)
        nc.scalar.activation(out=outsb, in_=yb[:, 0:1000], func=ACT.Exp, bias=negmx)
        nc.vector.tensor_reduce(out=sm, in_=outsb, axis=AX.X, op=ALU.add)
        nc.scalar.activation(out=lse, in_=sm, func=ACT.Ln, bias=0.0)
        nc.vector.tensor_scalar(out=outsb, in0=yb[:, 0:1000], scalar1=mx,
                                scalar2=lse, op0=ALU.subtract, op1=ALU.subtract)
        nc.gpsimd.dma_start(out=out_d[:], in_=outsb)


# ======================= host-side preparation =======================

def _sgn(a):
    return np.sign(np.asarray(a, np.float32)).astype(np.float32)


def _beta(bn):
    g, b, m, v = [np.asarray(t, np.float32) for t in bn]
    s = g / np.sqrt(v + EPS)
    return (b / s - m).astype(np.float32)


def prepare_weights(params):
    p = params
    w1b = _sgn(p['w1']); w2b = _sgn(p['w2']); w3b = _sgn(p['w3'])
    w4b = _sgn(p['w4']); w5b = _sgn(p['w5'])
    f1b = _sgn(p['fc1']); f2b = _sgn(p['fc2']); f3b = _sgn(p['fc3'])

    wt0 = np.zeros([27, 64], np.float32)
    w0b = _sgn(p['w0'])
    for c in range(3):
        for kh in range(3):
            for kw in range(3):
                wt0[c * 9 + kh * 3 + kw] = w0b[:, c, kh, kw]

    wc = np.zeros([128, WC_COLS], np.float32)

    def fill_maj(off, wb, F):
        W3 = wb[:, :, :, 0] * wb[:, :, :, 1] * wb[:, :, :, 2]
        for t in range(12):
            if t < 9:
                kh, kw = t // 3, t % 3
                blk = 0.5 * wb[:, :, kh, kw].T      # [c, F]
            else:
                blk = -0.5 * W3[:, :, t - 9].T
            wc[0:64, off + t * F: off + (t + 1) * F] = blk
            wc[64:128, off + t * F: off + (t + 1) * F] = blk

    fill_maj(OFF_W1, w1b, 64)
    fill_maj(OFF_W2, w2b, 128)
    for t in range(9):
        kh, kw = t // 3, t % 3
        wc[:, OFF_W3 + t * 128: OFF_W3 + (t + 1) * 128] = w3b[:, :, kh, kw].T
        for mb in range(2):
            wc[:, OFF_W4 + (t * 2 + mb) * 128: OFF_W4 + (t * 2 + mb + 1) * 128] = \
                w4b[mb * 128:(mb + 1) * 128, :, kh, kw].T
            for kb in range(2):
                wc[:, OFF_W5 + ((kb * 9 + t) * 2 + mb) * 128:
                   OFF_W5 + ((kb * 9 + t) * 2 + mb + 1) * 128] = \
                    w5b[mb * 128:(mb + 1) * 128, kb * 128:(kb + 1) * 128, kh, kw].T

    # FC1 main: wf1[kg=(cb*16+hw)][c_local, f] = 0.5*f1b[f, (cb*128+c)*16+hw+1]
    core = f1b[:, 1:4097].reshape(512, 256, 16)           # [f, c, hw]
    wf1 = np.zeros([32, 128, 512], np.float32)
    for cb in range(2):
        for hw in range(16):
            wf1[cb * 16 + hw] = 0.5 * core[:, cb * 128:(cb + 1) * 128, hw].T
    # FC1 Px term
    W3f = f1b[:, 0::3] * f1b[:, 1::3] * f1b[:, 2::3]      # [512, 1366]
    wpx = np.zeros([11, 128, 512], np.float32)
    W3p = np.zeros([512, 1408], np.float32)
    W3p[:, 0:1366] = -0.5 * W3f
    for gb in range(11):
        wpx[gb] = W3p[:, gb * 128:(gb + 1) * 128].T
    wf2 = np.zeros([4, 128, 512], np.float32)
    for kb in range(4):
        wf2[kb] = f2b[:, kb * 128:(kb + 1) * 128].T
    f3p = np.zeros([1024, 512], np.float32)
    f3p[0:1000] = f3b
    wf3 = np.zeros([4, 128, 1024], np.float32)
    for kb in range(4):
        wf3[kb] = f3p[:, kb * 128:(kb + 1) * 128].T

    bias = np.zeros([128, 8], np.float32)
    b0 = _beta(p['bn0']); b1 = _beta(p['bn1'])
    bias[:, 0] = np.concatenate([b0, b0]); bias[:, 1] = np.concatenate([b1, b1])
    bias[:, 2] = _beta(p['bn2']); bias[:, 3] = _beta(p['bn3'])
    b4 = _beta(p['bn4']); b5 = _beta(p['bn5'])
    bias[:, 4] = b4[0:128]; bias[:, 5] = b4[128:256]
    bias[:, 6] = b5[0:128]; bias[:, 7] = b5[128:256]

    biasf = np.zeros([4, 1024], np.float32)
    bf1 = _beta(p['bnf1']) - 0.5 * (f1b[:, 0] + f1b[:, 4097])
    biasf[0, 0:512] = bf1
    biasf[1, 0:512] = _beta(p['bnf2'])
    m3, v3 = [np.asarray(t, np.float32) for t in p['bnf3']]
    rs = 1.0 / np.sqrt(v3 + EPS)
    biasf[2, 0:1000] = rs
    biasf[3, 0:1000] = -m3 * rs

    bf = ml_dtypes.bfloat16
    f8 = ml_dtypes.float8_e4m3
    return {
        "wt0": wt0, "wc": wc.astype(bf), "wf1": wf1.astype(f8),
        "wpx": wpx.astype(f8), "wf2": wf2.astype(f8), "wf3": wf3.astype(f8),
        "bias": bias, "biasf": biasf,
    }


def prepare_xim(xc):
    """Per-core im2col with bf16 hi/lo split: -> [54, 2, 8, 1024] bf16."""
    xp = np.pad(np.asarray(xc, np.float32), ((0, 0), (0, 0), (1, 1), (1, 1)))
    xim = np.empty([27, 16, 1024], np.float32)
    for c in range(3):
        for kh in range(3):
            for kw in range(3):
                xim[c * 9 + kh * 3 + kw] = \
                    xp[:, c, kh:kh + 32, kw:kw + 32].reshape(16, 1024)
    return xim.reshape(27, 2, 8, 1024)


def make_in_maps(x, params):
    shared = prepare_weights(params)
    x = np.asarray(x, np.float32)
    in_maps = []
    for ci in range(NCORES):
        m = dict(shared)
        m["xim"] = prepare_xim(x[ci * NPC:(ci + 1) * NPC])
        in_maps.append(m)
    return in_maps


def kernel(x, params):
    in_maps = make_in_maps(x, params)
    nc = build_nc()
    res = run_bass_kernel_spmd(nc, in_maps, core_ids=list(range(NCORES)))
    out = np.concatenate([np.asarray(res.results[i]["out"], np.float32)
                          for i in range(NCORES)], axis=0)
    return out


# revision 38
# speedup vs baseline: 1.3502x; 1.0288x over previous
"""Trainium2 Bass kernel for the binary CNN (CNV/BNN) forward pass.

Strategy
--------
Pure data parallel: batch 128 -> 16 images per NeuronCore x 8 cores.

Math transformations (all exact, validated against the jax reference):
  * sign(htanh(bn(y))) == sign(y + beta) with beta = b/s - m, since the BN
    scale s = g*rsqrt(v+eps) > 0 always.  htanh and the BN multiply never
    need to be materialized on the trunk.
  * sign is monotone, so sign(maxpool(u)) == maxpool(sign(u)); signs are
    computed directly from PSUM (fused bias via the ScalarE Sign activation)
    and pooling runs on +-1 bf16 values.
  * maj3_conv (majority-of-3 popcount conv) uses the identity, exact for all
    zero-padding cases that occur:
        sign(w0x0 + w1x1 + w2x2) = (w0x0 + w1x1 + w2x2 - (w0w1w2)(x0x1x2))/2
    so  maj3_conv(x, w) = 0.5*conv3x3(xb, wb)
                          - 0.5*conv3x1_vertical(Q, W3)
    where Q = horizontal triple products of the padded activations and
    W3[f,c,kh] = prod_kw wb[f,c,kh,kw].  Same identity for fcmaj3.
  * The whole binary trunk runs in bf16: +-1 and +-0.5 are exact, popcount
    partial sums accumulate in fp32 PSUM, and bf16 rounding of (u + beta)
    preserves sign.

Layouts: activations [C(part), N, H, W]; for C=64 layers the batch is split
across partition halves (p = c + 64*nh) and the two halves run as
concurrent tile_position row/col tiles of the 128x128 PE array.
"""

import os
import numpy as np
import ml_dtypes

import concourse.bass as bass
import concourse.tile as tile
from concourse import bacc, mybir
from concourse.bass_utils import run_bass_kernel_spmd
from concourse.masks import make_identity

F32 = mybir.dt.float32
BF16 = mybir.dt.bfloat16
FP8 = mybir.dt.float8e4
AX = mybir.AxisListType
ALU = mybir.AluOpType
ACT = mybir.ActivationFunctionType

EPS = 1e-5
NCORES = 8
NPC = 16  # images per core

# conv-weight table free-dim offsets inside the packed "wc" tensor
OFF_W1 = 0            # 12 taps x 64
OFF_W2 = 768          # 12 taps x 128
OFF_W3 = 768 + 1536   # 9 taps x 128
OFF_W4 = OFF_W3 + 1152  # (9 taps x 2 mb) x 128
OFF_W5 = OFF_W4 + 2304  # (2 kb x 9 taps x 2 mb) x 128
WC_COLS = OFF_W5 + 4608


def build_nc():
    nc = bacc.Bacc()

    xim_d = nc.dram_tensor("xim", [27, 2, 8, 1024], F32, kind="ExternalInput")
    wt0_d = nc.dram_tensor("wt0", [27, 64], F32, kind="ExternalInput")
    wc_d = nc.dram_tensor("wc", [128, WC_COLS], BF16, kind="ExternalInput")
    wf1_d = nc.dram_tensor("wf1", [32, 128, 512], FP8, kind="ExternalInput")
    wpx_d = nc.dram_tensor("wpx", [11, 128, 512], FP8, kind="ExternalInput")
    wf2_d = nc.dram_tensor("wf2", [4, 128, 512], FP8, kind="ExternalInput")
    wf3_d = nc.dram_tensor("wf3", [4, 128, 1024], FP8, kind="ExternalInput")
    bias_d = nc.dram_tensor("bias", [128, 8], F32, kind="ExternalInput")
    biasf_d = nc.dram_tensor("biasf", [4, 1024], F32, kind="ExternalInput")
    out_d = nc.dram_tensor("out", [16, 1000], F32, kind="ExternalOutput")
    a6d = nc.dram_tensor("a6d", [2, 128, 256], BF16)

    with tile.TileContext(nc) as tc:
        emit(nc, tc, xim_d, wt0_d, wc_d, wf1_d, wpx_d, wf2_d, wf3_d,
             bias_d, biasf_d, out_d, a6d)
    nc.compile()
    return nc


def emit(nc, tc, xim_d, wt0_d, wc_d, wf1_d, wpx_d, wf2_d, wf3_d,
         bias_d, biasf_d, out_d, a6d):
    from contextlib import ExitStack
    ctx = ExitStack()
    with ctx:
        wpool = ctx.enter_context(tc.tile_pool(name="wpool", bufs=1))
        acts = ctx.enter_context(tc.tile_pool(name="acts", bufs=1))
        ximp = ctx.enter_context(tc.tile_pool(name="ximp", bufs=3))
        qtmp = ctx.enter_context(tc.tile_pool(name="qtmp", bufs=1))
        cps = ctx.enter_context(tc.tile_pool(name="cps", bufs=4, space="PSUM"))
        fps = ctx.enter_context(tc.tile_pool(name="fps", bufs=2, space="PSUM"))
        tps = ctx.enter_context(tc.tile_pool(name="tps", bufs=2, space="PSUM"))

        # ---------------- weights needed by L0 (issue first) ----------------
        wt0 = wpool.tile([27, 64], F32, name="wt0s")
        nc.sync.dma_start(out=wt0, in_=wt0_d[:])
        bias = wpool.tile([128, 8], F32, name="biass")
        nc.sync.dma_start(out=bias, in_=bias_d[:])
        # PE warm-up: ~4.5us of dense matmuls so the HAM clock-gate opens
        # before L0 (and stays open).  Results are discarded.
        wu = wpool.tile([128, 512], BF16, name="wu")
        nc.vector.memset(wu, 1.0)
        wups = fps.tile([16, 512], F32, name="wups", tag="fc")
        for _ in range(22):
            nc.tensor.matmul(out=wups, lhsT=wu[:, 0:16], rhs=wu,
                             start=True, stop=True, skip_group_check=True)

        # ---------------- persistent activation buffers ----------------
        xpad1 = acts.tile([128, 8, 34, 34], BF16, name="xpad1")
        q1 = acts.tile([128, 8, 34, 32], BF16, name="q1")
        s1 = acts.tile([128, 8, 32, 32], BF16, name="s1")
        p1h = acts.tile([128, 8, 16, 32], BF16, name="p1h")
        xpad2 = acts.tile([128, 8, 18, 18], BF16, name="xpad2")
        q2 = acts.tile([128, 8, 18, 16], BF16, name="q2")
        xpad3 = acts.tile([128, 16, 18, 18], BF16, name="xpad3")
        s3 = acts.tile([128, 16, 16, 16], BF16, name="s3")
        p3h = acts.tile([128, 16, 8, 16], BF16, name="p3h")
        xpad4 = acts.tile([128, 16, 10, 10], BF16, name="xpad4")
        a5 = [acts.tile([128, 16, 10, 10], BF16, name=f"a5_{i}") for i in (0, 1)]
        s5 = [acts.tile([128, 16, 8, 8], BF16, name=f"s5_{i}") for i in (0, 1)]
        p5h = [acts.tile([128, 16, 4, 8], BF16, name=f"p5h_{i}") for i in (0, 1)]
        a6 = [acts.tile([128, 16, 16], BF16, name=f"a6_{i}") for i in (0, 1)]

        xbuf = acts.tile([16, 4098], BF16, name="xbuf")
        px = acts.tile([16, 1408], BF16, name="px")
        pxt = acts.tile([128, 176], BF16, name="pxt")
        af2f = acts.tile([16, 512], BF16, name="af2f")
        af3f = acts.tile([16, 512], BF16, name="af3f")
        af2t = acts.tile([128, 64], BF16, name="af2t")
        af3t = acts.tile([128, 64], BF16, name="af3t")
        yb = acts.tile([16, 1024], F32, name="yb")
        mx = acts.tile([16, 1], F32, name="mx")
        negmx = acts.tile([16, 1], F32, name="negmx")
        sm = acts.tile([16, 1], F32, name="sm")
        lse = acts.tile([16, 1], F32, name="lse")
        outsb = acts.tile([16, 1000], F32, name="outsb")

        # zero only the padding halos (interiors are overwritten by drains)
        def halo_memset(eng, buf, H, W):
            eng.memset(buf[:, :, 0, :], 0.0)          # top row
            eng.memset(buf[:, :, H - 1, :], 0.0)      # bottom row
            eng.memset(buf[:, :, 1:H - 1, 0], 0.0)    # left col
            eng.memset(buf[:, :, 1:H - 1, W - 1], 0.0)  # right col
        halo_memset(nc.gpsimd, xpad1, 34, 34)
        halo_memset(nc.vector, xpad2, 18, 18)
        halo_memset(nc.gpsimd, xpad3, 18, 18)
        halo_memset(nc.vector, xpad4, 10, 10)
        halo_memset(nc.gpsimd, a5[0], 10, 10)
        halo_memset(nc.gpsimd, a5[1], 10, 10)
        nc.vector.memset(xbuf[:, 0:1], -1.0)
        nc.vector.memset(xbuf[:, 4097:4098], -1.0)
        nc.vector.memset(px[:, 1366:1408], 0.0)

        # ================ L0: conv 3->64 on real-valued x ================
        # K=27 im2col rows; two batch halves run as concurrent col tiles.
        for n8 in range(8):
            for hh in range(2):
                ximt = ximp.tile([27, 2, 512], F32, name="ximt")
                deng = nc.sync if (n8 + hh) % 2 == 0 else nc.scalar
                deng.dma_start(out=ximt, in_=xim_d[:][:, :, n8, hh * 512:(hh + 1) * 512])
                ps = cps.tile([128, 512], F32, name="ps0", tag="cpsum")
                nc.tensor.matmul(out=ps[0:64], lhsT=wt0, rhs=ximt[:, 0],
                                 start=True, stop=True, tile_position=(0, 0))
                nc.tensor.matmul(out=ps[64:128], lhsT=wt0, rhs=ximt[:, 1],
                                 start=True, stop=True, tile_position=(0, 64),
                                 skip_group_check=True)
                psv = ps.rearrange("p (h w) -> p h w", h=16)
                nc.scalar.activation(
                    out=xpad1[:, n8, 1 + hh * 16:17 + hh * 16, 1:33],
                    in_=psv, func=ACT.Sign, bias=bias[:, 0:1])

        # ------- remaining persistent weights (issued after L0's DMAs) -------
        wc = wpool.tile([128, WC_COLS], BF16, name="wcs")
        nc.sync.dma_start(out=wc, in_=wc_d[:])
        # bias rows broadcast to 16 partitions (DVE can't take step-0 APs)
        # layout [16, 3072]: bf1(512) | bf2(512) | rs(1024) | mrs(1024)
        biasf = wpool.tile([16, 3072], F32, name="biasfs")
        bf_ap = biasf_d[:]

        def _bcast_row(dst, row, n):
            nc.sync.dma_start(out=dst, in_=bass.AP(
                tensor=bf_ap.tensor, offset=bf_ap.offset + row * 1024,
                ap=[[0, 16], [1, n]]))
        _bcast_row(biasf[:, 0:512], 0, 512)
        _bcast_row(biasf[:, 512:1024], 1, 512)
        _bcast_row(biasf[:, 1024:2048], 2, 1024)
        _bcast_row(biasf[:, 2048:3072], 3, 1024)
        ident = wpool.tile([16, 16], BF16, name="idents")
        make_identity(nc, ident)

        # Q1 = horizontal triple products of xpad1 (zeros propagate)
        for n8 in range(8):
            qt = qtmp.tile([128, 34, 32], BF16, name="qt1", tag="qt1")
            nc.vector.tensor_mul(qt, xpad1[:, n8, :, 0:32], xpad1[:, n8, :, 1:33])
            nc.vector.tensor_mul(q1[:, n8], qt, xpad1[:, n8, :, 2:34])

        # ================ L1: maj3 64->64 @32x32 + pool ================
        # 12 taps (9 conv + 3 Q); batch halves as diagonal quadrants.
        for n8 in range(8):
            for hh in range(2):
                ps = cps.tile([128, 512], F32, name="ps1", tag="cpsum")
                for t in range(12):
                    for nh in (0, 1):
                        p0 = 64 * nh
                        if t < 9:
                            kh, kw = t // 3, t % 3
                            rhs = xpad1[p0:p0 + 64, n8,
                                        kh + hh * 16:kh + hh * 16 + 16, kw:kw + 32]
                        else:
                            kh = t - 9
                            rhs = q1[p0:p0 + 64, n8,
                                     kh + hh * 16:kh + hh * 16 + 16, :]
                        nc.tensor.matmul(
                            out=ps[p0:p0 + 64], lhsT=wc[p0:p0 + 64, OFF_W1 + t * 64:OFF_W1 + (t + 1) * 64],
                            rhs=rhs, start=(t == 0), stop=(t == 11),
                            tile_position=(p0, p0),
                            skip_group_check=(nh == 1))
                psv = ps.rearrange("p (h w) -> p h w", h=16)
                nc.scalar.activation(out=s1[:, n8, hh * 16:hh * 16 + 16, :],
                                     in_=psv, func=ACT.Sign, bias=bias[:, 1:2])

        # maxpool 2x2 on signs -> write interior of xpad2 (split for pipelining)
        s1r = s1.rearrange("p n (h2 pr) w -> p n h2 pr w", pr=2)
        p1r = p1h.rearrange("p n h (w2 pr) -> p n h w2 pr", pr=2)
        for j in range(4):
            sl = slice(2 * j, 2 * j + 2)
            nc.vector.tensor_max(p1h[:, sl], s1r[:, sl, :, 0, :], s1r[:, sl, :, 1, :])
            nc.vector.tensor_max(xpad2[:, sl, 1:17, 1:17],
                                 p1r[:, sl, :, :, 0], p1r[:, sl, :, :, 1])
            for n8 in (2 * j, 2 * j + 1):
                qt = qtmp.tile([128, 18, 16], BF16, name="qt2", tag="qt2")
                nc.vector.tensor_mul(qt, xpad2[:, n8, :, 0:16], xpad2[:, n8, :, 1:17])
                nc.vector.tensor_mul(q2[:, n8], qt, xpad2[:, n8, :, 2:18])

        # ================ L2: maj3 64->128 @16x16 ================
        # batch halves as row tiles -> two separate PSUM banks.
        for j in range(4):
            psA = cps.tile([128, 512], F32, name="ps2a", tag="cpsum")
            psB = cps.tile([128, 512], F32, name="ps2b", tag="cpsum")
            for t in range(12):
                for nh, pst in ((0, psA), (1, psB)):
                    p0 = 64 * nh
                    if t < 9:
                        kh, kw = t // 3, t % 3
                        rhs = xpad2[p0:p0 + 64, 2 * j:2 * j + 2,
                                    kh:kh + 16, kw:kw + 16]
                    else:
                        kh = t - 9
                        rhs = q2[p0:p0 + 64, 2 * j:2 * j + 2, kh:kh + 16, :]
                    nc.tensor.matmul(
                        out=pst, lhsT=wc[p0:p0 + 64, OFF_W2 + t * 128:OFF_W2 + (t + 1) * 128],
                        rhs=rhs, start=(t == 0), stop=(t == 11),
                        tile_position=(p0, 0))
            for nh, pst in ((0, psA), (1, psB)):
                psv = pst.rearrange("p (n h w) -> p n h w", n=2, h=16)
                nc.scalar.activation(
                    out=xpad3[:, 8 * nh + 2 * j:8 * nh + 2 * j + 2, 1:17, 1:17],
                    in_=psv, func=ACT.Sign, bias=bias[:, 2:3])

        wf1s = wpool.tile([128, 32, 512], FP8, name="wf1s")
        nc.sync.dma_start(out=wf1s, in_=wf1_d[:].rearrange("k c f -> c k f"))
        wpxs = wpool.tile([128, 11, 512], FP8, name="wpxs")
        nc.sync.dma_start(out=wpxs, in_=wpx_d[:].rearrange("k c f -> c k f"))
        wf2s = wpool.tile([128, 4, 512], FP8, name="wf2s")
        nc.sync.dma_start(out=wf2s, in_=wf2_d[:].rearrange("k c f -> c k f"))
        wf3s = wpool.tile([128, 4, 1024], FP8, name="wf3s")
        nc.sync.dma_start(out=wf3s, in_=wf3_d[:].rearrange("k c f -> c k f"))
        # ================ L3: bin_conv 128->128 @16x16 + pool ================
        for j in range(8):
            ps = cps.tile([128, 512], F32, name="ps3", tag="cpsum")
            for t in range(9):
                kh, kw = t // 3, t % 3
                nc.tensor.matmul(
                    out=ps, lhsT=wc[:, OFF_W3 + t * 128:OFF_W3 + (t + 1) * 128],
                    rhs=xpad3[:, 2 * j:2 * j + 2, kh:kh + 16, kw:kw + 16],
                    start=(t == 0), stop=(t == 8))
            psv = ps.rearrange("p (n h w) -> p n h w", n=2, h=16)
            nc.scalar.activation(out=s3[:, 2 * j:2 * j + 2], in_=psv,
                                 func=ACT.Sign, bias=bias[:, 3:4])

        s3r = s3.rearrange("p n (h2 pr) w -> p n h2 pr w", pr=2)
        p3r = p3h.rearrange("p n h (w2 pr) -> p n h w2 pr", pr=2)
        for j in range(2):
            sl = slice(8 * j, 8 * j + 8)
            nc.vector.tensor_max(p3h[:, sl], s3r[:, sl, :, 0, :], s3r[:, sl, :, 1, :])
            nc.vector.tensor_max(xpad4[:, sl, 1:9, 1:9],
                                 p3r[:, sl, :, :, 0], p3r[:, sl, :, :, 1])

        # ================ L4: bin_conv 128->256 @8x8 ================
        for mb in range(2):
            for c8 in range(2):
                ps = cps.tile([128, 512], F32, name="ps4", tag="cpsum")
                for t in range(9):
                    kh, kw = t // 3, t % 3
                    nc.tensor.matmul(
                        out=ps,
                        lhsT=wc[:, OFF_W4 + (t * 2 + mb) * 128:OFF_W4 + (t * 2 + mb + 1) * 128],
                        rhs=xpad4[:, c8 * 8:c8 * 8 + 8, kh:kh + 8, kw:kw + 8],
                        start=(t == 0), stop=(t == 8))
                psv = ps.rearrange("p (n h w) -> p n h w", n=8, h=8)
                nc.scalar.activation(
                    out=a5[mb][:, c8 * 8:c8 * 8 + 8, 1:9, 1:9], in_=psv,
                    func=ACT.Sign, bias=bias[:, 4 + mb:5 + mb])

        # ================ L5: bin_conv 256->256 @8x8 + pool ================
        for mb in range(2):
            for c8 in range(2):
                ps = cps.tile([128, 512], F32, name="ps5", tag="cpsum")
                for kb in range(2):
                    for t in range(9):
                        kh, kw = t // 3, t % 3
                        nc.tensor.matmul(
                            out=ps,
                            lhsT=wc[:, OFF_W5 + ((kb * 9 + t) * 2 + mb) * 128:
                                    OFF_W5 + ((kb * 9 + t) * 2 + mb + 1) * 128],
                            rhs=a5[kb][:, c8 * 8:c8 * 8 + 8, kh:kh + 8, kw:kw + 8],
                            start=(kb == 0 and t == 0), stop=(kb == 1 and t == 8))
                psv = ps.rearrange("p (n h w) -> p n h w", n=8, h=8)
                nc.scalar.activation(out=s5[mb][:, c8 * 8:c8 * 8 + 8], in_=psv,
                                     func=ACT.Sign, bias=bias[:, 6 + mb:7 + mb])
        # pool + gather a6 -> X[n, d] rows (d = c*16 + hw), -1 pads at ends.
        # Gathers go on the Scalar DMA queue so they don't head-of-line
        # block the FC weight stream on the Sync queue.
        for mb in range(2):
            s5r = s5[mb].rearrange("p n (h2 pr) w -> p n h2 pr w", pr=2)
            nc.vector.tensor_max(p5h[mb], s5r[:, :, :, 0, :], s5r[:, :, :, 1, :])
            p5r = p5h[mb].rearrange("p n h (w2 pr) -> p n h w2 pr", pr=2)
            a6v = a6[mb].rearrange("p n (h w) -> p n h w", h=4)
            nc.vector.tensor_max(a6v, p5r[:, :, :, :, 0], p5r[:, :, :, :, 1])
            nc.scalar.dma_start(out=a6d[:][mb], in_=a6[mb])
            a6d_ap = a6d[:]
            nc.scalar.dma_start(
                out=xbuf[:, 1 + mb * 2048:1 + (mb + 1) * 2048],
                in_=bass.AP(tensor=a6d_ap.tensor,
                            offset=a6d_ap.offset + mb * 32768,
                            ap=[[16, 16], [256, 128], [1, 16]]))

        # ================ FC head ================

        # Px = triple products of consecutive padded features, split per
        # a6-half so the first transposes can start before cb=1 lands.
        xr3 = xbuf.rearrange("p (g k) -> p g k", k=3)
        pxt1 = outsb.bitcast(BF16)[:, 0:1366]
        nc.vector.tensor_mul(pxt1[:, 0:682], xr3[:, 0:682, 0], xr3[:, 0:682, 1])
        nc.vector.tensor_mul(px[:, 0:682], pxt1[:, 0:682], xr3[:, 0:682, 2])
        nc.vector.tensor_mul(pxt1[:, 682:1366], xr3[:, 682:1366, 0], xr3[:, 682:1366, 1])
        nc.vector.tensor_mul(px[:, 682:1366], pxt1[:, 682:1366], xr3[:, 682:1366, 2])

        # transpose Px -> [g, n] blocks for use as matmul lhsT
        for gb in range(11):
            txp = tps.tile([128, 16], BF16, name="txp", tag="tx")
            nc.tensor.transpose(txp, px[:, gb * 128:(gb + 1) * 128], ident)
            nc.scalar.copy(pxt[:, gb * 16:(gb + 1) * 16], txp)

        # FC1: out[n, f] accumulated over 32 feature K-groups + 11 Px groups
        psf = fps.tile([16, 512], F32, name="psf1", tag="fc")
        nmm = 43
        i = 0
        for kg in range(32):
            cb, hw = kg // 16, kg % 16
            nc.tensor.matmul(out=psf, lhsT=a6[cb][:, :, hw], rhs=wf1s[:, kg],
                             start=(i == 0), stop=(i == nmm - 1))
            i += 1
        for gb in range(11):
            nc.tensor.matmul(out=psf, lhsT=pxt[:, gb * 16:(gb + 1) * 16],
                             rhs=wpxs[:, gb],
                             start=(i == 0), stop=(i == nmm - 1))
            i += 1
        # bias + sign (beta_f1 along the free dim -> DVE add then ACT sign)
        nc.vector.tensor_add(af2f, psf, biasf[:, 0:512])
        nc.scalar.activation(out=af2f, in_=af2f, func=ACT.Sign, bias=0.0)
        for fb in range(4):
            txp = tps.tile([128, 16], BF16, name="txa", tag="tx")
            nc.tensor.transpose(txp, af2f[:, fb * 128:(fb + 1) * 128], ident)
            nc.scalar.copy(af2t[:, fb * 16:(fb + 1) * 16], txp)

        # FC2
        psf2 = fps.tile([16, 512], F32, name="psf2", tag="fc")
        for kb in range(4):
            nc.tensor.matmul(out=psf2, lhsT=af2t[:, kb * 16:(kb + 1) * 16],
                             rhs=wf2s[:, kb], start=(kb == 0), stop=(kb == 3))
        nc.vector.tensor_add(af3f, psf2, biasf[:, 512:1024])
        nc.scalar.activation(out=af3f, in_=af3f, func=ACT.Sign, bias=0.0)
        for fb in range(4):
            txp = tps.tile([128, 16], BF16, name="txb", tag="tx")
            nc.tensor.transpose(txp, af3f[:, fb * 128:(fb + 1) * 128], ident)
            nc.scalar.copy(af3t[:, fb * 16:(fb + 1) * 16], txp)

        # FC3 (+ affine-only BN)
        psh = [fps.tile([16, 512], F32, name=f"psh{i}", tag="fc") for i in (0, 1)]
        for kb in range(4):
            for half in range(2):
                nc.tensor.matmul(out=psh[half], lhsT=af3t[:, kb * 16:(kb + 1) * 16],
                                 rhs=wf3s[:, kb, half * 512:(half + 1) * 512],
                                 start=(kb == 0), stop=(kb == 3))
        for half in range(2):
            nc.vector.tensor_mul(yb[:, half * 512:(half + 1) * 512], psh[half],
                                 biasf[:, 1024 + half * 512:1024 + (half + 1) * 512])
        nc.vector.tensor_add(yb[:, 0:1000], yb[:, 0:1000], biasf[:, 2048:3048])

        # log-softmax over classes
        nc.vector.tensor_reduce(out=mx, in_=yb[:, 0:1000], axis=AX.X, op=ALU.max)
        nc.vector.tensor_scalar_mul(negmx, mx, -1.0)
        nc.scalar.activation(out=outsb, in_=yb[:, 0:1000], func=ACT.Exp, bias=negmx)
        nc.vector.tensor_reduce(out=sm, in_=outsb, axis=AX.X, op=ALU.add)
        nc.scalar.activation(out=lse, in_=sm, func=ACT.Ln, bias=0.0)
        nc.vector.tensor_scalar(out=outsb, in0=yb[:, 0:1000], scalar1=mx,
                                scalar2=lse, op0=ALU.subtract, op1=ALU.subtract)
        nc.gpsimd.dma_start(out=out_d[:], in_=outsb)


# ======================= host-side preparation =======================

def _sgn(a):
    return np.sign(np.asarray(a, np.float32)).astype(np.float32)


def _beta(bn):
    g, b, m, v = [np.asarray(t, np.float32) for t in bn]
    s = g / np.sqrt(v + EPS)
    return (b / s - m).astype(np.float32)


def prepare_weights(params):
    p = params
    w1b = _sgn(p['w1']); w2b = _sgn(p['w2']); w3b = _sgn(p['w3'])
    w4b = _sgn(p['w4']); w5b = _sgn(p['w5'])
    f1b = _sgn(p['fc1']); f2b = _sgn(p['fc2']); f3b = _sgn(p['fc3'])

    wt0 = np.zeros([27, 64], np.float32)
    w0b = _sgn(p['w0'])
    for c in range(3):
        for kh in range(3):
            for kw in range(3):
                wt0[c * 9 + kh * 3 + kw] = w0b[:, c, kh, kw]

    wc = np.zeros([128, WC_COLS], np.float32)

    def fill_maj(off, wb, F):
        W3 = wb[:, :, :, 0] * wb[:, :, :, 1] * wb[:, :, :, 2]
        for t in range(12):
            if t < 9:
                kh, kw = t // 3, t % 3
                blk = 0.5 * wb[:, :, kh, kw].T      # [c, F]
            else:
                blk = -0.5 * W3[:, :, t - 9].T
            wc[0:64, off + t * F: off + (t + 1) * F] = blk
            wc[64:128, off + t * F: off + (t + 1) * F] = blk

    fill_maj(OFF_W1, w1b, 64)
    fill_maj(OFF_W2, w2b, 128)
    for t in range(9):
        kh, kw = t // 3, t % 3
        wc[:, OFF_W3 + t * 128: OFF_W3 + (t + 1) * 128] = w3b[:, :, kh, kw].T
        for mb in range(2):
            wc[:, OFF_W4 + (t * 2 + mb) * 128: OFF_W4 + (t * 2 + mb + 1) * 128] = \
                w4b[mb * 128:(mb + 1) * 128, :, kh, kw].T
            for kb in range(2):
                wc[:, OFF_W5 + ((kb * 9 + t) * 2 + mb) * 128:
                   OFF_W5 + ((kb * 9 + t) * 2 + mb + 1) * 128] = \
                    w5b[mb * 128:(mb + 1) * 128, kb * 128:(kb + 1) * 128, kh, kw].T

    # FC1 main: wf1[kg=(cb*16+hw)][c_local, f] = 0.5*f1b[f, (cb*128+c)*16+hw+1]
    core = f1b[:, 1:4097].reshape(512, 256, 16)           # [f, c, hw]
    wf1 = np.zeros([32, 128, 512], np.float32)
    for cb in range(2):
        for hw in range(16):
            wf1[cb * 16 + hw] = 0.5 * core[:, cb * 128:(cb + 1) * 128, hw].T
    # FC1 Px term
    W3f = f1b[:, 0::3] * f1b[:, 1::3] * f1b[:, 2::3]      # [512, 1366]
    wpx = np.zeros([11, 128, 512], np.float32)
    W3p = np.zeros([512, 1408], np.float32)
    W3p[:, 0:1366] = -0.5 * W3f
    for gb in range(11):
        wpx[gb] = W3p[:, gb * 128:(gb + 1) * 128].T
    wf2 = np.zeros([4, 128, 512], np.float32)
    for kb in range(4):
        wf2[kb] = f2b[:, kb * 128:(kb + 1) * 128].T
    f3p = np.zeros([1024, 512], np.float32)
    f3p[0:1000] = f3b
    wf3 = np.zeros([4, 128, 1024], np.float32)
    for kb in range(4):
        wf3[kb] = f3p[:, kb * 128:(kb + 1) * 128].T

    bias = np.zeros([128, 8], np.float32)
    b0 = _beta(p['bn0']); b1 = _beta(p['bn1'])
    bias[:, 0] = np.concatenate([b0, b0]); bias[:, 1] = np.concatenate([b1, b1])
    bias[:, 2] = _beta(p['bn2']); bias[:, 3] = _beta(p['bn3'])
    b4 = _beta(p['bn4']); b5 = _beta(p['bn5'])
    bias[:, 4] = b4[0:128]; bias[:, 5] = b4[128:256]
    bias[:, 6] = b5[0:128]; bias[:, 7] = b5[128:256]

    biasf = np.zeros([4, 1024], np.float32)
    bf1 = _beta(p['bnf1']) - 0.5 * (f1b[:, 0] + f1b[:, 4097])
    biasf[0, 0:512] = bf1
    biasf[1, 0:512] = _beta(p['bnf2'])
    m3, v3 = [np.asarray(t, np.float32) for t in p['bnf3']]
    rs = 1.0 / np.sqrt(v3 + EPS)
    biasf[2, 0:1000] = rs
    biasf[3, 0:1000] = -m3 * rs

    bf = ml_dtypes.bfloat16
    f8 = ml_dtypes.float8_e4m3
    return {
        "wt0": wt0, "wc": wc.astype(bf), "wf1": wf1.astype(f8),
        "wpx": wpx.astype(f8), "wf2": wf2.astype(f8), "wf3": wf3.astype(f8),
        "bias": bias, "biasf": biasf,
    }


def prepare_xim(xc):
    """Per-core im2col with bf16 hi/lo split: -> [54, 2, 8, 1024] bf16."""
    xp = np.pad(np.asarray(xc, np.float32), ((0, 0), (0, 0), (1, 1), (1, 1)))
    xim = np.empty([27, 16, 1024], np.float32)
    for c in range(3):
        for kh in range(3):
            for kw in range(3):
                xim[c * 9 + kh * 3 + kw] = \
                    xp[:, c, kh:kh + 32, kw:kw + 32].reshape(16, 1024)
    return xim.reshape(27, 2, 8, 1024)


def make_in_maps(x, params):
    shared = prepare_weights(params)
    x = np.asarray(x, np.float32)
    in_maps = []
    for ci in range(NCORES):
        m = dict(shared)
        m["xim"] = prepare_xim(x[ci * NPC:(ci + 1) * NPC])
        in_maps.append(m)
    return in_maps


def kernel(x, params):
    in_maps = make_in_maps(x, params)
    nc = build_nc()
    res = run_bass_kernel_spmd(nc, in_maps, core_ids=list(range(NCORES)))
    out = np.concatenate([np.asarray(res.results[i]["out"], np.float32)
                          for i in range(NCORES)], axis=0)
    return out


# revision 39
# speedup vs baseline: 1.3754x; 1.0187x over previous
"""Trainium2 Bass kernel for the binary CNN (CNV/BNN) forward pass.

Strategy
--------
Pure data parallel: batch 128 -> 16 images per NeuronCore x 8 cores.

Math transformations (all exact, validated against the jax reference):
  * sign(htanh(bn(y))) == sign(y + beta) with beta = b/s - m, since the BN
    scale s = g*rsqrt(v+eps) > 0 always.  htanh and the BN multiply never
    need to be materialized on the trunk.
  * sign is monotone, so sign(maxpool(u)) == maxpool(sign(u)); signs are
    computed directly from PSUM (fused bias via the ScalarE Sign activation)
    and pooling runs on +-1 bf16 values.
  * maj3_conv (majority-of-3 popcount conv) uses the identity, exact for all
    zero-padding cases that occur:
        sign(w0x0 + w1x1 + w2x2) = (w0x0 + w1x1 + w2x2 - (w0w1w2)(x0x1x2))/2
    so  maj3_conv(x, w) = 0.5*conv3x3(xb, wb)
                          - 0.5*conv3x1_vertical(Q, W3)
    where Q = horizontal triple products of the padded activations and
    W3[f,c,kh] = prod_kw wb[f,c,kh,kw].  Same identity for fcmaj3.
  * The whole binary trunk runs in bf16: +-1 and +-0.5 are exact, popcount
    partial sums accumulate in fp32 PSUM, and bf16 rounding of (u + beta)
    preserves sign.

Layouts: activations [C(part), N, H, W]; for C=64 layers the batch is split
across partition halves (p = c + 64*nh) and the two halves run as
concurrent tile_position row/col tiles of the 128x128 PE array.
"""

import os
import numpy as np
import ml_dtypes

import concourse.bass as bass
import concourse.tile as tile
from concourse import bacc, mybir
from concourse.bass_utils import run_bass_kernel_spmd
from concourse.masks import make_identity

F32 = mybir.dt.float32
BF16 = mybir.dt.bfloat16
FP8 = mybir.dt.float8e4
AX = mybir.AxisListType
ALU = mybir.AluOpType
ACT = mybir.ActivationFunctionType

EPS = 1e-5
NCORES = 8
NPC = 16  # images per core

# conv-weight table free-dim offsets inside the packed "wc" tensor
OFF_W1 = 0            # 12 taps x 64
OFF_W2 = 768          # 12 taps x 128
OFF_W3 = 768 + 1536   # 9 taps x 128
OFF_W4 = OFF_W3 + 1152  # (9 taps x 2 mb) x 128
OFF_W5 = OFF_W4 + 2304  # (2 kb x 9 taps x 2 mb) x 128
WC_COLS = OFF_W5 + 4608


def build_nc():
    nc = bacc.Bacc()

    xim_d = nc.dram_tensor("xim", [81, 2, 8, 1024], BF16, kind="ExternalInput")
    wt0_d = nc.dram_tensor("wt0", [81, 64], BF16, kind="ExternalInput")
    wc_d = nc.dram_tensor("wc", [128, WC_COLS], BF16, kind="ExternalInput")
    wf1_d = nc.dram_tensor("wf1", [32, 128, 512], FP8, kind="ExternalInput")
    wpx_d = nc.dram_tensor("wpx", [11, 128, 512], FP8, kind="ExternalInput")
    wf2_d = nc.dram_tensor("wf2", [4, 128, 512], FP8, kind="ExternalInput")
    wf3_d = nc.dram_tensor("wf3", [4, 128, 1024], FP8, kind="ExternalInput")
    bias_d = nc.dram_tensor("bias", [128, 8], F32, kind="ExternalInput")
    biasf_d = nc.dram_tensor("biasf", [4, 1024], F32, kind="ExternalInput")
    out_d = nc.dram_tensor("out", [16, 1000], F32, kind="ExternalOutput")
    a6d = nc.dram_tensor("a6d", [2, 128, 256], BF16)

    with tile.TileContext(nc) as tc:
        emit(nc, tc, xim_d, wt0_d, wc_d, wf1_d, wpx_d, wf2_d, wf3_d,
             bias_d, biasf_d, out_d, a6d)
    nc.compile()
    return nc


def emit(nc, tc, xim_d, wt0_d, wc_d, wf1_d, wpx_d, wf2_d, wf3_d,
         bias_d, biasf_d, out_d, a6d):
    from contextlib import ExitStack
    ctx = ExitStack()
    with ctx:
        wpool = ctx.enter_context(tc.tile_pool(name="wpool", bufs=1))
        acts = ctx.enter_context(tc.tile_pool(name="acts", bufs=1))
        ximp = ctx.enter_context(tc.tile_pool(name="ximp", bufs=3))
        qtmp = ctx.enter_context(tc.tile_pool(name="qtmp", bufs=1))
        cps = ctx.enter_context(tc.tile_pool(name="cps", bufs=4, space="PSUM"))
        fps = ctx.enter_context(tc.tile_pool(name="fps", bufs=2, space="PSUM"))
        tps = ctx.enter_context(tc.tile_pool(name="tps", bufs=2, space="PSUM"))

        # ---------------- weights needed by L0 (issue first) ----------------
        wt0 = wpool.tile([81, 64], BF16, name="wt0s")
        nc.sync.dma_start(out=wt0, in_=wt0_d[:])
        bias = wpool.tile([128, 8], F32, name="biass")
        nc.sync.dma_start(out=bias, in_=bias_d[:])
        # PE warm-up: ~4.5us of dense matmuls so the HAM clock-gate opens
        # before L0 (and stays open).  Results are discarded.
        wu = wpool.tile([128, 512], BF16, name="wu")
        nc.vector.memset(wu, 1.0)
        wups = fps.tile([16, 512], F32, name="wups", tag="fc")
        for _ in range(22):
            nc.tensor.matmul(out=wups, lhsT=wu[:, 0:16], rhs=wu,
                             start=True, stop=True, skip_group_check=True)

        # ---------------- persistent activation buffers ----------------
        xpad1 = acts.tile([128, 8, 34, 34], BF16, name="xpad1")
        q1 = acts.tile([128, 8, 34, 32], BF16, name="q1")
        s1 = acts.tile([128, 8, 32, 32], BF16, name="s1")
        p1h = acts.tile([128, 8, 16, 32], BF16, name="p1h")
        xpad2 = acts.tile([128, 8, 18, 18], BF16, name="xpad2")
        q2 = acts.tile([128, 8, 18, 16], BF16, name="q2")
        xpad3 = acts.tile([128, 16, 18, 18], BF16, name="xpad3")
        s3 = acts.tile([128, 16, 16, 16], BF16, name="s3")
        p3h = acts.tile([128, 16, 8, 16], BF16, name="p3h")
        xpad4 = acts.tile([128, 16, 10, 10], BF16, name="xpad4")
        a5 = [acts.tile([128, 16, 10, 10], BF16, name=f"a5_{i}") for i in (0, 1)]
        s5 = [acts.tile([128, 16, 8, 8], BF16, name=f"s5_{i}") for i in (0, 1)]
        p5h = [acts.tile([128, 16, 4, 8], BF16, name=f"p5h_{i}") for i in (0, 1)]
        a6 = [acts.tile([128, 16, 16], BF16, name=f"a6_{i}") for i in (0, 1)]

        xbuf = acts.tile([16, 4098], BF16, name="xbuf")
        px = acts.tile([16, 1408], BF16, name="px")
        pxt = acts.tile([128, 176], BF16, name="pxt")
        af2f = acts.tile([16, 512], BF16, name="af2f")
        af3f = acts.tile([16, 512], BF16, name="af3f")
        af2t = acts.tile([128, 64], BF16, name="af2t")
        af3t = acts.tile([128, 64], BF16, name="af3t")
        yb = acts.tile([16, 1024], F32, name="yb")
        negmx = acts.tile([16, 1], F32, name="negmx")
        sm = acts.tile([16, 1], F32, name="sm")
        lse = acts.tile([16, 1], F32, name="lse")
        outsb = acts.tile([16, 1000], F32, name="outsb")

        # zero only the padding halos (interiors are overwritten by drains)
        def halo_memset(eng, buf, H, W):
            eng.memset(buf[:, :, 0, :], 0.0)          # top row
            eng.memset(buf[:, :, H - 1, :], 0.0)      # bottom row
            eng.memset(buf[:, :, 1:H - 1, 0], 0.0)    # left col
            eng.memset(buf[:, :, 1:H - 1, W - 1], 0.0)  # right col
        halo_memset(nc.gpsimd, xpad1, 34, 34)
        halo_memset(nc.vector, xpad2, 18, 18)
        halo_memset(nc.gpsimd, xpad3, 18, 18)
        halo_memset(nc.vector, xpad4, 10, 10)
        halo_memset(nc.gpsimd, a5[0], 10, 10)
        halo_memset(nc.gpsimd, a5[1], 10, 10)
        nc.vector.memset(xbuf[:, 0:1], -1.0)
        nc.vector.memset(xbuf[:, 4097:4098], -1.0)
        nc.vector.memset(px[:, 1366:1408], 0.0)

        # ================ L0: conv 3->64 on real-valued x ================
        # K=27 im2col rows; two batch halves run as concurrent col tiles.
        for n8 in range(8):
            for hh in range(2):
                ximt = ximp.tile([81, 2, 512], BF16, name="ximt")
                deng = nc.sync if (n8 + hh) % 2 == 0 else nc.scalar
                deng.dma_start(out=ximt, in_=xim_d[:][:, :, n8, hh * 512:(hh + 1) * 512])
                ps = cps.tile([128, 512], F32, name="ps0", tag="cpsum")
                nc.tensor.matmul(out=ps[0:64], lhsT=wt0, rhs=ximt[:, 0],
                                 start=True, stop=True, tile_position=(0, 0))
                nc.tensor.matmul(out=ps[64:128], lhsT=wt0, rhs=ximt[:, 1],
                                 start=True, stop=True, tile_position=(0, 64),
                                 skip_group_check=True)
                psv = ps.rearrange("p (h w) -> p h w", h=16)
                nc.scalar.activation(
                    out=xpad1[:, n8, 1 + hh * 16:17 + hh * 16, 1:33],
                    in_=psv, func=ACT.Sign, bias=bias[:, 0:1])

        # ------- remaining persistent weights (issued after L0's DMAs) -------
        wc = wpool.tile([128, WC_COLS], BF16, name="wcs")
        nc.sync.dma_start(out=wc, in_=wc_d[:])
        # bias rows broadcast to 16 partitions (DVE can't take step-0 APs)
        # layout [16, 3072]: bf1(512) | bf2(512) | rs(1024) | mrs(1024)
        biasf = wpool.tile([16, 3072], F32, name="biasfs")
        bf_ap = biasf_d[:]

        def _bcast_row(dst, row, n):
            nc.sync.dma_start(out=dst, in_=bass.AP(
                tensor=bf_ap.tensor, offset=bf_ap.offset + row * 1024,
                ap=[[0, 16], [1, n]]))
        _bcast_row(biasf[:, 0:512], 0, 512)
        _bcast_row(biasf[:, 512:1024], 1, 512)
        _bcast_row(biasf[:, 1024:2048], 2, 1024)
        _bcast_row(biasf[:, 2048:3072], 3, 1024)
        ident = wpool.tile([16, 16], BF16, name="idents")
        make_identity(nc, ident)

        # Q1 = horizontal triple products of xpad1 (zeros propagate)
        for n8 in range(8):
            qt = qtmp.tile([128, 34, 32], BF16, name="qt1", tag="qt1")
            nc.vector.tensor_mul(qt, xpad1[:, n8, :, 0:32], xpad1[:, n8, :, 1:33])
            nc.vector.tensor_mul(q1[:, n8], qt, xpad1[:, n8, :, 2:34])

        # ================ L1: maj3 64->64 @32x32 + pool ================
        # 12 taps (9 conv + 3 Q); batch halves as diagonal quadrants.
        for n8 in range(8):
            for hh in range(2):
                ps = cps.tile([128, 512], F32, name="ps1", tag="cpsum")
                for t in range(12):
                    for nh in (0, 1):
                        p0 = 64 * nh
                        if t < 9:
                            kh, kw = t // 3, t % 3
                            rhs = xpad1[p0:p0 + 64, n8,
                                        kh + hh * 16:kh + hh * 16 + 16, kw:kw + 32]
                        else:
                            kh = t - 9
                            rhs = q1[p0:p0 + 64, n8,
                                     kh + hh * 16:kh + hh * 16 + 16, :]
                        nc.tensor.matmul(
                            out=ps[p0:p0 + 64], lhsT=wc[p0:p0 + 64, OFF_W1 + t * 64:OFF_W1 + (t + 1) * 64],
                            rhs=rhs, start=(t == 0), stop=(t == 11),
                            tile_position=(p0, p0),
                            skip_group_check=(nh == 1))
                psv = ps.rearrange("p (h w) -> p h w", h=16)
                nc.scalar.activation(out=s1[:, n8, hh * 16:hh * 16 + 16, :],
                                     in_=psv, func=ACT.Sign, bias=bias[:, 1:2])

        # maxpool 2x2 on signs -> write interior of xpad2 (split for pipelining)
        s1r = s1.rearrange("p n (h2 pr) w -> p n h2 pr w", pr=2)
        p1r = p1h.rearrange("p n h (w2 pr) -> p n h w2 pr", pr=2)
        for j in range(4):
            sl = slice(2 * j, 2 * j + 2)
            nc.vector.tensor_max(p1h[:, sl], s1r[:, sl, :, 0, :], s1r[:, sl, :, 1, :])
            nc.vector.tensor_max(xpad2[:, sl, 1:17, 1:17],
                                 p1r[:, sl, :, :, 0], p1r[:, sl, :, :, 1])
            for n8 in (2 * j, 2 * j + 1):
                qt = qtmp.tile([128, 18, 16], BF16, name="qt2", tag="qt2")
                nc.vector.tensor_mul(qt, xpad2[:, n8, :, 0:16], xpad2[:, n8, :, 1:17])
                nc.vector.tensor_mul(q2[:, n8], qt, xpad2[:, n8, :, 2:18])

        # ================ L2: maj3 64->128 @16x16 ================
        # batch halves as row tiles -> two separate PSUM banks.
        for j in range(4):
            psA = cps.tile([128, 512], F32, name="ps2a", tag="cpsum")
            psB = cps.tile([128, 512], F32, name="ps2b", tag="cpsum")
            for t in range(12):
                for nh, pst in ((0, psA), (1, psB)):
                    p0 = 64 * nh
                    if t < 9:
                        kh, kw = t // 3, t % 3
                        rhs = xpad2[p0:p0 + 64, 2 * j:2 * j + 2,
                                    kh:kh + 16, kw:kw + 16]
                    else:
                        kh = t - 9
                        rhs = q2[p0:p0 + 64, 2 * j:2 * j + 2, kh:kh + 16, :]
                    nc.tensor.matmul(
                        out=pst, lhsT=wc[p0:p0 + 64, OFF_W2 + t * 128:OFF_W2 + (t + 1) * 128],
                        rhs=rhs, start=(t == 0), stop=(t == 11),
                        tile_position=(p0, 0))
            for nh, pst in ((0, psA), (1, psB)):
                psv = pst.rearrange("p (n h w) -> p n h w", n=2, h=16)
                nc.scalar.activation(
                    out=xpad3[:, 8 * nh + 2 * j:8 * nh + 2 * j + 2, 1:17, 1:17],
                    in_=psv, func=ACT.Sign, bias=bias[:, 2:3])

        wf1s = wpool.tile([128, 32, 512], FP8, name="wf1s")
        nc.sync.dma_start(out=wf1s, in_=wf1_d[:].rearrange("k c f -> c k f"))
        wpxs = wpool.tile([128, 11, 512], FP8, name="wpxs")
        nc.sync.dma_start(out=wpxs, in_=wpx_d[:].rearrange("k c f -> c k f"))
        wf2s = wpool.tile([128, 4, 512], FP8, name="wf2s")
        nc.sync.dma_start(out=wf2s, in_=wf2_d[:].rearrange("k c f -> c k f"))
        wf3s = wpool.tile([128, 4, 1024], FP8, name="wf3s")
        nc.sync.dma_start(out=wf3s, in_=wf3_d[:].rearrange("k c f -> c k f"))
        # ================ L3: bin_conv 128->128 @16x16 + pool ================
        for j in range(8):
            ps = cps.tile([128, 512], F32, name="ps3", tag="cpsum")
            for t in range(9):
                kh, kw = t // 3, t % 3
                nc.tensor.matmul(
                    out=ps, lhsT=wc[:, OFF_W3 + t * 128:OFF_W3 + (t + 1) * 128],
                    rhs=xpad3[:, 2 * j:2 * j + 2, kh:kh + 16, kw:kw + 16],
                    start=(t == 0), stop=(t == 8))
            psv = ps.rearrange("p (n h w) -> p n h w", n=2, h=16)
            nc.scalar.activation(out=s3[:, 2 * j:2 * j + 2], in_=psv,
                                 func=ACT.Sign, bias=bias[:, 3:4])

        s3r = s3.rearrange("p n (h2 pr) w -> p n h2 pr w", pr=2)
        p3r = p3h.rearrange("p n h (w2 pr) -> p n h w2 pr", pr=2)
        for j in range(2):
            sl = slice(8 * j, 8 * j + 8)
            nc.vector.tensor_max(p3h[:, sl], s3r[:, sl, :, 0, :], s3r[:, sl, :, 1, :])
            nc.vector.tensor_max(xpad4[:, sl, 1:9, 1:9],
                                 p3r[:, sl, :, :, 0], p3r[:, sl, :, :, 1])

        # ================ L4: bin_conv 128->256 @8x8 ================
        for mb in range(2):
            for c8 in range(2):
                ps = cps.tile([128, 512], F32, name="ps4", tag="cpsum")
                for t in range(9):
                    kh, kw = t // 3, t % 3
                    nc.tensor.matmul(
                        out=ps,
                        lhsT=wc[:, OFF_W4 + (t * 2 + mb) * 128:OFF_W4 + (t * 2 + mb + 1) * 128],
                        rhs=xpad4[:, c8 * 8:c8 * 8 + 8, kh:kh + 8, kw:kw + 8],
                        start=(t == 0), stop=(t == 8))
                psv = ps.rearrange("p (n h w) -> p n h w", n=8, h=8)
                nc.scalar.activation(
                    out=a5[mb][:, c8 * 8:c8 * 8 + 8, 1:9, 1:9], in_=psv,
                    func=ACT.Sign, bias=bias[:, 4 + mb:5 + mb])

        # ================ L5: bin_conv 256->256 @8x8 + pool ================
        for mb in range(2):
            for c8 in range(2):
                ps = cps.tile([128, 512], F32, name="ps5", tag="cpsum")
                for kb in range(2):
                    for t in range(9):
                        kh, kw = t // 3, t % 3
                        nc.tensor.matmul(
                            out=ps,
                            lhsT=wc[:, OFF_W5 + ((kb * 9 + t) * 2 + mb) * 128:
                                    OFF_W5 + ((kb * 9 + t) * 2 + mb + 1) * 128],
                            rhs=a5[kb][:, c8 * 8:c8 * 8 + 8, kh:kh + 8, kw:kw + 8],
                            start=(kb == 0 and t == 0), stop=(kb == 1 and t == 8))
                psv = ps.rearrange("p (n h w) -> p n h w", n=8, h=8)
                nc.scalar.activation(out=s5[mb][:, c8 * 8:c8 * 8 + 8], in_=psv,
                                     func=ACT.Sign, bias=bias[:, 6 + mb:7 + mb])
        # pool + gather a6 -> X[n, d] rows (d = c*16 + hw), -1 pads at ends.
        # Gathers go on the Scalar DMA queue so they don't head-of-line
        # block the FC weight stream on the Sync queue.
        for mb in range(2):
            s5r = s5[mb].rearrange("p n (h2 pr) w -> p n h2 pr w", pr=2)
            nc.vector.tensor_max(p5h[mb], s5r[:, :, :, 0, :], s5r[:, :, :, 1, :])
            p5r = p5h[mb].rearrange("p n h (w2 pr) -> p n h w2 pr", pr=2)
            a6v = a6[mb].rearrange("p n (h w) -> p n h w", h=4)
            nc.vector.tensor_max(a6v, p5r[:, :, :, :, 0], p5r[:, :, :, :, 1])
            nc.scalar.dma_start(out=a6d[:][mb], in_=a6[mb])
            a6d_ap = a6d[:]
            nc.scalar.dma_start(
                out=xbuf[:, 1 + mb * 2048:1 + (mb + 1) * 2048],
                in_=bass.AP(tensor=a6d_ap.tensor,
                            offset=a6d_ap.offset + mb * 32768,
                            ap=[[16, 16], [256, 128], [1, 16]]))

        # ================ FC head ================

        # Px = triple products of consecutive padded features, split per
        # a6-half so the first transposes can start before cb=1 lands.
        xr3 = xbuf.rearrange("p (g k) -> p g k", k=3)
        pxt1 = outsb.bitcast(BF16)[:, 0:1366]
        nc.vector.tensor_mul(pxt1[:, 0:682], xr3[:, 0:682, 0], xr3[:, 0:682, 1])
        nc.vector.tensor_mul(px[:, 0:682], pxt1[:, 0:682], xr3[:, 0:682, 2])
        nc.vector.tensor_mul(pxt1[:, 682:1366], xr3[:, 682:1366, 0], xr3[:, 682:1366, 1])
        nc.vector.tensor_mul(px[:, 682:1366], pxt1[:, 682:1366], xr3[:, 682:1366, 2])

        # transpose Px -> [g, n] blocks for use as matmul lhsT
        for gb in range(11):
            txp = tps.tile([128, 16], BF16, name="txp", tag="tx")
            nc.tensor.transpose(txp, px[:, gb * 128:(gb + 1) * 128], ident)
            nc.scalar.copy(pxt[:, gb * 16:(gb + 1) * 16], txp)

        # FC1: out[n, f] accumulated over 32 feature K-groups + 11 Px groups
        psf = fps.tile([16, 512], F32, name="psf1", tag="fc")
        nmm = 43
        i = 0
        for kg in range(32):
            cb, hw = kg // 16, kg % 16
            nc.tensor.matmul(out=psf, lhsT=a6[cb][:, :, hw], rhs=wf1s[:, kg],
                             start=(i == 0), stop=(i == nmm - 1))
            i += 1
        # keep the PE clock-gate open while the Px chain lands
        wudum = fps.tile([16, 512], F32, name="wudum", tag="fc")
        for _ in range(16):
            nc.tensor.matmul(out=wudum, lhsT=wu[:, 0:16], rhs=wu,
                             start=True, stop=True, skip_group_check=True)
        for gb in range(11):
            nc.tensor.matmul(out=psf, lhsT=pxt[:, gb * 16:(gb + 1) * 16],
                             rhs=wpxs[:, gb],
                             start=(i == 0), stop=(i == nmm - 1))
            i += 1
        # bias + sign (beta_f1 along the free dim -> DVE add then ACT sign)
        nc.vector.tensor_add(af2f, psf, biasf[:, 0:512])
        nc.scalar.activation(out=af2f, in_=af2f, func=ACT.Sign, bias=0.0)
        for fb in range(4):
            txp = tps.tile([128, 16], BF16, name="txa", tag="tx")
            nc.tensor.transpose(txp, af2f[:, fb * 128:(fb + 1) * 128], ident)
            nc.scalar.copy(af2t[:, fb * 16:(fb + 1) * 16], txp)

        # FC2
        psf2 = fps.tile([16, 512], F32, name="psf2", tag="fc")
        for kb in range(4):
            nc.tensor.matmul(out=psf2, lhsT=af2t[:, kb * 16:(kb + 1) * 16],
                             rhs=wf2s[:, kb], start=(kb == 0), stop=(kb == 3))
        nc.vector.tensor_add(af3f, psf2, biasf[:, 512:1024])
        nc.scalar.activation(out=af3f, in_=af3f, func=ACT.Sign, bias=0.0)
        for fb in range(4):
            txp = tps.tile([128, 16], BF16, name="txb", tag="tx")
            nc.tensor.transpose(txp, af3f[:, fb * 128:(fb + 1) * 128], ident)
            nc.scalar.copy(af3t[:, fb * 16:(fb + 1) * 16], txp)

        # FC3 (+ affine-only BN)
        psh = [fps.tile([16, 512], F32, name=f"psh{i}", tag="fc") for i in (0, 1)]
        for kb in range(4):
            for half in range(2):
                nc.tensor.matmul(out=psh[half], lhsT=af3t[:, kb * 16:(kb + 1) * 16],
                                 rhs=wf3s[:, kb, half * 512:(half + 1) * 512],
                                 start=(kb == 0), stop=(kb == 3))
        for half in range(2):
            nc.vector.tensor_mul(yb[:, half * 512:(half + 1) * 512], psh[half],
                                 biasf[:, 1024 + half * 512:1024 + (half + 1) * 512])
        nc.vector.tensor_add(yb[:, 0:1000], yb[:, 0:1000], biasf[:, 2048:3048])

        # log-softmax over classes
        nc.vector.tensor_reduce(out=negmx, in_=yb[:, 0:1000], axis=AX.X,
                                op=ALU.max, negate=True)
        nc.scalar.activation(out=outsb, in_=yb[:, 0:1000], func=ACT.Exp, bias=negmx)
        nc.vector.tensor_reduce(out=sm, in_=outsb, axis=AX.X, op=ALU.add)
        nc.scalar.activation(out=lse, in_=sm, func=ACT.Ln, bias=0.0)
        nc.vector.tensor_scalar(out=outsb, in0=yb[:, 0:1000], scalar1=negmx,
                                scalar2=lse, op0=ALU.add, op1=ALU.subtract)
        nc.gpsimd.dma_start(out=out_d[:], in_=outsb)


# ======================= host-side preparation =======================

def _sgn(a):
    return np.sign(np.asarray(a, np.float32)).astype(np.float32)


def _beta(bn):
    g, b, m, v = [np.asarray(t, np.float32) for t in bn]
    s = g / np.sqrt(v + EPS)
    return (b / s - m).astype(np.float32)


def prepare_weights(params):
    p = params
    w1b = _sgn(p['w1']); w2b = _sgn(p['w2']); w3b = _sgn(p['w3'])
    w4b = _sgn(p['w4']); w5b = _sgn(p['w5'])
    f1b = _sgn(p['fc1']); f2b = _sgn(p['fc2']); f3b = _sgn(p['fc3'])

    wt0 = np.zeros([81, 64], np.float32)
    w0b = _sgn(p['w0'])
    for c in range(3):
        for kh in range(3):
            for kw in range(3):
                wt0[c * 9 + kh * 3 + kw] = w0b[:, c, kh, kw]
    wt0[27:54] = wt0[0:27]
    wt0[54:81] = wt0[0:27]

    wc = np.zeros([128, WC_COLS], np.float32)

    def fill_maj(off, wb, F):
        W3 = wb[:, :, :, 0] * wb[:, :, :, 1] * wb[:, :, :, 2]
        for t in range(12):
            if t < 9:
                kh, kw = t // 3, t % 3
                blk = 0.5 * wb[:, :, kh, kw].T      # [c, F]
            else:
                blk = -0.5 * W3[:, :, t - 9].T
            wc[0:64, off + t * F: off + (t + 1) * F] = blk
            wc[64:128, off + t * F: off + (t + 1) * F] = blk

    fill_maj(OFF_W1, w1b, 64)
    fill_maj(OFF_W2, w2b, 128)
    for t in range(9):
        kh, kw = t // 3, t % 3
        wc[:, OFF_W3 + t * 128: OFF_W3 + (t + 1) * 128] = w3b[:, :, kh, kw].T
        for mb in range(2):
            wc[:, OFF_W4 + (t * 2 + mb) * 128: OFF_W4 + (t * 2 + mb + 1) * 128] = \
                w4b[mb * 128:(mb + 1) * 128, :, kh, kw].T
            for kb in range(2):
                wc[:, OFF_W5 + ((kb * 9 + t) * 2 + mb) * 128:
                   OFF_W5 + ((kb * 9 + t) * 2 + mb + 1) * 128] = \
                    w5b[mb * 128:(mb + 1) * 128, kb * 128:(kb + 1) * 128, kh, kw].T

    # FC1 main: wf1[kg=(cb*16+hw)][c_local, f] = 0.5*f1b[f, (cb*128+c)*16+hw+1]
    core = f1b[:, 1:4097].reshape(512, 256, 16)           # [f, c, hw]
    wf1 = np.zeros([32, 128, 512], np.float32)
    for cb in range(2):
        for hw in range(16):
            wf1[cb * 16 + hw] = 0.5 * core[:, cb * 128:(cb + 1) * 128, hw].T
    # FC1 Px term
    W3f = f1b[:, 0::3] * f1b[:, 1::3] * f1b[:, 2::3]      # [512, 1366]
    wpx = np.zeros([11, 128, 512], np.float32)
    W3p = np.zeros([512, 1408], np.float32)
    W3p[:, 0:1366] = -0.5 * W3f
    for gb in range(11):
        wpx[gb] = W3p[:, gb * 128:(gb + 1) * 128].T
    wf2 = np.zeros([4, 128, 512], np.float32)
    for kb in range(4):
        wf2[kb] = f2b[:, kb * 128:(kb + 1) * 128].T
    f3p = np.zeros([1024, 512], np.float32)
    f3p[0:1000] = f3b
    wf3 = np.zeros([4, 128, 1024], np.float32)
    for kb in range(4):
        wf3[kb] = f3p[:, kb * 128:(kb + 1) * 128].T

    bias = np.zeros([128, 8], np.float32)
    b0 = _beta(p['bn0']); b1 = _beta(p['bn1'])
    bias[:, 0] = np.concatenate([b0, b0]); bias[:, 1] = np.concatenate([b1, b1])
    bias[:, 2] = _beta(p['bn2']); bias[:, 3] = _beta(p['bn3'])
    b4 = _beta(p['bn4']); b5 = _beta(p['bn5'])
    bias[:, 4] = b4[0:128]; bias[:, 5] = b4[128:256]
    bias[:, 6] = b5[0:128]; bias[:, 7] = b5[128:256]

    biasf = np.zeros([4, 1024], np.float32)
    bf1 = _beta(p['bnf1']) - 0.5 * (f1b[:, 0] + f1b[:, 4097])
    biasf[0, 0:512] = bf1
    biasf[1, 0:512] = _beta(p['bnf2'])
    m3, v3 = [np.asarray(t, np.float32) for t in p['bnf3']]
    rs = 1.0 / np.sqrt(v3 + EPS)
    biasf[2, 0:1000] = rs
    biasf[3, 0:1000] = -m3 * rs

    bf = ml_dtypes.bfloat16
    f8 = ml_dtypes.float8_e4m3
    return {
        "wt0": wt0.astype(bf), "wc": wc.astype(bf), "wf1": wf1.astype(f8),
        "wpx": wpx.astype(f8), "wf2": wf2.astype(f8), "wf3": wf3.astype(f8),
        "bias": bias, "biasf": biasf,
    }


def prepare_xim(xc):
    """Per-core im2col with bf16 hi/lo split: -> [54, 2, 8, 1024] bf16."""
    xp = np.pad(np.asarray(xc, np.float32), ((0, 0), (0, 0), (1, 1), (1, 1)))
    xim = np.empty([27, 16, 1024], np.float32)
    for c in range(3):
        for kh in range(3):
            for kw in range(3):
                xim[c * 9 + kh * 3 + kw] = \
                    xp[:, c, kh:kh + 32, kw:kw + 32].reshape(16, 1024)
    # exact 3-way bf16 split: hi+mid+lo reproduces all 24 fp32 mantissa bits
    hi = xim.astype(ml_dtypes.bfloat16)
    r1 = xim - hi.astype(np.float32)
    mid = r1.astype(ml_dtypes.bfloat16)
    lo = (r1 - mid.astype(np.float32)).astype(ml_dtypes.bfloat16)
    out = np.concatenate([hi, mid, lo], axis=0)      # [81, 16, 1024]
    return out.reshape(81, 2, 8, 1024)


def make_in_maps(x, params):
    shared = prepare_weights(params)
    x = np.asarray(x, np.float32)
    in_maps = []
    for ci in range(NCORES):
        m = dict(shared)
        m["xim"] = prepare_xim(x[ci * NPC:(ci + 1) * NPC])
        in_maps.append(m)
    return in_maps


def kernel(x, params):
    in_maps = make_in_maps(x, params)
    nc = build_nc()
    res = run_bass_kernel_spmd(nc, in_maps, core_ids=list(range(NCORES)))
    out = np.concatenate([np.asarray(res.results[i]["out"], np.float32)
                          for i in range(NCORES)], axis=0)
    return out


# revision 40
# speedup vs baseline: 1.4937x; 1.0861x over previous
"""Trainium2 Bass kernel for the binary CNN (CNV/BNN) forward pass.

Strategy
--------
Pure data parallel: batch 128 -> 16 images per NeuronCore x 8 cores.

Math transformations (all exact, validated against the jax reference):
  * sign(htanh(bn(y))) == sign(y + beta) with beta = b/s - m, since the BN
    scale s = g*rsqrt(v+eps) > 0 always.  htanh and the BN multiply never
    need to be materialized on the trunk.
  * sign is monotone, so sign(maxpool(u)) == maxpool(sign(u)); signs are
    computed directly from PSUM (fused bias via the ScalarE Sign activation)
    and pooling runs on +-1 bf16 values.
  * maj3_conv (majority-of-3 popcount conv) uses the identity, exact for all
    zero-padding cases that occur:
        sign(w0x0 + w1x1 + w2x2) = (w0x0 + w1x1 + w2x2 - (w0w1w2)(x0x1x2))/2
    so  maj3_conv(x, w) = 0.5*conv3x3(xb, wb)
                          - 0.5*conv3x1_vertical(Q, W3)
    where Q = horizontal triple products of the padded activations and
    W3[f,c,kh] = prod_kw wb[f,c,kh,kw].  Same identity for fcmaj3.
  * The whole binary trunk runs in bf16: +-1 and +-0.5 are exact, popcount
    partial sums accumulate in fp32 PSUM, and bf16 rounding of (u + beta)
    preserves sign.

Layouts: activations [C(part), N, H, W]; for C=64 layers the batch is split
across partition halves (p = c + 64*nh) and the two halves run as
concurrent tile_position row/col tiles of the 128x128 PE array.
"""

import os
import numpy as np
import ml_dtypes

import concourse.bass as bass
import concourse.tile as tile
from concourse import bacc, mybir
from concourse.bass_utils import run_bass_kernel_spmd
from concourse.masks import make_identity

F32 = mybir.dt.float32
BF16 = mybir.dt.bfloat16
FP8 = mybir.dt.float8e4
AX = mybir.AxisListType
ALU = mybir.AluOpType
ACT = mybir.ActivationFunctionType

EPS = 1e-5
NCORES = 8
NPC = 16  # images per core

# conv-weight table free-dim offsets inside the packed "wc" tensor
OFF_W1 = 0            # 12 taps x 64
OFF_W2 = 768          # 12 taps x 128
OFF_W3 = 768 + 1536   # 9 taps x 128
OFF_W4 = OFF_W3 + 1152  # (9 taps x 2 mb) x 128
OFF_W5 = OFF_W4 + 2304  # (2 kb x 9 taps x 2 mb) x 128
WC_COLS = OFF_W5 + 4608


def build_nc():
    nc = bacc.Bacc()

    xim_d = nc.dram_tensor("xim", [81, 2, 8, 1024], BF16, kind="ExternalInput")
    wt0_d = nc.dram_tensor("wt0", [81, 64], BF16, kind="ExternalInput")
    wc_d = nc.dram_tensor("wc", [128, WC_COLS], BF16, kind="ExternalInput")
    wf1_d = nc.dram_tensor("wf1", [32, 128, 512], FP8, kind="ExternalInput")
    wpx_d = nc.dram_tensor("wpx", [11, 128, 512], FP8, kind="ExternalInput")
    wf2_d = nc.dram_tensor("wf2", [4, 128, 512], FP8, kind="ExternalInput")
    wf3_d = nc.dram_tensor("wf3", [4, 128, 1024], FP8, kind="ExternalInput")
    bias_d = nc.dram_tensor("bias", [128, 8], F32, kind="ExternalInput")
    biasf_d = nc.dram_tensor("biasf", [4, 1024], F32, kind="ExternalInput")
    out_d = nc.dram_tensor("out", [16, 1000], F32, kind="ExternalOutput")
    a6d = nc.dram_tensor("a6d", [2, 128, 256], BF16)

    with tile.TileContext(nc) as tc:
        emit(nc, tc, xim_d, wt0_d, wc_d, wf1_d, wpx_d, wf2_d, wf3_d,
             bias_d, biasf_d, out_d, a6d)
    nc.compile()
    return nc


def emit(nc, tc, xim_d, wt0_d, wc_d, wf1_d, wpx_d, wf2_d, wf3_d,
         bias_d, biasf_d, out_d, a6d):
    from contextlib import ExitStack
    ctx = ExitStack()
    with ctx:
        wpool = ctx.enter_context(tc.tile_pool(name="wpool", bufs=1))
        acts = ctx.enter_context(tc.tile_pool(name="acts", bufs=1))
        ximp = ctx.enter_context(tc.tile_pool(name="ximp", bufs=3))
        qtmp = ctx.enter_context(tc.tile_pool(name="qtmp", bufs=1))
        cps = ctx.enter_context(tc.tile_pool(name="cps", bufs=4, space="PSUM"))
        fps = ctx.enter_context(tc.tile_pool(name="fps", bufs=2, space="PSUM"))
        tps = ctx.enter_context(tc.tile_pool(name="tps", bufs=2, space="PSUM"))

        # ---------------- weights needed by L0 (issue first) ----------------
        wt0 = wpool.tile([81, 64], BF16, name="wt0s")
        nc.sync.dma_start(out=wt0, in_=wt0_d[:])
        bias = wpool.tile([128, 8], F32, name="biass")
        nc.sync.dma_start(out=bias, in_=bias_d[:])
        # PE warm-up: ~4.5us of dense matmuls so the HAM clock-gate opens
        # before L0 (and stays open).  Results are discarded.
        wu = wpool.tile([128, 512], BF16, name="wu")
        nc.vector.memset(wu, 1.0)
        wups = fps.tile([16, 512], F32, name="wups", tag="fc")
        for _ in range(22):
            nc.tensor.matmul(out=wups, lhsT=wu[:, 0:16], rhs=wu,
                             start=True, stop=True, skip_group_check=True)

        # ---------------- persistent activation buffers ----------------
        xpad1 = acts.tile([128, 8, 34, 34], BF16, name="xpad1")
        q1 = acts.tile([128, 8, 34, 32], BF16, name="q1")
        s1 = acts.tile([128, 8, 32, 32], BF16, name="s1")
        p1h = acts.tile([128, 8, 16, 32], BF16, name="p1h")
        xpad2 = acts.tile([128, 8, 18, 18], BF16, name="xpad2")
        q2 = acts.tile([128, 8, 18, 16], BF16, name="q2")
        xpad3 = acts.tile([128, 16, 18, 18], BF16, name="xpad3")
        s3 = acts.tile([128, 16, 16, 16], BF16, name="s3")
        p3h = acts.tile([128, 16, 8, 16], BF16, name="p3h")
        xpad4 = acts.tile([128, 16, 10, 10], BF16, name="xpad4")
        a5 = [acts.tile([128, 16, 10, 10], BF16, name=f"a5_{i}") for i in (0, 1)]
        s5 = [acts.tile([128, 16, 8, 8], BF16, name=f"s5_{i}") for i in (0, 1)]
        p5h = [acts.tile([128, 16, 4, 8], BF16, name=f"p5h_{i}") for i in (0, 1)]
        a6 = [acts.tile([128, 16, 16], BF16, name=f"a6_{i}") for i in (0, 1)]

        xbuf = acts.tile([16, 4098], BF16, name="xbuf")
        px = acts.tile([16, 1408], BF16, name="px")
        pxt = acts.tile([128, 176], BF16, name="pxt")
        af2f = acts.tile([16, 512], BF16, name="af2f")
        af3f = acts.tile([16, 512], BF16, name="af3f")
        af2t = acts.tile([128, 64], BF16, name="af2t")
        af3t = acts.tile([128, 64], BF16, name="af3t")
        yb = acts.tile([16, 1024], F32, name="yb")
        negmx = acts.tile([16, 1], F32, name="negmx")
        sm = acts.tile([16, 1], F32, name="sm")
        lse = acts.tile([16, 1], F32, name="lse")
        outsb = acts.tile([16, 1000], F32, name="outsb")

        # zero only the padding halos (interiors are overwritten by drains)
        def halo_memset(eng, buf, H, W):
            eng.memset(buf[:, :, 0, :], 0.0)          # top row
            eng.memset(buf[:, :, H - 1, :], 0.0)      # bottom row
            eng.memset(buf[:, :, 1:H - 1, 0], 0.0)    # left col
            eng.memset(buf[:, :, 1:H - 1, W - 1], 0.0)  # right col
        halo_memset(nc.gpsimd, xpad1, 34, 34)
        halo_memset(nc.vector, xpad2, 18, 18)
        halo_memset(nc.gpsimd, xpad3, 18, 18)
        halo_memset(nc.vector, xpad4, 10, 10)
        halo_memset(nc.gpsimd, a5[0], 10, 10)
        halo_memset(nc.gpsimd, a5[1], 10, 10)
        nc.vector.memset(xbuf[:, 0:1], -1.0)
        nc.vector.memset(xbuf[:, 4097:4098], -1.0)
        nc.vector.memset(px[:, 1366:1408], 0.0)

        # ------- remaining persistent weights (scalar queue; sync belongs to ximt) -------
        wc = wpool.tile([128, WC_COLS], BF16, name="wcs")
        nc.scalar.dma_start(out=wc, in_=wc_d[:])
        # bias rows broadcast to 16 partitions (DVE can't take step-0 APs)
        # layout [16, 3072]: bf1(512) | bf2(512) | rs(1024) | mrs(1024)
        biasf = wpool.tile([16, 3072], F32, name="biasfs")
        bf_ap = biasf_d[:]

        def _bcast_row(dst, row, n):
            nc.scalar.dma_start(out=dst, in_=bass.AP(
                tensor=bf_ap.tensor, offset=bf_ap.offset + row * 1024,
                ap=[[0, 16], [1, n]]))
        _bcast_row(biasf[:, 0:512], 0, 512)
        _bcast_row(biasf[:, 512:1024], 1, 512)
        _bcast_row(biasf[:, 1024:2048], 2, 1024)
        _bcast_row(biasf[:, 2048:3072], 3, 1024)
        ident = wpool.tile([16, 16], BF16, name="idents")
        make_identity(nc, ident)

        # ================ L0: conv 3->64 on real-valued x ================
        # K=27 im2col rows; two batch halves run as concurrent col tiles.
        # ===== L0 (conv 3->64, K=81 exact bf16 split) interleaved with =====
        # ===== L1 (maj3 64->64): L1 chunk n8-1 fills L0's DMA-wait gaps =====
        def l0_chunk(n8, hh):
            ximt = ximp.tile([81, 2, 512], BF16, name="ximt")
            nc.sync.dma_start(out=ximt, in_=xim_d[:][:, :, n8, hh * 512:(hh + 1) * 512])
            ps = cps.tile([128, 512], F32, name="ps0", tag="cpsum")
            nc.tensor.matmul(out=ps[0:64], lhsT=wt0, rhs=ximt[:, 0],
                             start=True, stop=True, tile_position=(0, 0))
            nc.tensor.matmul(out=ps[64:128], lhsT=wt0, rhs=ximt[:, 1],
                             start=True, stop=True, tile_position=(0, 64),
                             skip_group_check=True)
            psv = ps.rearrange("p (h w) -> p h w", h=16)
            nc.scalar.activation(
                out=xpad1[:, n8, 1 + hh * 16:17 + hh * 16, 1:33],
                in_=psv, func=ACT.Sign, bias=bias[:, 0:1])

        def q1_chunk(n8):
            qt = qtmp.tile([128, 34, 32], BF16, name="qt1", tag="qt1")
            nc.vector.tensor_mul(qt, xpad1[:, n8, :, 0:32], xpad1[:, n8, :, 1:33])
            nc.vector.tensor_mul(q1[:, n8], qt, xpad1[:, n8, :, 2:34])

        def l1_chunk(n8, hh):
            ps = cps.tile([128, 512], F32, name="ps1", tag="cpsum")
            for t in range(12):
                for nh in (0, 1):
                    p0 = 64 * nh
                    if t < 9:
                        kh, kw = t // 3, t % 3
                        rhs = xpad1[p0:p0 + 64, n8,
                                    kh + hh * 16:kh + hh * 16 + 16, kw:kw + 32]
                    else:
                        kh = t - 9
                        rhs = q1[p0:p0 + 64, n8,
                                 kh + hh * 16:kh + hh * 16 + 16, :]
                    nc.tensor.matmul(
                        out=ps[p0:p0 + 64], lhsT=wc[p0:p0 + 64, OFF_W1 + t * 64:OFF_W1 + (t + 1) * 64],
                        rhs=rhs, start=(t == 0), stop=(t == 11),
                        tile_position=(p0, p0),
                        skip_group_check=(nh == 1))
            psv = ps.rearrange("p (h w) -> p h w", h=16)
            nc.scalar.activation(out=s1[:, n8, hh * 16:hh * 16 + 16, :],
                                 in_=psv, func=ACT.Sign, bias=bias[:, 1:2])

        for n8 in range(8):
            for hh in range(2):
                l0_chunk(n8, hh)
            q1_chunk(n8)
            if n8 >= 1:
                for hh in range(2):
                    l1_chunk(n8 - 1, hh)
        for hh in range(2):
            l1_chunk(7, hh)

        # maxpool 2x2 on signs -> write interior of xpad2 (split for pipelining)
        s1r = s1.rearrange("p n (h2 pr) w -> p n h2 pr w", pr=2)
        p1r = p1h.rearrange("p n h (w2 pr) -> p n h w2 pr", pr=2)
        for j in range(4):
            sl = slice(2 * j, 2 * j + 2)
            nc.vector.tensor_max(p1h[:, sl], s1r[:, sl, :, 0, :], s1r[:, sl, :, 1, :])
            nc.vector.tensor_max(xpad2[:, sl, 1:17, 1:17],
                                 p1r[:, sl, :, :, 0], p1r[:, sl, :, :, 1])
            for n8 in (2 * j, 2 * j + 1):
                qt = qtmp.tile([128, 18, 16], BF16, name="qt2", tag="qt2")
                nc.vector.tensor_mul(qt, xpad2[:, n8, :, 0:16], xpad2[:, n8, :, 1:17])
                nc.vector.tensor_mul(q2[:, n8], qt, xpad2[:, n8, :, 2:18])

        # ================ L2: maj3 64->128 @16x16 ================
        # batch halves as row tiles -> two separate PSUM banks.
        for j in range(4):
            psA = cps.tile([128, 512], F32, name="ps2a", tag="cpsum")
            psB = cps.tile([128, 512], F32, name="ps2b", tag="cpsum")
            for t in range(12):
                for nh, pst in ((0, psA), (1, psB)):
                    p0 = 64 * nh
                    if t < 9:
                        kh, kw = t // 3, t % 3
                        rhs = xpad2[p0:p0 + 64, 2 * j:2 * j + 2,
                                    kh:kh + 16, kw:kw + 16]
                    else:
                        kh = t - 9
                        rhs = q2[p0:p0 + 64, 2 * j:2 * j + 2, kh:kh + 16, :]
                    nc.tensor.matmul(
                        out=pst, lhsT=wc[p0:p0 + 64, OFF_W2 + t * 128:OFF_W2 + (t + 1) * 128],
                        rhs=rhs, start=(t == 0), stop=(t == 11),
                        tile_position=(p0, 0))
            for nh, pst in ((0, psA), (1, psB)):
                psv = pst.rearrange("p (n h w) -> p n h w", n=2, h=16)
                nc.scalar.activation(
                    out=xpad3[:, 8 * nh + 2 * j:8 * nh + 2 * j + 2, 1:17, 1:17],
                    in_=psv, func=ACT.Sign, bias=bias[:, 2:3])

        wf1s = wpool.tile([128, 32, 512], FP8, name="wf1s")
        nc.sync.dma_start(out=wf1s, in_=wf1_d[:].rearrange("k c f -> c k f"))
        wpxs = wpool.tile([128, 11, 512], FP8, name="wpxs")
        nc.sync.dma_start(out=wpxs, in_=wpx_d[:].rearrange("k c f -> c k f"))
        wf2s = wpool.tile([128, 4, 512], FP8, name="wf2s")
        nc.sync.dma_start(out=wf2s, in_=wf2_d[:].rearrange("k c f -> c k f"))
        wf3s = wpool.tile([128, 4, 1024], FP8, name="wf3s")
        nc.sync.dma_start(out=wf3s, in_=wf3_d[:].rearrange("k c f -> c k f"))
        # ================ L3: bin_conv 128->128 @16x16 + pool ================
        for j in range(8):
            ps = cps.tile([128, 512], F32, name="ps3", tag="cpsum")
            for t in range(9):
                kh, kw = t // 3, t % 3
                nc.tensor.matmul(
                    out=ps, lhsT=wc[:, OFF_W3 + t * 128:OFF_W3 + (t + 1) * 128],
                    rhs=xpad3[:, 2 * j:2 * j + 2, kh:kh + 16, kw:kw + 16],
                    start=(t == 0), stop=(t == 8))
            psv = ps.rearrange("p (n h w) -> p n h w", n=2, h=16)
            nc.scalar.activation(out=s3[:, 2 * j:2 * j + 2], in_=psv,
                                 func=ACT.Sign, bias=bias[:, 3:4])

        s3r = s3.rearrange("p n (h2 pr) w -> p n h2 pr w", pr=2)
        p3r = p3h.rearrange("p n h (w2 pr) -> p n h w2 pr", pr=2)
        for j in range(2):
            sl = slice(8 * j, 8 * j + 8)
            nc.vector.tensor_max(p3h[:, sl], s3r[:, sl, :, 0, :], s3r[:, sl, :, 1, :])
            nc.vector.tensor_max(xpad4[:, sl, 1:9, 1:9],
                                 p3r[:, sl, :, :, 0], p3r[:, sl, :, :, 1])

        # ================ L4: bin_conv 128->256 @8x8 ================
        for mb in range(2):
            for c8 in range(2):
                ps = cps.tile([128, 512], F32, name="ps4", tag="cpsum")
                for t in range(9):
                    kh, kw = t // 3, t % 3
                    nc.tensor.matmul(
                        out=ps,
                        lhsT=wc[:, OFF_W4 + (t * 2 + mb) * 128:OFF_W4 + (t * 2 + mb + 1) * 128],
                        rhs=xpad4[:, c8 * 8:c8 * 8 + 8, kh:kh + 8, kw:kw + 8],
                        start=(t == 0), stop=(t == 8))
                psv = ps.rearrange("p (n h w) -> p n h w", n=8, h=8)
                nc.scalar.activation(
                    out=a5[mb][:, c8 * 8:c8 * 8 + 8, 1:9, 1:9], in_=psv,
                    func=ACT.Sign, bias=bias[:, 4 + mb:5 + mb])

        # ================ L5: bin_conv 256->256 @8x8 + pool ================
        for mb in range(2):
            for c8 in range(2):
                ps = cps.tile([128, 512], F32, name="ps5", tag="cpsum")
                for kb in range(2):
                    for t in range(9):
                        kh, kw = t // 3, t % 3
                        nc.tensor.matmul(
                            out=ps,
                            lhsT=wc[:, OFF_W5 + ((kb * 9 + t) * 2 + mb) * 128:
                                    OFF_W5 + ((kb * 9 + t) * 2 + mb + 1) * 128],
                            rhs=a5[kb][:, c8 * 8:c8 * 8 + 8, kh:kh + 8, kw:kw + 8],
                            start=(kb == 0 and t == 0), stop=(kb == 1 and t == 8))
                psv = ps.rearrange("p (n h w) -> p n h w", n=8, h=8)
                nc.scalar.activation(out=s5[mb][:, c8 * 8:c8 * 8 + 8], in_=psv,
                                     func=ACT.Sign, bias=bias[:, 6 + mb:7 + mb])
        # pool + gather a6 -> X[n, d] rows (d = c*16 + hw), -1 pads at ends.
        # Gathers go on the Scalar DMA queue so they don't head-of-line
        # block the FC weight stream on the Sync queue.
        for mb in range(2):
            s5r = s5[mb].rearrange("p n (h2 pr) w -> p n h2 pr w", pr=2)
            nc.vector.tensor_max(p5h[mb], s5r[:, :, :, 0, :], s5r[:, :, :, 1, :])
            p5r = p5h[mb].rearrange("p n h (w2 pr) -> p n h w2 pr", pr=2)
            a6v = a6[mb].rearrange("p n (h w) -> p n h w", h=4)
            nc.vector.tensor_max(a6v, p5r[:, :, :, :, 0], p5r[:, :, :, :, 1])
            nc.scalar.dma_start(out=a6d[:][mb], in_=a6[mb])
            a6d_ap = a6d[:]
            nc.scalar.dma_start(
                out=xbuf[:, 1 + mb * 2048:1 + (mb + 1) * 2048],
                in_=bass.AP(tensor=a6d_ap.tensor,
                            offset=a6d_ap.offset + mb * 32768,
                            ap=[[16, 16], [256, 128], [1, 16]]))

        # ================ FC head ================

        # Px = triple products of consecutive padded features, split per
        # a6-half so the first transposes can start before cb=1 lands.
        xr3 = xbuf.rearrange("p (g k) -> p g k", k=3)
        pxt1 = outsb.bitcast(BF16)[:, 0:1366]
        nc.vector.tensor_mul(pxt1[:, 0:682], xr3[:, 0:682, 0], xr3[:, 0:682, 1])
        nc.vector.tensor_mul(px[:, 0:682], pxt1[:, 0:682], xr3[:, 0:682, 2])
        nc.vector.tensor_mul(pxt1[:, 682:1366], xr3[:, 682:1366, 0], xr3[:, 682:1366, 1])
        nc.vector.tensor_mul(px[:, 682:1366], pxt1[:, 682:1366], xr3[:, 682:1366, 2])

        # transpose Px -> [g, n] blocks for use as matmul lhsT
        for gb in range(11):
            txp = tps.tile([128, 16], BF16, name="txp", tag="tx")
            nc.tensor.transpose(txp, px[:, gb * 128:(gb + 1) * 128], ident)
            nc.scalar.copy(pxt[:, gb * 16:(gb + 1) * 16], txp)

        # FC1: out[n, f] accumulated over 32 feature K-groups + 11 Px groups
        psf = fps.tile([16, 512], F32, name="psf1", tag="fc")
        nmm = 43
        i = 0
        for kg in range(32):
            cb, hw = kg // 16, kg % 16
            nc.tensor.matmul(out=psf, lhsT=a6[cb][:, :, hw], rhs=wf1s[:, kg],
                             start=(i == 0), stop=(i == nmm - 1))
            i += 1
        # keep the PE clock-gate open while the Px chain lands
        wudum = fps.tile([16, 512], F32, name="wudum", tag="fc")
        for _ in range(16):
            nc.tensor.matmul(out=wudum, lhsT=wu[:, 0:16], rhs=wu,
                             start=True, stop=True, skip_group_check=True)
        for gb in range(11):
            nc.tensor.matmul(out=psf, lhsT=pxt[:, gb * 16:(gb + 1) * 16],
                             rhs=wpxs[:, gb],
                             start=(i == 0), stop=(i == nmm - 1))
            i += 1
        # bias + sign (beta_f1 along the free dim -> DVE add then ACT sign)
        nc.vector.tensor_add(af2f, psf, biasf[:, 0:512])
        nc.scalar.activation(out=af2f, in_=af2f, func=ACT.Sign, bias=0.0)
        for fb in range(4):
            txp = tps.tile([128, 16], BF16, name="txa", tag="tx")
            nc.tensor.transpose(txp, af2f[:, fb * 128:(fb + 1) * 128], ident)
            nc.scalar.copy(af2t[:, fb * 16:(fb + 1) * 16], txp)

        # FC2
        psf2 = fps.tile([16, 512], F32, name="psf2", tag="fc")
        for kb in range(4):
            nc.tensor.matmul(out=psf2, lhsT=af2t[:, kb * 16:(kb + 1) * 16],
                             rhs=wf2s[:, kb], start=(kb == 0), stop=(kb == 3))
        nc.vector.tensor_add(af3f, psf2, biasf[:, 512:1024])
        nc.scalar.activation(out=af3f, in_=af3f, func=ACT.Sign, bias=0.0)
        for fb in range(4):
            txp = tps.tile([128, 16], BF16, name="txb", tag="tx")
            nc.tensor.transpose(txp, af3f[:, fb * 128:(fb + 1) * 128], ident)
            nc.scalar.copy(af3t[:, fb * 16:(fb + 1) * 16], txp)

        # FC3 (+ affine-only BN)
        psh = [fps.tile([16, 512], F32, name=f"psh{i}", tag="fc") for i in (0, 1)]
        for kb in range(4):
            for half in range(2):
                nc.tensor.matmul(out=psh[half], lhsT=af3t[:, kb * 16:(kb + 1) * 16],
                                 rhs=wf3s[:, kb, half * 512:(half + 1) * 512],
                                 start=(kb == 0), stop=(kb == 3))
        for half in range(2):
            nc.vector.tensor_mul(yb[:, half * 512:(half + 1) * 512], psh[half],
                                 biasf[:, 1024 + half * 512:1024 + (half + 1) * 512])
        nc.vector.tensor_add(yb[:, 0:1000], yb[:, 0:1000], biasf[:, 2048:3048])

        # log-softmax over classes
        nc.vector.tensor_reduce(out=negmx, in_=yb[:, 0:1000], axis=AX.X,
                                op=ALU.max, negate=True)
        nc.scalar.activation(out=outsb, in_=yb[:, 0:1000], func=ACT.Exp, bias=negmx)
        nc.vector.tensor_reduce(out=sm, in_=outsb, axis=AX.X, op=ALU.add)
        nc.scalar.activation(out=lse, in_=sm, func=ACT.Ln, bias=0.0)
        nc.vector.tensor_scalar(out=outsb, in0=yb[:, 0:1000], scalar1=negmx,
                                scalar2=lse, op0=ALU.add, op1=ALU.subtract)
        nc.gpsimd.dma_start(out=out_d[:], in_=outsb)


# ======================= host-side preparation =======================

def _sgn(a):
    return np.sign(np.asarray(a, np.float32)).astype(np.float32)


def _beta(bn):
    g, b, m, v = [np.asarray(t, np.float32) for t in bn]
    s = g / np.sqrt(v + EPS)
    return (b / s - m).astype(np.float32)


def prepare_weights(params):
    p = params
    w1b = _sgn(p['w1']); w2b = _sgn(p['w2']); w3b = _sgn(p['w3'])
    w4b = _sgn(p['w4']); w5b = _sgn(p['w5'])
    f1b = _sgn(p['fc1']); f2b = _sgn(p['fc2']); f3b = _sgn(p['fc3'])

    wt0 = np.zeros([81, 64], np.float32)
    w0b = _sgn(p['w0'])
    for c in range(3):
        for kh in range(3):
            for kw in range(3):
                wt0[c * 9 + kh * 3 + kw] = w0b[:, c, kh, kw]
    wt0[27:54] = wt0[0:27]
    wt0[54:81] = wt0[0:27]

    wc = np.zeros([128, WC_COLS], np.float32)

    def fill_maj(off, wb, F):
        W3 = wb[:, :, :, 0] * wb[:, :, :, 1] * wb[:, :, :, 2]
        for t in range(12):
            if t < 9:
                kh, kw = t // 3, t % 3
                blk = 0.5 * wb[:, :, kh, kw].T      # [c, F]
            else:
                blk = -0.5 * W3[:, :, t - 9].T
            wc[0:64, off + t * F: off + (t + 1) * F] = blk
            wc[64:128, off + t * F: off + (t + 1) * F] = blk

    fill_maj(OFF_W1, w1b, 64)
    fill_maj(OFF_W2, w2b, 128)
    for t in range(9):
        kh, kw = t // 3, t % 3
        wc[:, OFF_W3 + t * 128: OFF_W3 + (t + 1) * 128] = w3b[:, :, kh, kw].T
        for mb in range(2):
            wc[:, OFF_W4 + (t * 2 + mb) * 128: OFF_W4 + (t * 2 + mb + 1) * 128] = \
                w4b[mb * 128:(mb + 1) * 128, :, kh, kw].T
            for kb in range(2):
                wc[:, OFF_W5 + ((kb * 9 + t) * 2 + mb) * 128:
                   OFF_W5 + ((kb * 9 + t) * 2 + mb + 1) * 128] = \
                    w5b[mb * 128:(mb + 1) * 128, kb * 128:(kb + 1) * 128, kh, kw].T

    # FC1 main: wf1[kg=(cb*16+hw)][c_local, f] = 0.5*f1b[f, (cb*128+c)*16+hw+1]
    core = f1b[:, 1:4097].reshape(512, 256, 16)           # [f, c, hw]
    wf1 = np.zeros([32, 128, 512], np.float32)
    for cb in range(2):
        for hw in range(16):
            wf1[cb * 16 + hw] = 0.5 * core[:, cb * 128:(cb + 1) * 128, hw].T
    # FC1 Px term
    W3f = f1b[:, 0::3] * f1b[:, 1::3] * f1b[:, 2::3]      # [512, 1366]
    wpx = np.zeros([11, 128, 512], np.float32)
    W3p = np.zeros([512, 1408], np.float32)
    W3p[:, 0:1366] = -0.5 * W3f
    for gb in range(11):
        wpx[gb] = W3p[:, gb * 128:(gb + 1) * 128].T
    wf2 = np.zeros([4, 128, 512], np.float32)
    for kb in range(4):
        wf2[kb] = f2b[:, kb * 128:(kb + 1) * 128].T
    f3p = np.zeros([1024, 512], np.float32)
    f3p[0:1000] = f3b
    wf3 = np.zeros([4, 128, 1024], np.float32)
    for kb in range(4):
        wf3[kb] = f3p[:, kb * 128:(kb + 1) * 128].T

    bias = np.zeros([128, 8], np.float32)
    b0 = _beta(p['bn0']); b1 = _beta(p['bn1'])
    bias[:, 0] = np.concatenate([b0, b0]); bias[:, 1] = np.concatenate([b1, b1])
    bias[:, 2] = _beta(p['bn2']); bias[:, 3] = _beta(p['bn3'])
    b4 = _beta(p['bn4']); b5 = _beta(p['bn5'])
    bias[:, 4] = b4[0:128]; bias[:, 5] = b4[128:256]
    bias[:, 6] = b5[0:128]; bias[:, 7] = b5[128:256]

    biasf = np.zeros([4, 1024], np.float32)
    bf1 = _beta(p['bnf1']) - 0.5 * (f1b[:, 0] + f1b[:, 4097])
    biasf[0, 0:512] = bf1
    biasf[1, 0:512] = _beta(p['bnf2'])
    m3, v3 = [np.asarray(t, np.float32) for t in p['bnf3']]
    rs = 1.0 / np.sqrt(v3 + EPS)
    biasf[2, 0:1000] = rs
    biasf[3, 0:1000] = -m3 * rs

    bf = ml_dtypes.bfloat16
    f8 = ml_dtypes.float8_e4m3
    return {
        "wt0": wt0.astype(bf), "wc": wc.astype(bf), "wf1": wf1.astype(f8),
        "wpx": wpx.astype(f8), "wf2": wf2.astype(f8), "wf3": wf3.astype(f8),
        "bias": bias, "biasf": biasf,
    }


def prepare_xim(xc):
    """Per-core im2col with bf16 hi/lo split: -> [54, 2, 8, 1024] bf16."""
    xp = np.pad(np.asarray(xc, np.float32), ((0, 0), (0, 0), (1, 1), (1, 1)))
    xim = np.empty([27, 16, 1024], np.float32)
    for c in range(3):
        for kh in range(3):
            for kw in range(3):
                xim[c * 9 + kh * 3 + kw] = \
                    xp[:, c, kh:kh + 32, kw:kw + 32].reshape(16, 1024)
    # exact 3-way bf16 split: hi+mid+lo reproduces all 24 fp32 mantissa bits
    hi = xim.astype(ml_dtypes.bfloat16)
    r1 = xim - hi.astype(np.float32)
    mid = r1.astype(ml_dtypes.bfloat16)
    lo = (r1 - mid.astype(np.float32)).astype(ml_dtypes.bfloat16)
    out = np.concatenate([hi, mid, lo], axis=0)      # [81, 16, 1024]
    return out.reshape(81, 2, 8, 1024)


def make_in_maps(x, params):
    shared = prepare_weights(params)
    x = np.asarray(x, np.float32)
    in_maps = []
    for ci in range(NCORES):
        m = dict(shared)
        m["xim"] = prepare_xim(x[ci * NPC:(ci + 1) * NPC])
        in_maps.append(m)
    return in_maps


def kernel(x, params):
    in_maps = make_in_maps(x, params)
    nc = build_nc()
    res = run_bass_kernel_spmd(nc, in_maps, core_ids=list(range(NCORES)))
    out = np.concatenate([np.asarray(res.results[i]["out"], np.float32)
                          for i in range(NCORES)], axis=0)
    return out
